# revision 11
# baseline (speedup 1.0000x reference)
"""GPT-NeoX attention (s=2048, b=1, h=2048, nh=16, hd=128, rot=32) on 8 NeuronCores.

Sharding: tensor-parallel over heads (2 heads per core), row-parallel dense
with host-side partial reduction.

Per core: the QKV projections run as fp8e4 DoubleRow matmuls with 3-term
residual compensation (X@W ~ X8@W8 + Xe@W8 + X8@We, each plane pre-scaled
into e4m3's dynamic range) - 0.75x the bf16 PE cost at ~0.1% error. The
attention core (scores, softmax, context) and the dense matmul run in fp16.
Scores use a transposed layout; context is computed in natural layout with a
ones-column so the softmax denominator falls out of the matmul; a per-row
reciprocal scale normalizes on the vector engine; context transposes back on
the PE for the dense slice. Dense output of chunk i is deferred into chunk
i+1's window so the scalar engine's softmax-exp latency never stalls the PE.
"""

import math
import numpy as np
import ml_dtypes

S = 2048
HID = 2048
NH = 16
D = 128
ROT = 32
NCORES = 8
HPC = 2  # heads per core
CHUNK = 512
NKT = HID // 128  # 16 contraction tiles
NKP = NKT // 2    # 8 DoubleRow k-tile pairs
NCH = S // CHUNK  # 4 i-chunks
NST = S // 128    # 16 s-tiles
NORM = 1.0 / math.sqrt(D)
MASK_NEG = -1000.0 / NORM  # -1000 after the exp scale; exp() underflows to 0

# fp8 plane scales: place values in e4m3's sweet spot (max 240, min normal 2^-6)
SX = 32.0      # hidden-state planes
SWQK = 2048.0  # Wq/Wk planes -> q,k psum at 2^16, descaled in the PSUM->SBUF copy
SWV = 128.0    # Wv planes    -> v psum at 2^12 = 4096*v, cancelled by the ones column
QK_DESCALE = 1.0 / (SX * SWQK)
VSCALE = SX * SWV  # 4096; vn holds 4096*(v+b); ones col = 4096 so cn = ctx

E4NP = ml_dtypes.float8_e4m3
F16NP = np.float16

_cache = {}


def _build_program():
    from concourse import bass, bacc, tile
    from concourse.bass import mybir

    f32 = mybir.dt.float32
    fp16 = mybir.dt.float16
    fp8 = mybir.dt.float8e4
    Exp = mybir.ActivationFunctionType.Exp
    Ident = mybir.ActivationFunctionType.Identity
    ADD = mybir.AluOpType.add
    MULT = mybir.AluOpType.mult
    DR = mybir.MatmulPerfMode.DoubleRow

    nc = bacc.Bacc()

    # all dram tensors laid out [128, free] with >=512B contiguous runs
    ht8_d = nc.dram_tensor("ht8", [128, NKT * S], fp8, kind="ExternalInput")
    hte_d = nc.dram_tensor("hte", [128, NKT * S], fp8, kind="ExternalInput")
    wq8_d = nc.dram_tensor("wq8", [128, NKT * HPC * D], fp8, kind="ExternalInput")
    wqe_d = nc.dram_tensor("wqe", [128, NKT * HPC * D], fp8, kind="ExternalInput")
    wk8_d = nc.dram_tensor("wk8", [128, NKT * HPC * D], fp8, kind="ExternalInput")
    wke_d = nc.dram_tensor("wke", [128, NKT * HPC * D], fp8, kind="ExternalInput")
    wv8_d = nc.dram_tensor("wv8", [128, NKT * HPC * D], fp8, kind="ExternalInput")
    wve_d = nc.dram_tensor("wve", [128, NKT * HPC * D], fp8, kind="ExternalInput")
    wd_d = nc.dram_tensor("wd", [128, HPC * HID], fp16, kind="ExternalInput")
    cos_d = nc.dram_tensor("cosT", [ROT, S], fp16, kind="ExternalInput")
    sin_d = nc.dram_tensor("sinTeff", [ROT, S], fp16, kind="ExternalInput")
    mask_d = nc.dram_tensor("maskbias", [128, 128], fp16, kind="ExternalInput")
    ident_d = nc.dram_tensor("ident", [128, 128], fp16, kind="ExternalInput")
    bqk_d = nc.dram_tensor("bqk", [128, 4], f32, kind="ExternalInput")
    bvb_d = nc.dram_tensor("bvb", [128, HPC * D], f32, kind="ExternalInput")
    out_d = nc.dram_tensor("partial", [S, HID], fp16, kind="ExternalOutput")

    with tile.TileContext(nc) as tc:
        with (
            tc.tile_pool(name="persist", bufs=1) as pp,
            tc.tile_pool(name="probs", bufs=36) as prp,
            tc.tile_pool(name="rotu", bufs=2) as rop,
            tc.tile_pool(name="ctxn", bufs=6) as cnp,
            tc.tile_pool(name="rec", bufs=8) as rcp,
            tc.tile_pool(name="stage", bufs=3) as stp,
            tc.tile_pool(name="psA", bufs=2, space="PSUM") as psA,
            tc.tile_pool(name="psB", bufs=2, space="PSUM") as psB,
            tc.tile_pool(name="psC", bufs=2, space="PSUM") as psC,
            tc.tile_pool(name="psD", bufs=2, space="PSUM") as psD,
        ):
            pools = [psA, psB, psC, psD]

            # ---- persistent SBUF tiles ----
            ht8 = pp.tile([128, NKT, S], fp8, tag="ht8")
            hte = pp.tile([128, NKT, S], fp8, tag="hte")
            wq8 = pp.tile([128, NKT, HPC * D], fp8, tag="wq8")
            wqe = pp.tile([128, NKT, HPC * D], fp8, tag="wqe")
            wk8 = pp.tile([128, NKT, HPC * D], fp8, tag="wk8")
            wke = pp.tile([128, NKT, HPC * D], fp8, tag="wke")
            wv8 = pp.tile([128, NKT, HPC * D], fp8, tag="wv8")
            wve = pp.tile([128, NKT, HPC * D], fp8, tag="wve")
            wd = pp.tile([128, HPC, HID], fp16, tag="wd")
            cosT = pp.tile([ROT, S], fp16, tag="cos")
            sinT = pp.tile([ROT, S], fp16, tag="sin")
            maskb = pp.tile([128, 128], fp16, tag="mask")
            ident = pp.tile([128, 128], fp16, tag="ident")
            bqk = pp.tile([128, 4], f32, tag="bqk")
            bvb = pp.tile([128, HPC * D], f32, tag="bvb")
            qT = [pp.tile([128, S], fp16, tag=f"qT{h}", name=f"qT{h}") for h in range(HPC)]
            kT = [pp.tile([128, S], fp16, tag=f"kT{h}", name=f"kT{h}") for h in range(HPC)]
            # V natural layout (both heads) + ones column for the denominator
            vn = pp.tile([128, NST, HPC, D + 1], fp16, tag="vn")
            ctxT = [pp.tile([128, S], fp16, tag=f"ctxT{h}", name=f"ctxT{h}")
                    for h in range(HPC)]

            nc.vector.memset(vn[:, :, :, D:D + 1], VSCALE)

            # ---- input DMAs: the first chains' (main/xres) terms need only
            # wk8/wq8 + ht planes; the we planes (wres pass) can arrive late ----
            wk8_r = wk8_d[:].rearrange("p (k m) -> p k m", k=NKT)
            wq8_r = wq8_d[:].rearrange("p (k m) -> p k m", k=NKT)
            ht8_r = ht8_d[:].rearrange("p (k s) -> p k s", k=NKT)
            hte_r = hte_d[:].rearrange("p (k s) -> p k s", k=NKT)
            nc.sync.dma_start(wk8[:, 0:4, :], wk8_r[:, 0:4, :])
            nc.sync.dma_start(ht8[:, 0, :], ht8_r[:, 0, :])
            nc.sync.dma_start(hte[:, 0, :], hte_r[:, 0, :])
            nc.sync.dma_start(wq8[:, 0:4, :], wq8_r[:, 0:4, :])
            nc.sync.dma_start(ht8[:, 1, :], ht8_r[:, 1, :])
            nc.sync.dma_start(hte[:, 1, :], hte_r[:, 1, :])
            nc.sync.dma_start(wk8[:, 4:, :], wk8_r[:, 4:, :])
            nc.sync.dma_start(wq8[:, 4:, :], wq8_r[:, 4:, :])
            for k in range(2, NKT):
                nc.sync.dma_start(ht8[:, k, :], ht8_r[:, k, :])
                nc.sync.dma_start(hte[:, k, :], hte_r[:, k, :])
                if k == 4:
                    nc.sync.dma_start(wke[:], wke_d[:].rearrange("p (k m) -> p k m", k=NKT))
                if k == 6:
                    nc.sync.dma_start(wqe[:], wqe_d[:].rearrange("p (k m) -> p k m", k=NKT))
            nc.scalar.dma_start(cosT[:], cos_d[:])
            nc.scalar.dma_start(sinT[:], sin_d[:])
            nc.scalar.dma_start(maskb[:], mask_d[:])
            nc.scalar.dma_start(ident[:], ident_d[:])
            nc.scalar.dma_start(bqk[:], bqk_d[:])
            nc.scalar.dma_start(bvb[:], bvb_d[:])
            nc.sync.dma_start(wv8[:], wv8_d[:].rearrange("p (k m) -> p k m", k=NKT))
            nc.sync.dma_start(wve[:], wve_d[:].rearrange("p (k m) -> p k m", k=NKT))
            nc.sync.dma_start(wd[:], wd_d[:].rearrange("p (c o) -> p c o", c=HPC))

            def dr3_step(ps, j, w8, we, hcols, sl, first, last):
                # one k-pair step of a 3-term compensated chain
                kk = slice(2 * j, 2 * j + 2)
                nc.tensor.matmul(ps[:], w8[:, kk, hcols], ht8[:, kk, sl],
                                 start=first, stop=False, perf_mode=DR)
                nc.tensor.matmul(ps[:], w8[:, kk, hcols], hte[:, kk, sl],
                                 start=False, stop=False, perf_mode=DR)
                nc.tensor.matmul(ps[:], we[:, kk, hcols], ht8[:, kk, sl],
                                 start=False, stop=last, perf_mode=DR)

            def qk_proj(h, interleave):
                # qT/kT[h][d=128, s]; bias + 2^-16 descale in the PSUM->SBUF copy
                hcols = slice(h * D, (h + 1) * D)
                chains = []
                for ci in range(NCH):
                    sl = slice(ci * CHUNK, (ci + 1) * CHUNK)
                    for (w8, we, dst, bcol) in ((wk8, wke, kT, 2), (wq8, wqe, qT, 0)):
                        chains.append((w8, we, dst, bcol, sl))
                if interleave:
                    # j-major across all 8 chains, main+xres first (they only
                    # need the w8 planes): rides the ht DMA staircase; the
                    # wres pass runs once the we planes have landed
                    pss = [pools[c % 4].tile([128, CHUNK], f32, tag=f"ps{c % 4}",
                                             name=f"pss{c}")
                           for c in range(8)]
                    for j in range(NKP):
                        for c, (w8, we, dst, bcol, sl) in enumerate(chains):
                            kk = slice(2 * j, 2 * j + 2)
                            nc.tensor.matmul(pss[c][:], w8[:, kk, hcols],
                                             ht8[:, kk, sl], start=(j == 0),
                                             stop=False, perf_mode=DR)
                            nc.tensor.matmul(pss[c][:], w8[:, kk, hcols],
                                             hte[:, kk, sl], start=False,
                                             stop=False, perf_mode=DR)
                    for j in range(NKP):
                        for c, (w8, we, dst, bcol, sl) in enumerate(chains):
                            kk = slice(2 * j, 2 * j + 2)
                            nc.tensor.matmul(pss[c][:], we[:, kk, hcols],
                                             ht8[:, kk, sl], start=False,
                                             stop=(j == NKP - 1), perf_mode=DR)
                    for c, (w8, we, dst, bcol, sl) in enumerate(chains):
                        nc.scalar.activation(dst[h][:, sl], pss[c][:], Ident,
                                             bias=bqk[:, bcol + h:bcol + h + 1],
                                             scale=QK_DESCALE)
                else:
                    for c, (w8, we, dst, bcol, sl) in enumerate(chains):
                        ps = pools[c % 4].tile([128, CHUNK], f32, tag=f"ps{c % 4}")
                        for j in range(NKP):
                            dr3_step(ps, j, w8, we, hcols, sl, j == 0, j == NKP - 1)
                        nc.scalar.activation(dst[h][:, sl], ps[:], Ident,
                                             bias=bqk[:, bcol + h:bcol + h + 1],
                                             scale=QK_DESCALE)

            def v_proj(st_lo, st_hi):
                # vn[s-part, st, h, d] natural layout, both heads per chain
                for st in range(st_lo, st_hi):
                    ssl = slice(st * 128, (st + 1) * 128)
                    ps = pools[st % 4].tile([128, HPC * D], f32, tag=f"ps{st % 4}")
                    for j in range(NKP):
                        kk = slice(2 * j, 2 * j + 2)
                        nc.tensor.matmul(ps[:], ht8[:, kk, ssl], wv8[:, kk, :],
                                         start=(j == 0), stop=False, perf_mode=DR)
                        nc.tensor.matmul(ps[:], hte[:, kk, ssl], wv8[:, kk, :],
                                         start=False, stop=False, perf_mode=DR)
                        nc.tensor.matmul(ps[:], ht8[:, kk, ssl], wve[:, kk, :],
                                         start=False, stop=(j == NKP - 1), perf_mode=DR)
                    # vn = psum + 4096*b, both heads in one op
                    nc.vector.tensor_tensor(
                        vn[:, st, :, 0:D],
                        ps[:].rearrange("p (c d) -> p c d", c=HPC),
                        bvb[:].rearrange("p (c d) -> p c d", c=HPC), ADD)

            def rope(t):
                # rows 0..31: t = t*cos + rotate_half(t)*sin, per-chunk
                rotu = rop.tile([ROT, S], fp16, tag="rotu")
                for ci in range(NCH):
                    sl = slice(ci * CHUNK, (ci + 1) * CHUNK)
                    nc.scalar.dma_start(rotu[0:16, sl], t[16:32, sl])
                    nc.scalar.dma_start(rotu[16:32, sl], t[0:16, sl])
                    nc.vector.tensor_tensor(rotu[:, sl], rotu[:, sl], sinT[:, sl], MULT)
                    nc.vector.tensor_tensor(t[0:ROT, sl], t[0:ROT, sl], cosT[:, sl], MULT)
                    nc.vector.tensor_tensor(t[0:ROT, sl], t[0:ROT, sl], rotu[:, sl], ADD)

            def scores(ci, h):
                # transposed scores + exp -> fp16 probs tiles
                prs = []
                ntile = 4 * ci + 4
                for t in range(ntile):
                    pool = pools[t % 2]
                    pss = pool.tile([128, CHUNK], f32, tag=f"ps{t % 2}")
                    off = (t - 4 * ci) * 128
                    lo = max(off, 0)  # cols i < off never consumed
                    nc.tensor.matmul(
                        pss[:, lo:], kT[h][:, t * 128:(t + 1) * 128],
                        qT[h][:, ci * CHUNK + lo:(ci + 1) * CHUNK],
                        start=True, stop=(off < 0))
                    if off >= 0:
                        # diagonal tile: add causal mask via I @ maskb
                        nc.tensor.matmul(pss[:, off:off + 128], ident[:], maskb[:],
                                         start=False, stop=True)
                    pr = prp.tile([128, CHUNK], fp16, tag="probs")
                    nc.scalar.activation(pr[:, lo:], pss[:, lo:], Exp, scale=NORM)
                    prs.append(pr)
                return prs

            def ctx(ci, h, prs):
                # context + denominator; normalize; transpose back via PE.
                # pc rotates over psA/psB (4 chains in flight) so the DVE
                # reciprocal+scale latency never starves the PE.
                for io in range(4):
                    it = 4 * ci + io
                    pc = pools[io % 2].tile([128, CHUNK], f32, tag=f"ps{io % 2}",
                                            name=f"pc{io}")
                    for t in range(it + 1):
                        nc.tensor.matmul(
                            pc[:, 0:D + 1],
                            prs[t][:, io * 128:(io + 1) * 128],
                            vn[:, t, h, :],
                            start=(t == 0), stop=(t == it))
                    rec = rcp.tile([128, 1], f32, tag="rec")
                    nc.vector.reciprocal(rec[:], pc[:, D:D + 1])
                    cn = cnp.tile([128, D], fp16, tag="ctxn")
                    nc.vector.tensor_scalar_mul(cn[:], pc[:, 0:D], rec[:, 0:1])
                    pt = psC.tile([128, D], fp16, tag="ps2", name="pt")
                    nc.tensor.transpose(pt[:], cn[:], ident[:])
                    nc.vector.tensor_copy(ctxT[h][:, it * 128:(it + 1) * 128], pt[:])

            def dense(ci, piecewise=False):
                # row-parallel dense slice for s-tiles of chunk ci, fp16
                # partial; copies split DVE/Act; piecewise DMA shortens the
                # final tail
                for st in range(4 * ci, 4 * ci + 4):
                    stg = stp.tile([128, HID], fp16, tag="stg")
                    for oc in range(NCH):
                        po = psD.tile([128, CHUNK], f32, tag="ps3")
                        for c in range(HPC):
                            nc.tensor.matmul(
                                po[:], ctxT[c][:, st * 128:(st + 1) * 128],
                                wd[:, c, oc * CHUNK:(oc + 1) * CHUNK],
                                start=(c == 0), stop=(c == HPC - 1))
                        osl = slice(oc * CHUNK, (oc + 1) * CHUNK)
                        if oc % 2 == 0:
                            nc.vector.tensor_copy(stg[:, osl], po[:])
                        else:
                            nc.scalar.activation(stg[:, osl], po[:], Ident)
                        if piecewise:
                            nc.sync.dma_start(
                                out_d[st * 128:(st + 1) * 128, osl],
                                stg[:, osl])
                    if not piecewise:
                        nc.sync.dma_start(out_d[st * 128:(st + 1) * 128, :], stg[:])

            # ---- schedule: dense(ci) deferred into chunk ci+1's window so the
            # scalar engine's exp backlog never blocks the PE ----
            qk_proj(0, interleave=True)
            rope(kT[0])
            rope(qT[0])
            qk_proj(1, interleave=False)
            rope(kT[1])
            rope(qT[1])
            pr0 = scores(0, 0)
            v_proj(0, 8)
            pr1 = scores(0, 1)
            v_proj(8, 16)
            ctx(0, 0, pr0)
            ctx(0, 1, pr1)
            for ci in range(1, NCH):
                pr0 = scores(ci, 0)
                pr1 = scores(ci, 1)
                dense(ci - 1)
                ctx(ci, 0, pr0)
                ctx(ci, 1, pr1)
            dense(NCH - 1, piecewise=True)

    nc.compile()
    return nc


def _q8pair(x, scale):
    """Scaled 2-plane e4m3 split: x*scale = hi + lo to ~0.1%."""
    xs = (np.asarray(x, np.float32) * scale).astype(np.float32)
    hi = xs.astype(E4NP)
    lo = (xs - hi.astype(np.float32)).astype(E4NP)
    return hi, lo


def _row_major_128(a, ngroups):
    """[(g p), m] -> [p, (g m)] so DMA runs are >=512B contiguous."""
    g, m = ngroups, a.shape[1]
    return np.ascontiguousarray(
        a.reshape(g, 128, m).transpose(1, 0, 2).reshape(128, g * m))


def _prep_inputs(hidden_states, W_qkv, b_qkv, W_dense, b_dense):
    hid = np.asarray(hidden_states).reshape(S, HID)
    hT = np.ascontiguousarray(hid.T).astype(np.float32)   # [HID, S]
    ht8, hte = _q8pair(hT, SX)
    ht8 = _row_major_128(ht8, NKT)
    hte = _row_major_128(hte, NKT)

    inv_freq = 1.0 / (10000.0 ** (np.arange(0, ROT, 2, dtype=np.float64) / ROT))
    t = np.arange(S, dtype=np.float64)
    freqs = np.outer(t, inv_freq)                      # [s, rot/2]
    emb = np.concatenate([freqs, freqs], axis=1)       # [s, rot]
    cosT = np.ascontiguousarray(np.cos(emb).T).astype(F16NP)
    sinT = np.cos(emb - np.pi / 2).T                   # = sin
    sinTeff = np.concatenate([-sinT[: ROT // 2], sinT[ROT // 2:]], axis=0)
    sinTeff = np.ascontiguousarray(sinTeff).astype(F16NP)

    maskb = np.where(
        np.arange(128)[:, None] > np.arange(128)[None, :], MASK_NEG, 0.0
    ).astype(F16NP)
    ident = np.eye(128, dtype=F16NP)

    in_maps = []
    for c in range(NCORES):
        heads = [HPC * c, HPC * c + 1]
        wq = np.concatenate([W_qkv[:, n * 384: n * 384 + 128] for n in heads], 1)
        wk = np.concatenate([W_qkv[:, n * 384 + 128: n * 384 + 256] for n in heads], 1)
        wv = np.concatenate([W_qkv[:, n * 384 + 256: n * 384 + 384] for n in heads], 1)
        wq8, wqe = _q8pair(wq, SWQK)
        wk8, wke = _q8pair(wk, SWQK)
        wv8, wve = _q8pair(wv, SWV)
        bq = np.stack([b_qkv[n * 384: n * 384 + 128] for n in heads], 1)
        bk = np.stack([b_qkv[n * 384 + 128: n * 384 + 256] for n in heads], 1)
        bv = np.concatenate([b_qkv[n * 384 + 256: n * 384 + 384] for n in heads])
        bqk = np.concatenate([bq, bk], axis=1).astype(np.float32)  # [128,4] q0 q1 k0 k1
        bvb = (VSCALE * np.broadcast_to(bv, (128, HPC * D))).astype(np.float32)
        wdd = np.asarray(W_dense[c * HPC * D:(c + 1) * HPC * D, :], np.float32)
        in_maps.append({
            "ht8": ht8,
            "hte": hte,
            "wq8": _row_major_128(wq8, NKT),
            "wqe": _row_major_128(wqe, NKT),
            "wk8": _row_major_128(wk8, NKT),
            "wke": _row_major_128(wke, NKT),
            "wv8": _row_major_128(wv8, NKT),
            "wve": _row_major_128(wve, NKT),
            "wd": _row_major_128(wdd.astype(F16NP), HPC),
            "cosT": cosT,
            "sinTeff": sinTeff,
            "maskbias": maskb,
            "ident": ident,
            "bqk": np.ascontiguousarray(bqk),
            "bvb": np.ascontiguousarray(bvb),
        })
    return in_maps


def _reduce(results, inputs):
    partial = np.zeros((S, HID), np.float64)
    for r in results:
        partial += r["partial"].astype(np.float64)
    out = (partial + np.asarray(inputs["b_dense"])[None, :]).astype(np.float32)
    return out.reshape(S, 1, HID)


def _run(inputs, trace=False):
    from concourse.bass_utils import run_bass_kernel_spmd

    if "nc" not in _cache:
        _cache["nc"] = _build_program()
    nc = _cache["nc"]
    in_maps = _prep_inputs(
        inputs["hidden_states"], inputs["W_qkv"], inputs["b_qkv"],
        inputs["W_dense"], inputs["b_dense"],
    )
    res = run_bass_kernel_spmd(nc, in_maps, list(range(NCORES)), trace=trace)
    return _reduce(res.results, inputs), res


def kernel(**inputs):
    out, _ = _run(inputs, trace=False)
    return out


# revision 30
# speedup vs baseline: 1.0742x; 1.0742x over previous
"""GPT-NeoX attention (s=2048, b=1, h=2048, nh=16, hd=128, rot=32) on 8 NeuronCores.

Sharding: tensor-parallel over heads (2 heads per core), row-parallel dense
with host-side partial reduction.

Per core: the QKV projections run as fp8e4 DoubleRow matmuls with 3-term
residual compensation (X@W ~ X8@W8 + Xe@W8 + X8@We, each plane pre-scaled
into e4m3's dynamic range) - 0.75x the bf16 PE cost at ~0.1% error. The
attention core (scores, softmax, context) and the dense matmul run in fp16.
Scores use a transposed layout; context is computed in natural layout with a
ones-column so the softmax denominator falls out of the matmul; a per-row
reciprocal scale normalizes on the vector engine; context transposes back on
the PE for the dense slice. Dense output of chunk i is deferred into chunk
i+1's window so the scalar engine's softmax-exp latency never stalls the PE.
"""

import math
import numpy as np
import ml_dtypes

S = 2048
HID = 2048
NH = 16
D = 128
ROT = 32
NCORES = 8
HPC = 2  # heads per core
CHUNK = 512
NKT = HID // 128  # 16 contraction tiles
NKP = NKT // 2    # 8 DoubleRow k-tile pairs
NCH = S // CHUNK  # 4 i-chunks
NST = S // 128    # 16 s-tiles
NORM = 1.0 / math.sqrt(D)
MASK_NEG = -1000.0 / NORM  # -1000 after the exp scale; exp() underflows to 0

# fp8 plane scales: place values in e4m3's sweet spot (max 240, min normal 2^-6)
SX = 32.0      # hidden-state planes
SWQK = 2048.0  # Wq/Wk planes -> q,k psum at 2^16, descaled in the PSUM->SBUF copy
SWV = 128.0    # Wv planes    -> v psum at 2^12 = 4096*v, cancelled by the ones column
QK_DESCALE = 1.0 / (SX * SWQK)
VSCALE = SX * SWV  # 4096; vn holds 4096*(v+b); ones col = 4096 so cn = ctx

E4NP = ml_dtypes.float8_e4m3
F16NP = np.float16

_cache = {}


def _build_program():
    from concourse import bass, bacc, tile
    from concourse.bass import mybir

    f32 = mybir.dt.float32
    fp16 = mybir.dt.float16
    fp8 = mybir.dt.float8e4
    Exp = mybir.ActivationFunctionType.Exp
    Ident = mybir.ActivationFunctionType.Identity
    ADD = mybir.AluOpType.add
    MULT = mybir.AluOpType.mult
    DR = mybir.MatmulPerfMode.DoubleRow

    nc = bacc.Bacc()

    # all dram tensors laid out [128, free] with >=512B contiguous runs
    ht8_d = nc.dram_tensor("ht8", [128, NKT * S], fp8, kind="ExternalInput")
    hte_d = nc.dram_tensor("hte", [128, NKT * S], fp8, kind="ExternalInput")
    wq8_d = nc.dram_tensor("wq8", [128, NKT * HPC * D], fp8, kind="ExternalInput")
    wqe_d = nc.dram_tensor("wqe", [128, NKT * HPC * D], fp8, kind="ExternalInput")
    wk8_d = nc.dram_tensor("wk8", [128, NKT * HPC * D], fp8, kind="ExternalInput")
    wke_d = nc.dram_tensor("wke", [128, NKT * HPC * D], fp8, kind="ExternalInput")
    wv8_d = nc.dram_tensor("wv8", [128, NKT * HPC * D], fp8, kind="ExternalInput")
    wve_d = nc.dram_tensor("wve", [128, NKT * HPC * D], fp8, kind="ExternalInput")
    wd_d = nc.dram_tensor("wd", [128, HPC * HID], fp16, kind="ExternalInput")
    cos_d = nc.dram_tensor("cosT", [ROT, S], fp16, kind="ExternalInput")
    sin_d = nc.dram_tensor("sinTeff", [ROT, S], fp16, kind="ExternalInput")
    mask_d = nc.dram_tensor("maskbias", [128, 128], fp16, kind="ExternalInput")
    perm_d = nc.dram_tensor("perm", [ROT, ROT], fp16, kind="ExternalInput")
    ident_d = nc.dram_tensor("ident", [128, 128], fp16, kind="ExternalInput")
    bqk_d = nc.dram_tensor("bqk", [128, 4], f32, kind="ExternalInput")
    bvc_d = nc.dram_tensor("bvc", [128, HPC], f32, kind="ExternalInput")
    out_d = nc.dram_tensor("partial", [S, HID], fp16, kind="ExternalOutput")

    with tile.TileContext(nc) as tc:
        with (
            tc.tile_pool(name="persist", bufs=1) as pp,
            tc.tile_pool(name="probs", bufs=36) as prp,
            tc.tile_pool(name="rotu", bufs=2) as rop,
            tc.tile_pool(name="ctxn", bufs=6) as cnp,
            tc.tile_pool(name="rec", bufs=8) as rcp,
            tc.tile_pool(name="stage", bufs=3) as stp,
            tc.tile_pool(name="psA", bufs=3, space="PSUM") as psA,
            tc.tile_pool(name="psB", bufs=3, space="PSUM") as psB,
            tc.tile_pool(name="psD", bufs=2, space="PSUM") as psD,
        ):
            pools = [psA, psB, psD]

            # ---- persistent SBUF tiles ----
            ht8 = pp.tile([128, NKT, S], fp8, tag="ht8")
            hte = pp.tile([128, NKT, S], fp8, tag="hte")
            wq8 = pp.tile([128, NKT, HPC * D], fp8, tag="wq8")
            wqe = pp.tile([128, NKT, HPC * D], fp8, tag="wqe")
            wk8 = pp.tile([128, NKT, HPC * D], fp8, tag="wk8")
            wke = pp.tile([128, NKT, HPC * D], fp8, tag="wke")
            wv8 = pp.tile([128, NKT, HPC * D], fp8, tag="wv8")
            wve = pp.tile([128, NKT, HPC * D], fp8, tag="wve")
            wd = pp.tile([128, HPC, HID], fp16, tag="wd")
            cosT = pp.tile([ROT, S], fp16, tag="cos")
            sinT = pp.tile([ROT, S], fp16, tag="sin")
            maskb = pp.tile([128, 128], fp16, tag="mask")
            perm = pp.tile([ROT, ROT], fp16, tag="perm")
            ident = pp.tile([128, 128], fp16, tag="ident")
            bqk = pp.tile([128, 4], f32, tag="bqk")
            bvc = pp.tile([128, HPC], f32, tag="bvc")
            qT = [pp.tile([128, S], fp16, tag=f"qT{h}", name=f"qT{h}") for h in range(HPC)]
            kT = [pp.tile([128, S], fp16, tag=f"kT{h}", name=f"kT{h}") for h in range(HPC)]
            # V natural layout (both heads) + ones column for the denominator
            vn = pp.tile([128, NST, HPC, D + 1], fp16, tag="vn")
            ctxT = [pp.tile([128, S], fp16, tag=f"ctxT{h}", name=f"ctxT{h}")
                    for h in range(HPC)]

            nc.vector.memset(vn[:, :, :, D:D + 1], VSCALE)

            # warm the activation function table while DMAs stream
            warm = pp.tile([128, 1], f32, tag="warm")
            nc.vector.memset(warm[:], 0.0)
            nc.scalar.activation(warm[:], warm[:], Exp)

            # ---- input DMAs: the first chains' (main/xres) terms need only
            # wk8/wq8 + ht planes; the we planes (wres pass) can arrive late ----
            wk8_r = wk8_d[:].rearrange("p (k m) -> p k m", k=NKT)
            wq8_r = wq8_d[:].rearrange("p (k m) -> p k m", k=NKT)
            ht8_r = ht8_d[:].rearrange("p (k s) -> p k s", k=NKT)
            hte_r = hte_d[:].rearrange("p (k s) -> p k s", k=NKT)
            # k-pair ht DMAs halve the HWDGE issue slots (the scarce
            # resource: ~630ns serialized issue per DMA); small side tensors
            # go through the Pool engine's SWDGE path, which skips HWDGE
            nc.sync.dma_start(wk8[:, 0:4, :], wk8_r[:, 0:4, :])
            nc.sync.dma_start(ht8[:, 0:2, 0:CHUNK], ht8_r[:, 0:2, 0:CHUNK])
            nc.sync.dma_start(wq8[:, 0:4, :], wq8_r[:, 0:4, :])
            nc.sync.dma_start(hte[:, 0:2, 0:CHUNK], hte_r[:, 0:2, 0:CHUNK])
            nc.sync.dma_start(ht8[:, 0:2, CHUNK:], ht8_r[:, 0:2, CHUNK:])
            nc.sync.dma_start(hte[:, 0:2, CHUNK:], hte_r[:, 0:2, CHUNK:])
            for k in range(2, NKT):
                nc.sync.dma_start(ht8[:, k, :], ht8_r[:, k, :])
                nc.sync.dma_start(hte[:, k, :], hte_r[:, k, :])
                if k == 2:
                    nc.sync.dma_start(wk8[:, 4:, :], wk8_r[:, 4:, :])
                    nc.sync.dma_start(wq8[:, 4:, :], wq8_r[:, 4:, :])
                if k == 4:
                    nc.sync.dma_start(wke[:], wke_d[:].rearrange("p (k m) -> p k m", k=NKT))
                if k == 5:
                    nc.sync.dma_start(wqe[:], wqe_d[:].rearrange("p (k m) -> p k m", k=NKT))
            nc.gpsimd.dma_start(cosT[:], cos_d[:])
            nc.gpsimd.dma_start(sinT[:], sin_d[:])
            nc.gpsimd.dma_start(maskb[:], mask_d[:])
            nc.gpsimd.dma_start(perm[:], perm_d[:])
            nc.gpsimd.dma_start(ident[:], ident_d[:])
            nc.gpsimd.dma_start(bqk[:], bqk_d[:])
            nc.gpsimd.dma_start(bvc[:], bvc_d[:])
            nc.sync.dma_start(wv8[:], wv8_d[:].rearrange("p (k m) -> p k m", k=NKT))
            nc.sync.dma_start(wve[:], wve_d[:].rearrange("p (k m) -> p k m", k=NKT))
            nc.sync.dma_start(wd[:], wd_d[:].rearrange("p (c o) -> p c o", c=HPC))

            def dr3_step(ps, j, w8, we, hcols, sl, first, last):
                # one k-pair step of a 3-term compensated chain
                kk = slice(2 * j, 2 * j + 2)
                nc.tensor.matmul(ps[:], w8[:, kk, hcols], ht8[:, kk, sl],
                                 start=first, stop=False, perf_mode=DR)
                nc.tensor.matmul(ps[:], w8[:, kk, hcols], hte[:, kk, sl],
                                 start=False, stop=False, perf_mode=DR)
                nc.tensor.matmul(ps[:], we[:, kk, hcols], ht8[:, kk, sl],
                                 start=False, stop=last, perf_mode=DR)

            def qk_proj(h, interleave):
                # qT/kT[h][d=128, s]; bias + 2^-16 descale in the PSUM->SBUF copy
                hcols = slice(h * D, (h + 1) * D)
                chains = []
                for ci in range(NCH):
                    sl = slice(ci * CHUNK, (ci + 1) * CHUNK)
                    for (w8, we, dst, bcol) in ((wk8, wke, kT, 2), (wq8, wqe, qT, 0)):
                        chains.append((w8, we, dst, bcol, sl))
                def chain_pool(c):
                    return (psA, "ps0") if c < 3 else (psB, "ps1") if c < 6 else (psD, "ps3")

                if interleave:
                    # j-major across all 8 chains so the PE rides the ht DMA
                    # staircase; the wres term lags 2 j-steps so the we
                    # planes (DMA'd mid-stream) never stall the pipeline
                    LAG = 3
                    pss = [chain_pool(c)[0].tile([128, CHUNK], f32,
                                                  tag=chain_pool(c)[1],
                                                  name=f"pss{c}")
                           for c in range(8)]
                    for j in range(NKP + LAG):
                        if j < NKP:
                            kk = slice(2 * j, 2 * j + 2)
                            for c, (w8, we, dst, bcol, sl) in enumerate(chains):
                                nc.tensor.matmul(pss[c][:], w8[:, kk, hcols],
                                                 ht8[:, kk, sl], start=(j == 0),
                                                 stop=False, perf_mode=DR)
                            for c, (w8, we, dst, bcol, sl) in enumerate(chains):
                                nc.tensor.matmul(pss[c][:], w8[:, kk, hcols],
                                                 hte[:, kk, sl], start=False,
                                                 stop=False, perf_mode=DR)
                        if j >= LAG:
                            jw = j - LAG
                            kk = slice(2 * jw, 2 * jw + 2)
                            for c, (w8, we, dst, bcol, sl) in enumerate(chains):
                                nc.tensor.matmul(pss[c][:], we[:, kk, hcols],
                                                 ht8[:, kk, sl], start=False,
                                                 stop=(jw == NKP - 1),
                                                 perf_mode=DR)
                    for c, (w8, we, dst, bcol, sl) in enumerate(chains):
                        nc.scalar.activation(dst[h][:, sl], pss[c][:], Ident,
                                             bias=bqk[:, bcol + h:bcol + h + 1],
                                             scale=QK_DESCALE)
                else:
                    for c, (w8, we, dst, bcol, sl) in enumerate(chains):
                        ps = chain_pool(c)[0].tile([128, CHUNK], f32,
                                                   tag=chain_pool(c)[1], name="ps")
                        for j in range(NKP):
                            dr3_step(ps, j, w8, we, hcols, sl, j == 0, j == NKP - 1)
                        nc.scalar.activation(dst[h][:, sl], ps[:], Ident,
                                             bias=bqk[:, bcol + h:bcol + h + 1],
                                             scale=QK_DESCALE)

            def v_proj(st_lo, st_hi):
                # vn[s-part, st, h, d] natural layout, both heads per chain
                for st in range(st_lo, st_hi):
                    ssl = slice(st * 128, (st + 1) * 128)
                    vp, vt = [(psA, "ps0"), (psB, "ps1"), (psD, "ps3")][st % 3]
                    ps = vp.tile([128, HPC * D], f32, tag=vt, name="ps")
                    for j in range(NKP):
                        kk = slice(2 * j, 2 * j + 2)
                        nc.tensor.matmul(ps[:], ht8[:, kk, ssl], wv8[:, kk, :],
                                         start=(j == 0), stop=False, perf_mode=DR)
                        nc.tensor.matmul(ps[:], hte[:, kk, ssl], wv8[:, kk, :],
                                         start=False, stop=False, perf_mode=DR)
                        nc.tensor.matmul(ps[:], ht8[:, kk, ssl], wve[:, kk, :],
                                         start=False, stop=(j == NKP - 1), perf_mode=DR)
                    # vn = 4096*v cast to fp16 (the v-bias is added
                    # per-partition in the post-transpose ctxT copy)
                    nc.vector.tensor_copy(
                        vn[:, st, :, 0:D],
                        ps[:].rearrange("p (c d) -> p c d", c=HPC))

            def rope(tensors):
                # rows 0..31: t = t*cos + rotate_half(t)*sin; the half-swap
                # runs on the PE as a permutation matmul (no DMA latency).
                # ci-major across tensors so early chunks unblock scores fast
                for ci in range(NCH):
                    sl = slice(ci * CHUNK, (ci + 1) * CHUNK)
                    for ti, t in enumerate(tensors):
                        rps = pools[(2 * ci + ti) % 2].tile(
                            [ROT, CHUNK], f32, tag=f"ps{(2 * ci + ti) % 2}",
                            name="rps")
                        nc.tensor.matmul(rps[:], perm[:], t[0:ROT, sl],
                                         start=True, stop=True)
                        rotu = rop.tile([ROT, CHUNK], fp16, tag="rotu")
                        nc.vector.tensor_tensor(rotu[:], rps[:], sinT[:, sl], MULT)
                        nc.vector.tensor_tensor(t[0:ROT, sl], t[0:ROT, sl], cosT[:, sl], MULT)
                        nc.vector.tensor_tensor(t[0:ROT, sl], t[0:ROT, sl], rotu[:], ADD)

            def scores_t(ci, h, t, prs):
                # one transposed scores tile + exp -> fp16 probs tile
                pool = pools[t % 2]
                pss = pool.tile([128, CHUNK], f32, tag=f"ps{t % 2}", name="pss")
                off = (t - 4 * ci) * 128
                lo = max(off, 0)  # cols i < off never consumed
                nc.tensor.matmul(
                    pss[:, lo:], kT[h][:, t * 128:(t + 1) * 128],
                    qT[h][:, ci * CHUNK + lo:(ci + 1) * CHUNK],
                    start=True, stop=(off < 0))
                if off >= 0:
                    # diagonal tile: add causal mask via I @ maskb
                    nc.tensor.matmul(pss[:, off:off + 128], ident[:], maskb[:],
                                     start=False, stop=True)
                pr = prp.tile([128, CHUNK], fp16, tag="probs")
                nc.scalar.activation(pr[:, lo:], pss[:, lo:], Exp, scale=NORM)
                prs.append(pr)

            def ctx_io(ci, h, io, prs):
                # context + denominator; normalize; transpose back via PE.
                # pc rotates over psA/psB (4 chains in flight) so the DVE
                # reciprocal+scale latency never starves the PE.
                it = 4 * ci + io
                pc = pools[io % 2].tile([128, CHUNK], f32, tag=f"ps{io % 2}",
                                        name=f"pc{io}")
                for t in range(it + 1):
                    nc.tensor.matmul(
                        pc[:, 0:D + 1],
                        prs[t][:, io * 128:(io + 1) * 128],
                        vn[:, t, h, :],
                        start=(t == 0), stop=(t == it))
                rec = rcp.tile([128, 1], f32, tag="rec")
                nc.vector.reciprocal(rec[:], pc[:, D:D + 1])
                cn = cnp.tile([128, D], fp16, tag="ctxn")
                nc.vector.tensor_scalar_mul(cn[:], pc[:, 0:D], rec[:, 0:1])
                pt = psD.tile([128, D], fp16, tag="ps3", name="pt")
                nc.tensor.transpose(pt[:], cn[:], ident[:])
                nc.vector.tensor_scalar(
                    ctxT[h][:, it * 128:(it + 1) * 128], pt[:],
                    bvc[:, h:h + 1], None, op0=ADD)

            def ctx(ci, h, prs):
                for io in range(4):
                    ctx_io(ci, h, io, prs)

            def dense_pieces(ci, on_act=False):
                # 16 oc-chain closures for chunk ci's dense s-tiles; callers
                # interleave them between scores tiles to keep the PE fed
                # while the scalar engine drains the exp backlog
                items = []
                state = {}

                def mk(st, oc):
                    def run():
                        if oc == 0:
                            state[st] = stp.tile([128, HID], fp16, tag="stg",
                                                 name=f"stg{st}")
                        stg = state[st]
                        po = psD.tile([128, CHUNK], f32, tag="ps3", name="po")
                        for c in range(HPC):
                            nc.tensor.matmul(
                                po[:], ctxT[c][:, st * 128:(st + 1) * 128],
                                wd[:, c, oc * CHUNK:(oc + 1) * CHUNK],
                                start=(c == 0), stop=(c == HPC - 1))
                        osl = slice(oc * CHUNK, (oc + 1) * CHUNK)
                        if on_act:
                            nc.scalar.activation(stg[:, osl], po[:], Ident)
                        else:
                            nc.vector.tensor_copy(stg[:, osl], po[:])
                        if on_act and oc % 2 == 1:
                            hsl = slice((oc - 1) * CHUNK, (oc + 1) * CHUNK)
                            nc.sync.dma_start(
                                out_d[st * 128:(st + 1) * 128, hsl], stg[:, hsl])
                        elif not on_act and oc == NCH - 1:
                            nc.sync.dma_start(
                                out_d[st * 128:(st + 1) * 128, :], stg[:])
                    return run

                for st in range(4 * ci, 4 * ci + 4):
                    for oc in range(NCH):
                        items.append(mk(st, oc))
                return items

            def dense_st(st):
                for item in dense_pieces_for_st(st):
                    item()

            def dense_pieces_for_st(st):
                ci = st // 4
                all_items = dense_pieces(ci)
                return all_items[(st % 4) * NCH:(st % 4 + 1) * NCH]

            # ---- schedule: dense(ci) deferred into chunk ci+1's window so the
            # scalar engine's exp backlog never blocks the PE ----
            qk_proj(0, interleave=True)
            rope([kT[0], qT[0]])
            qk_proj(1, interleave=False)
            rope([kT[1], qT[1]])
            # chunk order 1,2,3,0: the big chunks' exp backlogs overlap
            # mid-kernel compute; tiny chunk 0 (8 exp tiles) lands last so
            # the tail is not exp-bound. dense(prev) fills each window.
            order = [0, 1, 2, 3]
            prs = {}
            for wi, ci in enumerate(order):
                ntile = 4 * ci + 4
                pr0, pr1 = [], []
                prs[ci] = (pr0, pr1)
                for t in range(ntile):
                    scores_t(ci, 0, t, pr0)
                if wi == 0:
                    v_proj(0, 8)
                for t in range(ntile):
                    scores_t(ci, 1, t, pr1)
                if wi == 0:
                    v_proj(8, 16)
                if wi >= 1:
                    for item in dense_pieces(order[wi - 1]):
                        item()
                ctx(ci, 0, pr0)
                if wi < len(order) - 1:
                    ctx(ci, 1, pr1)
                else:
                    # final window: stream each dense s-tile right after its
                    # ctx; copies on the now-empty scalar engine so DVE
                    # stays clear for the recip/scale chain
                    dnl = dense_pieces(ci, on_act=True)
                    for io in range(4):
                        ctx_io(ci, 1, io, pr1)
                        for item in dnl[io * NCH:(io + 1) * NCH]:
                            item()

    nc.compile()
    return nc


def _q8pair(x, scale):
    """Scaled 2-plane e4m3 split: x*scale = hi + lo to ~0.1%."""
    xs = (np.asarray(x, np.float32) * scale).astype(np.float32)
    hi = xs.astype(E4NP)
    lo = (xs - hi.astype(np.float32)).astype(E4NP)
    return hi, lo


def _row_major_128(a, ngroups):
    """[(g p), m] -> [p, (g m)] so DMA runs are >=512B contiguous."""
    g, m = ngroups, a.shape[1]
    return np.ascontiguousarray(
        a.reshape(g, 128, m).transpose(1, 0, 2).reshape(128, g * m))


def _prep_inputs(hidden_states, W_qkv, b_qkv, W_dense, b_dense):
    hid = np.asarray(hidden_states).reshape(S, HID)
    hT = np.ascontiguousarray(hid.T).astype(np.float32)   # [HID, S]
    ht8, hte = _q8pair(hT, SX)
    ht8 = _row_major_128(ht8, NKT)
    hte = _row_major_128(hte, NKT)

    inv_freq = 1.0 / (10000.0 ** (np.arange(0, ROT, 2, dtype=np.float64) / ROT))
    t = np.arange(S, dtype=np.float64)
    freqs = np.outer(t, inv_freq)                      # [s, rot/2]
    emb = np.concatenate([freqs, freqs], axis=1)       # [s, rot]
    cosT = np.ascontiguousarray(np.cos(emb).T).astype(F16NP)
    sinT = np.cos(emb - np.pi / 2).T                   # = sin
    sinTeff = np.concatenate([-sinT[: ROT // 2], sinT[ROT // 2:]], axis=0)
    sinTeff = np.ascontiguousarray(sinTeff).astype(F16NP)

    maskb = np.where(
        np.arange(128)[:, None] > np.arange(128)[None, :], MASK_NEG, 0.0
    ).astype(F16NP)
    ident = np.eye(128, dtype=F16NP)
    # rotate-half permutation: out[r] = t[(r+16) % 32]
    perm = np.zeros((ROT, ROT), F16NP)
    perm[(np.arange(ROT) + ROT // 2) % ROT, np.arange(ROT)] = 1.0

    in_maps = []
    for c in range(NCORES):
        heads = [HPC * c, HPC * c + 1]
        wq = np.concatenate([W_qkv[:, n * 384: n * 384 + 128] for n in heads], 1)
        wk = np.concatenate([W_qkv[:, n * 384 + 128: n * 384 + 256] for n in heads], 1)
        wv = np.concatenate([W_qkv[:, n * 384 + 256: n * 384 + 384] for n in heads], 1)
        wq8, wqe = _q8pair(wq, SWQK)
        wk8, wke = _q8pair(wk, SWQK)
        wv8, wve = _q8pair(wv, SWV)
        bq = np.stack([b_qkv[n * 384: n * 384 + 128] for n in heads], 1)
        bk = np.stack([b_qkv[n * 384 + 128: n * 384 + 256] for n in heads], 1)
        bv = np.concatenate([b_qkv[n * 384 + 256: n * 384 + 384] for n in heads])
        bqk = np.concatenate([bq, bk], axis=1).astype(np.float32)  # [128,4] q0 q1 k0 k1
        bvc = np.stack([bv[0:D], bv[D:2 * D]], 1).astype(np.float32)  # [128, 2]
        wdd = np.asarray(W_dense[c * HPC * D:(c + 1) * HPC * D, :], np.float32)
        in_maps.append({
            "ht8": ht8,
            "hte": hte,
            "wq8": _row_major_128(wq8, NKT),
            "wqe": _row_major_128(wqe, NKT),
            "wk8": _row_major_128(wk8, NKT),
            "wke": _row_major_128(wke, NKT),
            "wv8": _row_major_128(wv8, NKT),
            "wve": _row_major_128(wve, NKT),
            "wd": _row_major_128(wdd.astype(F16NP), HPC),
            "cosT": cosT,
            "sinTeff": sinTeff,
            "maskbias": maskb,
            "ident": ident,
            "perm": perm,
            "bqk": np.ascontiguousarray(bqk),
            "bvc": np.ascontiguousarray(bvc),
        })
    return in_maps


def _reduce(results, inputs):
    partial = np.zeros((S, HID), np.float64)
    for r in results:
        partial += r["partial"].astype(np.float64)
    out = (partial + np.asarray(inputs["b_dense"])[None, :]).astype(np.float32)
    return out.reshape(S, 1, HID)


def _run(inputs, trace=False):
    from concourse.bass_utils import run_bass_kernel_spmd

    if "nc" not in _cache:
        _cache["nc"] = _build_program()
    nc = _cache["nc"]
    in_maps = _prep_inputs(
        inputs["hidden_states"], inputs["W_qkv"], inputs["b_qkv"],
        inputs["W_dense"], inputs["b_dense"],
    )
    res = run_bass_kernel_spmd(nc, in_maps, list(range(NCORES)), trace=trace)
    return _reduce(res.results, inputs), res


def kernel(**inputs):
    out, _ = _run(inputs, trace=False)
    return out


# revision 51
# speedup vs baseline: 1.1039x; 1.0277x over previous
"""GPT-NeoX attention (s=2048, b=1, h=2048, nh=16, hd=128, rot=32) on 8 NeuronCores.

Sharding: tensor-parallel over heads (2 heads per core), row-parallel dense
with host-side partial reduction.

Per core: the QKV projections run as fp8e4 DoubleRow matmuls with 3-term
residual compensation (X@W ~ X8@W8 + Xe@W8 + X8@We, each plane pre-scaled
into e4m3's dynamic range) - 0.75x the bf16 PE cost at ~0.1% error. The
attention core (scores, softmax, context) and the dense matmul run in fp16.
Scores use a transposed layout; context is computed in natural layout with a
ones-column so the softmax denominator falls out of the matmul; a per-row
reciprocal scale normalizes on the vector engine; context transposes back on
the PE for the dense slice. Dense output of chunk i is deferred into chunk
i+1's window so the scalar engine's softmax-exp latency never stalls the PE.
"""

import math
import numpy as np
import ml_dtypes

S = 2048
HID = 2048
NH = 16
D = 128
ROT = 32
NCORES = 8
HPC = 2  # heads per core
CHUNK = 512
NKT = HID // 128  # 16 contraction tiles
NKP = NKT // 2    # 8 DoubleRow k-tile pairs
NCH = S // CHUNK  # 4 i-chunks
NST = S // 128    # 16 s-tiles
NORM = 1.0 / math.sqrt(D)
MASK_NEG = -1000.0 / NORM  # -1000 after the exp scale; exp() underflows to 0

# fp8 plane scales: place values in e4m3's sweet spot (max 240, min normal 2^-6)
SX = 32.0      # hidden-state planes
SWQK = 2048.0  # Wq/Wk planes -> q,k psum at 2^16, descaled in the PSUM->SBUF copy
SWV = 128.0    # Wv planes    -> v psum at 2^12 = 4096*v, cancelled by the ones column
QK_DESCALE = 1.0 / (SX * SWQK)
VSCALE = SX * SWV  # 4096; vn holds 4096*(v+b); ones col = 4096 so cn = ctx

E4NP = ml_dtypes.float8_e4m3
F16NP = np.float16

_cache = {}


def _build_program():
    from concourse import bass, bacc, tile
    from concourse.bass import mybir

    f32 = mybir.dt.float32
    fp16 = mybir.dt.float16
    fp8 = mybir.dt.float8e4
    Exp = mybir.ActivationFunctionType.Exp
    Ident = mybir.ActivationFunctionType.Identity
    ADD = mybir.AluOpType.add
    MULT = mybir.AluOpType.mult
    DR = mybir.MatmulPerfMode.DoubleRow

    nc = bacc.Bacc()

    # all dram tensors laid out [128, free] with >=512B contiguous runs
    ht8_d = nc.dram_tensor("ht8", [128, NKT * S], fp8, kind="ExternalInput")
    hte_d = nc.dram_tensor("hte", [128, NKT * S], fp8, kind="ExternalInput")
    wq8_d = nc.dram_tensor("wq8", [128, NKT * HPC * D], fp8, kind="ExternalInput")
    wqe_d = nc.dram_tensor("wqe", [128, NKT * HPC * D], fp8, kind="ExternalInput")
    wk8_d = nc.dram_tensor("wk8", [128, NKT * HPC * D], fp8, kind="ExternalInput")
    wke_d = nc.dram_tensor("wke", [128, NKT * HPC * D], fp8, kind="ExternalInput")
    wv8_d = nc.dram_tensor("wv8", [128, NKT * HPC * D], fp8, kind="ExternalInput")
    wve_d = nc.dram_tensor("wve", [128, NKT * HPC * D], fp8, kind="ExternalInput")
    wd_d = nc.dram_tensor("wd", [128, HPC * HID], fp16, kind="ExternalInput")
    cos_d = nc.dram_tensor("cosT", [ROT, S], fp16, kind="ExternalInput")
    sin_d = nc.dram_tensor("sinTeff", [ROT, S], fp16, kind="ExternalInput")
    mask_d = nc.dram_tensor("maskbias", [128, 128], fp16, kind="ExternalInput")
    perm_d = nc.dram_tensor("perm", [ROT, ROT], fp16, kind="ExternalInput")
    ident_d = nc.dram_tensor("ident", [128, 128], fp16, kind="ExternalInput")
    bqk_d = nc.dram_tensor("bqk", [128, 4], f32, kind="ExternalInput")
    bvc_d = nc.dram_tensor("bvc", [128, HPC], f32, kind="ExternalInput")
    out_d = nc.dram_tensor("partial", [S, HID], fp16, kind="ExternalOutput")

    with tile.TileContext(nc) as tc:
        with (
            tc.tile_pool(name="persist", bufs=1) as pp,
            tc.tile_pool(name="probs", bufs=40) as prp,
            tc.tile_pool(name="rotu", bufs=2) as rop,
            tc.tile_pool(name="ctxn", bufs=8) as cnp,
            tc.tile_pool(name="rec", bufs=12) as rcp,
            tc.tile_pool(name="stage", bufs=4) as stp,
            tc.tile_pool(name="psA", bufs=3, space="PSUM") as psA,
            tc.tile_pool(name="psB", bufs=3, space="PSUM") as psB,
            tc.tile_pool(name="psD", bufs=2, space="PSUM") as psD,
        ):
            pools = [psA, psB, psD]

            # ---- persistent SBUF tiles ----
            ht8 = pp.tile([128, NKT, S], fp8, tag="ht8")
            hte = pp.tile([128, NKT, S], fp8, tag="hte")
            wq8 = pp.tile([128, NKT, HPC * D], fp8, tag="wq8")
            wqe = pp.tile([128, NKT, HPC * D], fp8, tag="wqe")
            wk8 = pp.tile([128, NKT, HPC * D], fp8, tag="wk8")
            wke = pp.tile([128, NKT, HPC * D], fp8, tag="wke")
            wv8 = pp.tile([128, NKT, HPC * D], fp8, tag="wv8")
            wve = pp.tile([128, NKT, HPC * D], fp8, tag="wve")
            wd = pp.tile([128, HPC, HID], fp16, tag="wd")
            cosT = pp.tile([ROT, S], fp16, tag="cos")
            sinT = pp.tile([ROT, S], fp16, tag="sin")
            maskb = pp.tile([128, 128], fp16, tag="mask")
            perm = pp.tile([ROT, ROT], fp16, tag="perm")
            ident = pp.tile([128, 128], fp16, tag="ident")
            bqk = pp.tile([128, 4], f32, tag="bqk")
            bvc = pp.tile([128, HPC], f32, tag="bvc")
            qT = [pp.tile([128, S], fp16, tag=f"qT{h}", name=f"qT{h}") for h in range(HPC)]
            kT = [pp.tile([128, S], fp16, tag=f"kT{h}", name=f"kT{h}") for h in range(HPC)]
            # V natural layout (both heads) + ones column for the denominator
            vn = pp.tile([128, NST, HPC, D + 1], fp16, tag="vn")
            ctxT = [pp.tile([128, S], fp16, tag=f"ctxT{h}", name=f"ctxT{h}")
                    for h in range(HPC)]

            nc.vector.memset(vn[:, :, :, D:D + 1], VSCALE)

            # warm the activation function table while DMAs stream
            warm = pp.tile([128, 1], f32, tag="warm")
            nc.vector.memset(warm[:], 0.0)
            nc.scalar.activation(warm[:], warm[:], Exp)

            # ---- input DMAs: the first chains' (main/xres) terms need only
            # wk8/wq8 + ht planes; the we planes (wres pass) can arrive late ----
            wk8_r = wk8_d[:].rearrange("p (k m) -> p k m", k=NKT)
            wq8_r = wq8_d[:].rearrange("p (k m) -> p k m", k=NKT)
            ht8_r = ht8_d[:].rearrange("p (k s) -> p k s", k=NKT)
            hte_r = hte_d[:].rearrange("p (k s) -> p k s", k=NKT)
            # k-pair ht DMAs halve the HWDGE issue slots (the scarce
            # resource: ~630ns serialized issue per DMA); small side tensors
            # go through the Pool engine's SWDGE path, which skips HWDGE
            nc.sync.dma_start(wk8[:, 0:4, :], wk8_r[:, 0:4, :])
            nc.sync.dma_start(ht8[:, 0:2, 0:CHUNK], ht8_r[:, 0:2, 0:CHUNK])
            nc.sync.dma_start(wq8[:, 0:4, :], wq8_r[:, 0:4, :])
            nc.sync.dma_start(hte[:, 0:2, 0:CHUNK], hte_r[:, 0:2, 0:CHUNK])
            nc.sync.dma_start(ht8[:, 0:2, CHUNK:], ht8_r[:, 0:2, CHUNK:])
            nc.sync.dma_start(hte[:, 0:2, CHUNK:], hte_r[:, 0:2, CHUNK:])
            for k in range(2, NKT):
                nc.sync.dma_start(ht8[:, k, :], ht8_r[:, k, :])
                nc.sync.dma_start(hte[:, k, :], hte_r[:, k, :])
                if k == 2:
                    nc.sync.dma_start(wke[:], wke_d[:].rearrange("p (k m) -> p k m", k=NKT))
                    nc.sync.dma_start(wqe[:], wqe_d[:].rearrange("p (k m) -> p k m", k=NKT))
                if k == 4:
                    nc.sync.dma_start(wk8[:, 4:, :], wk8_r[:, 4:, :])
                    nc.sync.dma_start(wq8[:, 4:, :], wq8_r[:, 4:, :])
            nc.gpsimd.dma_start(cosT[:], cos_d[:])
            nc.gpsimd.dma_start(sinT[:], sin_d[:])
            nc.gpsimd.dma_start(maskb[:], mask_d[:])
            nc.gpsimd.dma_start(perm[:], perm_d[:])
            nc.gpsimd.dma_start(ident[:], ident_d[:])
            nc.gpsimd.dma_start(bqk[:], bqk_d[:])
            nc.gpsimd.dma_start(bvc[:], bvc_d[:])
            nc.sync.dma_start(wv8[:], wv8_d[:].rearrange("p (k m) -> p k m", k=NKT))
            nc.sync.dma_start(wve[:], wve_d[:].rearrange("p (k m) -> p k m", k=NKT))
            nc.sync.dma_start(wd[:], wd_d[:].rearrange("p (c o) -> p c o", c=HPC))

            def dr3_step(ps, j, w8, we, hcols, sl, first, last):
                # one k-pair step of a 3-term compensated chain
                kk = slice(2 * j, 2 * j + 2)
                nc.tensor.matmul(ps[:], w8[:, kk, hcols], ht8[:, kk, sl],
                                 start=first, stop=False, perf_mode=DR)
                nc.tensor.matmul(ps[:], w8[:, kk, hcols], hte[:, kk, sl],
                                 start=False, stop=False, perf_mode=DR)
                nc.tensor.matmul(ps[:], we[:, kk, hcols], ht8[:, kk, sl],
                                 start=False, stop=last, perf_mode=DR)

            def qk_proj(h, interleave):
                # qT/kT[h][d=128, s]; bias + 2^-16 descale in the PSUM->SBUF copy
                hcols = slice(h * D, (h + 1) * D)
                chains = []
                for ci in range(NCH):
                    sl = slice(ci * CHUNK, (ci + 1) * CHUNK)
                    for (w8, we, dst, bcol) in ((wk8, wke, kT, 2), (wq8, wqe, qT, 0)):
                        chains.append((w8, we, dst, bcol, sl))
                def chain_pool(c):
                    return (psA, "ps0") if c < 3 else (psB, "ps1") if c < 6 else (psD, "ps3")

                if interleave:
                    # j-major across all 8 chains so the PE rides the ht DMA
                    # staircase; the wres term lags 2 j-steps so the we
                    # planes (DMA'd mid-stream) never stall the pipeline
                    LAG = 1
                    pss = [chain_pool(c)[0].tile([128, CHUNK], f32,
                                                  tag=chain_pool(c)[1],
                                                  name=f"pss{c}")
                           for c in range(8)]
                    for j in range(NKP + LAG):
                        if j < NKP:
                            kk = slice(2 * j, 2 * j + 2)
                            for c, (w8, we, dst, bcol, sl) in enumerate(chains):
                                nc.tensor.matmul(pss[c][:], w8[:, kk, hcols],
                                                 ht8[:, kk, sl], start=(j == 0),
                                                 stop=False, perf_mode=DR)
                            for c, (w8, we, dst, bcol, sl) in enumerate(chains):
                                nc.tensor.matmul(pss[c][:], w8[:, kk, hcols],
                                                 hte[:, kk, sl], start=False,
                                                 stop=False, perf_mode=DR)
                        if j >= LAG:
                            jw = j - LAG
                            kk = slice(2 * jw, 2 * jw + 2)
                            for c, (w8, we, dst, bcol, sl) in enumerate(chains):
                                nc.tensor.matmul(pss[c][:], we[:, kk, hcols],
                                                 ht8[:, kk, sl], start=False,
                                                 stop=(jw == NKP - 1),
                                                 perf_mode=DR)
                    for c, (w8, we, dst, bcol, sl) in enumerate(chains):
                        nc.scalar.activation(dst[h][:, sl], pss[c][:], Ident,
                                             bias=bqk[:, bcol + h:bcol + h + 1],
                                             scale=QK_DESCALE)
                else:
                    for c, (w8, we, dst, bcol, sl) in enumerate(chains):
                        ps = chain_pool(c)[0].tile([128, CHUNK], f32,
                                                   tag=chain_pool(c)[1], name="ps")
                        for j in range(NKP):
                            dr3_step(ps, j, w8, we, hcols, sl, j == 0, j == NKP - 1)
                        nc.scalar.activation(dst[h][:, sl], ps[:], Ident,
                                             bias=bqk[:, bcol + h:bcol + h + 1],
                                             scale=QK_DESCALE)

            def v_proj(st_lo, st_hi):
                # vn[s-part, st, h, d] natural layout, both heads per chain
                for st in range(st_lo, st_hi):
                    ssl = slice(st * 128, (st + 1) * 128)
                    vp, vt = [(psA, "ps0"), (psB, "ps1"), (psD, "ps3")][st % 3]
                    ps = vp.tile([128, HPC * D], f32, tag=vt, name="ps")
                    for j in range(NKP):
                        kk = slice(2 * j, 2 * j + 2)
                        nc.tensor.matmul(ps[:], ht8[:, kk, ssl], wv8[:, kk, :],
                                         start=(j == 0), stop=False, perf_mode=DR)
                        nc.tensor.matmul(ps[:], hte[:, kk, ssl], wv8[:, kk, :],
                                         start=False, stop=False, perf_mode=DR)
                        nc.tensor.matmul(ps[:], ht8[:, kk, ssl], wve[:, kk, :],
                                         start=False, stop=(j == NKP - 1), perf_mode=DR)
                    # vn = 4096*v cast to fp16 (the v-bias is added
                    # per-partition in the post-transpose ctxT copy)
                    nc.scalar.activation(
                        vn[:, st, :, 0:D],
                        ps[:].rearrange("p (c d) -> p c d", c=HPC), Ident)

            def rope(tensors):
                # rows 0..31: t = t*cos + rotate_half(t)*sin; the half-swap
                # runs on the PE as a permutation matmul (no DMA latency).
                # ci-major across tensors so early chunks unblock scores fast
                for ci in range(NCH):
                    sl = slice(ci * CHUNK, (ci + 1) * CHUNK)
                    for ti, t in enumerate(tensors):
                        rps = pools[(2 * ci + ti) % 2].tile(
                            [ROT, CHUNK], f32, tag=f"ps{(2 * ci + ti) % 2}",
                            name="rps")
                        nc.tensor.matmul(rps[:], perm[:], t[0:ROT, sl],
                                         start=True, stop=True)
                        rotu = rop.tile([ROT, CHUNK], fp16, tag="rotu")
                        nc.vector.tensor_tensor(rotu[:], rps[:], sinT[:, sl], MULT)
                        nc.vector.tensor_tensor(t[0:ROT, sl], t[0:ROT, sl], cosT[:, sl], MULT)
                        nc.vector.tensor_tensor(t[0:ROT, sl], t[0:ROT, sl], rotu[:], ADD)

            def scores_t(ci, h, t, prs):
                # one transposed scores tile + exp -> fp16 probs tile
                pool = pools[t % 2]
                pss = pool.tile([128, CHUNK], f32, tag=f"ps{t % 2}", name="pss")
                off = (t - 4 * ci) * 128
                lo = max(off, 0)  # cols i < off never consumed
                nc.tensor.matmul(
                    pss[:, lo:], kT[h][:, t * 128:(t + 1) * 128],
                    qT[h][:, ci * CHUNK + lo:(ci + 1) * CHUNK],
                    start=True, stop=(off < 0))
                if off >= 0:
                    # diagonal tile: add causal mask via I @ maskb
                    nc.tensor.matmul(pss[:, off:off + 128], ident[:], maskb[:],
                                     start=False, stop=True)
                pr = prp.tile([128, CHUNK], fp16, tag="probs")
                nc.scalar.activation(pr[:, lo:], pss[:, lo:], Exp, scale=NORM)
                prs.append(pr)

            def ctx_io(ci, h, io, prs):
                # context + denominator; normalize; transpose back via PE.
                # pc rotates over psA/psB (4 chains in flight) so the DVE
                # reciprocal+scale latency never starves the PE.
                it = 4 * ci + io
                pc = pools[io % 2].tile([128, CHUNK], f32, tag=f"ps{io % 2}",
                                        name=f"pc{io}")
                for t in range(it + 1):
                    nc.tensor.matmul(
                        pc[:, 0:D + 1],
                        prs[t][:, io * 128:(io + 1) * 128],
                        vn[:, t, h, :],
                        start=(t == 0), stop=(t == it))
                rec = rcp.tile([128, 1], f32, tag="rec")
                nc.vector.reciprocal(rec[:], pc[:, D:D + 1])
                cn = cnp.tile([128, D], fp16, tag="ctxn")
                nc.vector.tensor_scalar_mul(cn[:], pc[:, 0:D], rec[:, 0:1])
                pt = psD.tile([128, D], fp16, tag="ps3", name="pt")
                nc.tensor.transpose(pt[:], cn[:], ident[:])
                nc.vector.tensor_scalar(
                    ctxT[h][:, it * 128:(it + 1) * 128], pt[:],
                    bvc[:, h:h + 1], None, op0=ADD)

            def ctx(ci, h, prs):
                for io in range(4):
                    ctx_io(ci, h, io, prs)

            def dense_pieces(ci, on_act=False):
                # 16 oc-chain closures for chunk ci's dense s-tiles; callers
                # interleave them between scores tiles to keep the PE fed
                # while the scalar engine drains the exp backlog
                items = []
                state = {}

                def mk(st, oc):
                    def run():
                        if oc == 0:
                            state[st] = stp.tile([128, HID], fp16, tag="stg",
                                                 name=f"stg{st}")
                        stg = state[st]
                        po = psD.tile([128, CHUNK], f32, tag="ps3", name="po")
                        for c in range(HPC):
                            nc.tensor.matmul(
                                po[:], ctxT[c][:, st * 128:(st + 1) * 128],
                                wd[:, c, oc * CHUNK:(oc + 1) * CHUNK],
                                start=(c == 0), stop=(c == HPC - 1))
                        osl = slice(oc * CHUNK, (oc + 1) * CHUNK)
                        if on_act and oc % 2 == 0:
                            nc.scalar.activation(stg[:, osl], po[:], Ident)
                        else:
                            nc.vector.tensor_copy(stg[:, osl], po[:])
                        if on_act and st == NST - 1:
                            nc.sync.dma_start(
                                out_d[st * 128:(st + 1) * 128, osl], stg[:, osl])
                        elif on_act and oc % 2 == 1:
                            hsl = slice((oc - 1) * CHUNK, (oc + 1) * CHUNK)
                            nc.sync.dma_start(
                                out_d[st * 128:(st + 1) * 128, hsl], stg[:, hsl])
                        elif not on_act and oc == NCH - 1:
                            nc.sync.dma_start(
                                out_d[st * 128:(st + 1) * 128, :], stg[:])
                    return run

                for st in range(4 * ci, 4 * ci + 4):
                    for oc in range(NCH):
                        items.append(mk(st, oc))
                return items

            def dense_st(st):
                for item in dense_pieces_for_st(st):
                    item()

            def dense_pieces_for_st(st):
                ci = st // 4
                all_items = dense_pieces(ci)
                return all_items[(st % 4) * NCH:(st % 4 + 1) * NCH]

            # ---- schedule: dense(ci) deferred into chunk ci+1's window so the
            # scalar engine's exp backlog never blocks the PE ----
            qk_proj(0, interleave=True)
            rope([kT[0], qT[0]])
            qk_proj(1, interleave=False)
            rope([kT[1], qT[1]])
            # chunk order 1,2,3,0: the big chunks' exp backlogs overlap
            # mid-kernel compute; tiny chunk 0 (8 exp tiles) lands last so
            # the tail is not exp-bound. dense(prev) fills each window.
            order = [0, 1, 2, 3]
            prs = {}
            for wi, ci in enumerate(order):
                ntile = 4 * ci + 4
                pr0, pr1 = [], []
                prs[ci] = (pr0, pr1)
                for t in range(ntile):
                    scores_t(ci, 0, t, pr0)
                if wi == 0:
                    v_proj(0, 8)
                for t in range(ntile):
                    scores_t(ci, 1, t, pr1)
                if wi == 0:
                    v_proj(8, 16)
                if wi >= 1:
                    for item in dense_pieces(order[wi - 1]):
                        item()
                ctx(ci, 0, pr0)
                if wi < len(order) - 1:
                    ctx(ci, 1, pr1)
                else:
                    # final window: stream each dense s-tile right after its
                    # ctx; copies on the now-empty scalar engine so DVE
                    # stays clear for the recip/scale chain
                    dnl = dense_pieces(ci, on_act=True)
                    for io in range(4):
                        ctx_io(ci, 1, io, pr1)
                        for item in dnl[io * NCH:(io + 1) * NCH]:
                            item()

    nc.compile()
    return nc


def _q8pair(x, scale):
    """Scaled 2-plane e4m3 split: x*scale = hi + lo to ~0.1%."""
    xs = (np.asarray(x, np.float32) * scale).astype(np.float32)
    hi = xs.astype(E4NP)
    lo = (xs - hi.astype(np.float32)).astype(E4NP)
    return hi, lo


def _row_major_128(a, ngroups):
    """[(g p), m] -> [p, (g m)] so DMA runs are >=512B contiguous."""
    g, m = ngroups, a.shape[1]
    return np.ascontiguousarray(
        a.reshape(g, 128, m).transpose(1, 0, 2).reshape(128, g * m))


def _prep_inputs(hidden_states, W_qkv, b_qkv, W_dense, b_dense):
    hid = np.asarray(hidden_states).reshape(S, HID)
    hT = np.ascontiguousarray(hid.T).astype(np.float32)   # [HID, S]
    ht8, hte = _q8pair(hT, SX)
    ht8 = _row_major_128(ht8, NKT)
    hte = _row_major_128(hte, NKT)

    inv_freq = 1.0 / (10000.0 ** (np.arange(0, ROT, 2, dtype=np.float64) / ROT))
    t = np.arange(S, dtype=np.float64)
    freqs = np.outer(t, inv_freq)                      # [s, rot/2]
    emb = np.concatenate([freqs, freqs], axis=1)       # [s, rot]
    cosT = np.ascontiguousarray(np.cos(emb).T).astype(F16NP)
    sinT = np.cos(emb - np.pi / 2).T                   # = sin
    sinTeff = np.concatenate([-sinT[: ROT // 2], sinT[ROT // 2:]], axis=0)
    sinTeff = np.ascontiguousarray(sinTeff).astype(F16NP)

    maskb = np.where(
        np.arange(128)[:, None] > np.arange(128)[None, :], MASK_NEG, 0.0
    ).astype(F16NP)
    ident = np.eye(128, dtype=F16NP)
    # rotate-half permutation: out[r] = t[(r+16) % 32]
    perm = np.zeros((ROT, ROT), F16NP)
    perm[(np.arange(ROT) + ROT // 2) % ROT, np.arange(ROT)] = 1.0

    in_maps = []
    for c in range(NCORES):
        heads = [HPC * c, HPC * c + 1]
        wq = np.concatenate([W_qkv[:, n * 384: n * 384 + 128] for n in heads], 1)
        wk = np.concatenate([W_qkv[:, n * 384 + 128: n * 384 + 256] for n in heads], 1)
        wv = np.concatenate([W_qkv[:, n * 384 + 256: n * 384 + 384] for n in heads], 1)
        wq8, wqe = _q8pair(wq, SWQK)
        wk8, wke = _q8pair(wk, SWQK)
        wv8, wve = _q8pair(wv, SWV)
        bq = np.stack([b_qkv[n * 384: n * 384 + 128] for n in heads], 1)
        bk = np.stack([b_qkv[n * 384 + 128: n * 384 + 256] for n in heads], 1)
        bv = np.concatenate([b_qkv[n * 384 + 256: n * 384 + 384] for n in heads])
        bqk = np.concatenate([bq, bk], axis=1).astype(np.float32)  # [128,4] q0 q1 k0 k1
        bvc = np.stack([bv[0:D], bv[D:2 * D]], 1).astype(np.float32)  # [128, 2]
        wdd = np.asarray(W_dense[c * HPC * D:(c + 1) * HPC * D, :], np.float32)
        in_maps.append({
            "ht8": ht8,
            "hte": hte,
            "wq8": _row_major_128(wq8, NKT),
            "wqe": _row_major_128(wqe, NKT),
            "wk8": _row_major_128(wk8, NKT),
            "wke": _row_major_128(wke, NKT),
            "wv8": _row_major_128(wv8, NKT),
            "wve": _row_major_128(wve, NKT),
            "wd": _row_major_128(wdd.astype(F16NP), HPC),
            "cosT": cosT,
            "sinTeff": sinTeff,
            "maskbias": maskb,
            "ident": ident,
            "perm": perm,
            "bqk": np.ascontiguousarray(bqk),
            "bvc": np.ascontiguousarray(bvc),
        })
    return in_maps


def _reduce(results, inputs):
    partial = np.zeros((S, HID), np.float64)
    for r in results:
        partial += r["partial"].astype(np.float64)
    out = (partial + np.asarray(inputs["b_dense"])[None, :]).astype(np.float32)
    return out.reshape(S, 1, HID)


def _run(inputs, trace=False):
    from concourse.bass_utils import run_bass_kernel_spmd

    if "nc" not in _cache:
        _cache["nc"] = _build_program()
    nc = _cache["nc"]
    in_maps = _prep_inputs(
        inputs["hidden_states"], inputs["W_qkv"], inputs["b_qkv"],
        inputs["W_dense"], inputs["b_dense"],
    )
    res = run_bass_kernel_spmd(nc, in_maps, list(range(NCORES)), trace=trace)
    return _reduce(res.results, inputs), res


def kernel(**inputs):
    out, _ = _run(inputs, trace=False)
    return out


# revision 66
# speedup vs baseline: 1.1120x; 1.0073x over previous
"""GPT-NeoX attention (s=2048, b=1, h=2048, nh=16, hd=128, rot=32) on 8 NeuronCores.

Sharding: tensor-parallel over heads (2 heads per core), row-parallel dense
with host-side partial reduction.

Per core: the QKV projections run as fp8e4 DoubleRow matmuls with 3-term
residual compensation (X@W ~ X8@W8 + Xe@W8 + X8@We, each plane pre-scaled
into e4m3's dynamic range) - 0.75x the bf16 PE cost at ~0.1% error. The
attention core (scores, softmax, context) and the dense matmul run in fp16.
Scores use a transposed layout; context is computed in natural layout with a
ones-column so the softmax denominator falls out of the matmul; a per-row
reciprocal scale normalizes on the vector engine; context transposes back on
the PE for the dense slice. Dense output of chunk i is deferred into chunk
i+1's window so the scalar engine's softmax-exp latency never stalls the PE.
"""

import math
import numpy as np
import ml_dtypes

S = 2048
HID = 2048
NH = 16
D = 128
ROT = 32
NCORES = 8
HPC = 2  # heads per core
CHUNK = 512
NKT = HID // 128  # 16 contraction tiles
NKP = NKT // 2    # 8 DoubleRow k-tile pairs
NCH = S // CHUNK  # 4 i-chunks
NST = S // 128    # 16 s-tiles
NORM = 1.0 / math.sqrt(D)
MASK_NEG = -1000.0 / NORM  # -1000 after the exp scale; exp() underflows to 0

# fp8 plane scales: place values in e4m3's sweet spot (max 240, min normal 2^-6)
SX = 32.0      # hidden-state planes
SWQK = 2048.0  # Wq/Wk planes -> q,k psum at 2^16, descaled in the PSUM->SBUF copy
SWV = 128.0    # Wv planes    -> v psum at 2^12 = 4096*v, cancelled by the ones column
QK_DESCALE = 1.0 / (SX * SWQK)
VSCALE = SX * SWV  # 4096; vn holds 4096*(v+b); ones col = 4096 so cn = ctx

E4NP = ml_dtypes.float8_e4m3
F16NP = np.float16

_cache = {}


def _build_program():
    from concourse import bass, bacc, tile
    from concourse.bass import mybir

    f32 = mybir.dt.float32
    fp16 = mybir.dt.float16
    fp8 = mybir.dt.float8e4
    Exp = mybir.ActivationFunctionType.Exp
    Ident = mybir.ActivationFunctionType.Identity
    ADD = mybir.AluOpType.add
    MULT = mybir.AluOpType.mult
    DR = mybir.MatmulPerfMode.DoubleRow

    nc = bacc.Bacc()

    # all dram tensors laid out [128, free] with >=512B contiguous runs
    ht8_d = nc.dram_tensor("ht8", [128, NKT * S], fp8, kind="ExternalInput")
    hte_d = nc.dram_tensor("hte", [128, NKT * S], fp8, kind="ExternalInput")
    wq8_d = nc.dram_tensor("wq8", [128, NKT * HPC * D], fp8, kind="ExternalInput")
    wqe_d = nc.dram_tensor("wqe", [128, NKT * HPC * D], fp8, kind="ExternalInput")
    wk8_d = nc.dram_tensor("wk8", [128, NKT * HPC * D], fp8, kind="ExternalInput")
    wke_d = nc.dram_tensor("wke", [128, NKT * HPC * D], fp8, kind="ExternalInput")
    wv8_d = nc.dram_tensor("wv8", [128, NKT * HPC * D], fp8, kind="ExternalInput")
    wve_d = nc.dram_tensor("wve", [128, NKT * HPC * D], fp8, kind="ExternalInput")
    wd_d = nc.dram_tensor("wd", [128, HPC * HID], fp16, kind="ExternalInput")
    cos_d = nc.dram_tensor("cosT", [ROT, S], fp16, kind="ExternalInput")
    sin_d = nc.dram_tensor("sinTeff", [ROT, S], fp16, kind="ExternalInput")
    mask_d = nc.dram_tensor("maskbias", [128, 128], fp16, kind="ExternalInput")
    perm_d = nc.dram_tensor("perm", [ROT, ROT], fp16, kind="ExternalInput")
    ident_d = nc.dram_tensor("ident", [128, 128], fp16, kind="ExternalInput")
    bqk_d = nc.dram_tensor("bqk", [128, 4], f32, kind="ExternalInput")
    bvc_d = nc.dram_tensor("bvc", [128, HPC], f32, kind="ExternalInput")
    out_d = nc.dram_tensor("partial", [S, HID], fp16, kind="ExternalOutput")

    with tile.TileContext(nc) as tc:
        with (
            tc.tile_pool(name="persist", bufs=1) as pp,
            tc.tile_pool(name="probs", bufs=40) as prp,
            tc.tile_pool(name="rotu", bufs=2) as rop,
            tc.tile_pool(name="ctxn", bufs=8) as cnp,
            tc.tile_pool(name="rec", bufs=12) as rcp,
            tc.tile_pool(name="stage", bufs=4) as stp,
            tc.tile_pool(name="psA", bufs=3, space="PSUM") as psA,
            tc.tile_pool(name="psB", bufs=2, space="PSUM") as psB,
            tc.tile_pool(name="psD", bufs=3, space="PSUM") as psD,
        ):
            pools = [psA, psB, psD]

            # ---- persistent SBUF tiles ----
            ht8 = pp.tile([128, NKT, S], fp8, tag="ht8")
            hte = pp.tile([128, NKT, S], fp8, tag="hte")
            wq8 = pp.tile([128, NKT, HPC * D], fp8, tag="wq8")
            wqe = pp.tile([128, NKT, HPC * D], fp8, tag="wqe")
            wk8 = pp.tile([128, NKT, HPC * D], fp8, tag="wk8")
            wke = pp.tile([128, NKT, HPC * D], fp8, tag="wke")
            wv8 = pp.tile([128, NKT, HPC * D], fp8, tag="wv8")
            wve = pp.tile([128, NKT, HPC * D], fp8, tag="wve")
            wd = pp.tile([128, HPC, HID], fp16, tag="wd")
            cosT = pp.tile([ROT, S], fp16, tag="cos")
            sinT = pp.tile([ROT, S], fp16, tag="sin")
            maskb = pp.tile([128, 128], fp16, tag="mask")
            perm = pp.tile([ROT, ROT], fp16, tag="perm")
            ident = pp.tile([128, 128], fp16, tag="ident")
            bqk = pp.tile([128, 4], f32, tag="bqk")
            bvc = pp.tile([128, HPC], f32, tag="bvc")
            qT = [pp.tile([128, S], fp16, tag=f"qT{h}", name=f"qT{h}") for h in range(HPC)]
            kT = [pp.tile([128, S], fp16, tag=f"kT{h}", name=f"kT{h}") for h in range(HPC)]
            # V natural layout (both heads) + ones column for the denominator
            vn = pp.tile([128, NST, HPC, D + 1], fp16, tag="vn")
            ctxT = [pp.tile([128, S], fp16, tag=f"ctxT{h}", name=f"ctxT{h}")
                    for h in range(HPC)]

            nc.vector.memset(vn[:, :, :, D:D + 1], VSCALE)

            # warm the activation function table while DMAs stream
            warm = pp.tile([128, 1], f32, tag="warm")
            nc.vector.memset(warm[:], 0.0)
            nc.scalar.activation(warm[:], warm[:], Exp)

            # ---- input DMAs: the first chains' (main/xres) terms need only
            # wk8/wq8 + ht planes; the we planes (wres pass) can arrive late ----
            wk8_r = wk8_d[:].rearrange("p (k m) -> p k m", k=NKT)
            wq8_r = wq8_d[:].rearrange("p (k m) -> p k m", k=NKT)
            ht8_r = ht8_d[:].rearrange("p (k s) -> p k s", k=NKT)
            hte_r = hte_d[:].rearrange("p (k s) -> p k s", k=NKT)
            # k-pair ht DMAs halve the HWDGE issue slots (the scarce
            # resource: ~630ns serialized issue per DMA); small side tensors
            # go through the Pool engine's SWDGE path, which skips HWDGE
            nc.sync.dma_start(wk8[:, 0:4, :], wk8_r[:, 0:4, :])
            nc.sync.dma_start(ht8[:, 0:2, 0:CHUNK], ht8_r[:, 0:2, 0:CHUNK])
            nc.sync.dma_start(wq8[:, 0:4, :], wq8_r[:, 0:4, :])
            nc.sync.dma_start(hte[:, 0:2, 0:CHUNK], hte_r[:, 0:2, 0:CHUNK])
            nc.sync.dma_start(ht8[:, 0:2, CHUNK:], ht8_r[:, 0:2, CHUNK:])
            nc.sync.dma_start(hte[:, 0:2, CHUNK:], hte_r[:, 0:2, CHUNK:])
            for k in range(2, NKT):
                nc.sync.dma_start(ht8[:, k, :], ht8_r[:, k, :])
                nc.sync.dma_start(hte[:, k, :], hte_r[:, k, :])
                if k == 2:
                    nc.sync.dma_start(wke[:], wke_d[:].rearrange("p (k m) -> p k m", k=NKT))
                    nc.sync.dma_start(wqe[:], wqe_d[:].rearrange("p (k m) -> p k m", k=NKT))
                if k == 4:
                    nc.sync.dma_start(wk8[:, 4:, :], wk8_r[:, 4:, :])
                    nc.sync.dma_start(wq8[:, 4:, :], wq8_r[:, 4:, :])
            nc.gpsimd.dma_start(cosT[:], cos_d[:])
            nc.gpsimd.dma_start(sinT[:], sin_d[:])
            nc.gpsimd.dma_start(maskb[:], mask_d[:])
            nc.gpsimd.dma_start(perm[:], perm_d[:])
            nc.gpsimd.dma_start(ident[:], ident_d[:])
            nc.gpsimd.dma_start(bqk[:], bqk_d[:])
            nc.gpsimd.dma_start(bvc[:], bvc_d[:])
            nc.sync.dma_start(wv8[:], wv8_d[:].rearrange("p (k m) -> p k m", k=NKT))
            nc.sync.dma_start(wve[:], wve_d[:].rearrange("p (k m) -> p k m", k=NKT))
            nc.sync.dma_start(wd[:], wd_d[:].rearrange("p (c o) -> p c o", c=HPC))

            def dr3_step(ps, j, w8, we, hcols, sl, first, last):
                # one k-pair step of a 3-term compensated chain
                kk = slice(2 * j, 2 * j + 2)
                nc.tensor.matmul(ps[:], w8[:, kk, hcols], ht8[:, kk, sl],
                                 start=first, stop=False, perf_mode=DR)
                nc.tensor.matmul(ps[:], w8[:, kk, hcols], hte[:, kk, sl],
                                 start=False, stop=False, perf_mode=DR)
                nc.tensor.matmul(ps[:], we[:, kk, hcols], ht8[:, kk, sl],
                                 start=False, stop=last, perf_mode=DR)

            def qk_proj(h, interleave):
                # qT/kT[h][d=128, s]; bias + 2^-16 descale in the PSUM->SBUF copy
                hcols = slice(h * D, (h + 1) * D)
                chains = []
                for ci in range(NCH):
                    sl = slice(ci * CHUNK, (ci + 1) * CHUNK)
                    for (w8, we, dst, bcol) in ((wk8, wke, kT, 2), (wq8, wqe, qT, 0)):
                        chains.append((w8, we, dst, bcol, sl))
                def chain_pool(c):
                    return (psA, "ps0") if c < 3 else (psB, "ps1") if c < 6 else (psD, "ps3")

                if interleave:
                    # j-major across all 8 chains so the PE rides the ht DMA
                    # staircase; the wres term lags 2 j-steps so the we
                    # planes (DMA'd mid-stream) never stall the pipeline
                    LAG = 1
                    pss = [chain_pool(c)[0].tile([128, CHUNK], f32,
                                                  tag=chain_pool(c)[1],
                                                  name=f"pss{c}")
                           for c in range(8)]
                    for j in range(NKP + LAG):
                        if j < NKP:
                            kk = slice(2 * j, 2 * j + 2)
                            for c, (w8, we, dst, bcol, sl) in enumerate(chains):
                                nc.tensor.matmul(pss[c][:], w8[:, kk, hcols],
                                                 ht8[:, kk, sl], start=(j == 0),
                                                 stop=False, perf_mode=DR)
                            for c, (w8, we, dst, bcol, sl) in enumerate(chains):
                                nc.tensor.matmul(pss[c][:], w8[:, kk, hcols],
                                                 hte[:, kk, sl], start=False,
                                                 stop=False, perf_mode=DR)
                        if j >= LAG:
                            jw = j - LAG
                            kk = slice(2 * jw, 2 * jw + 2)
                            for c, (w8, we, dst, bcol, sl) in enumerate(chains):
                                nc.tensor.matmul(pss[c][:], we[:, kk, hcols],
                                                 ht8[:, kk, sl], start=False,
                                                 stop=(jw == NKP - 1),
                                                 perf_mode=DR)
                    for c, (w8, we, dst, bcol, sl) in enumerate(chains):
                        nc.scalar.activation(dst[h][:, sl], pss[c][:], Ident,
                                             bias=bqk[:, bcol + h:bcol + h + 1],
                                             scale=QK_DESCALE)
                else:
                    for c, (w8, we, dst, bcol, sl) in enumerate(chains):
                        ps = chain_pool(c)[0].tile([128, CHUNK], f32,
                                                   tag=chain_pool(c)[1], name="ps")
                        for j in range(NKP):
                            dr3_step(ps, j, w8, we, hcols, sl, j == 0, j == NKP - 1)
                        nc.scalar.activation(dst[h][:, sl], ps[:], Ident,
                                             bias=bqk[:, bcol + h:bcol + h + 1],
                                             scale=QK_DESCALE)

            def v_proj(st_lo, st_hi):
                # vn[s-part, st, h, d] natural layout, both heads per chain
                for st in range(st_lo, st_hi):
                    ssl = slice(st * 128, (st + 1) * 128)
                    vp, vt = [(psA, "ps0"), (psB, "ps1"), (psD, "ps3")][st % 3]
                    ps = vp.tile([128, HPC * D], f32, tag=vt, name="ps")
                    for j in range(NKP):
                        kk = slice(2 * j, 2 * j + 2)
                        nc.tensor.matmul(ps[:], ht8[:, kk, ssl], wv8[:, kk, :],
                                         start=(j == 0), stop=False, perf_mode=DR)
                        nc.tensor.matmul(ps[:], hte[:, kk, ssl], wv8[:, kk, :],
                                         start=False, stop=False, perf_mode=DR)
                        nc.tensor.matmul(ps[:], ht8[:, kk, ssl], wve[:, kk, :],
                                         start=False, stop=(j == NKP - 1), perf_mode=DR)
                    # vn = 4096*v cast to fp16 (the v-bias is added
                    # per-partition in the post-transpose ctxT copy)
                    nc.scalar.activation(
                        vn[:, st, :, 0:D],
                        ps[:].rearrange("p (c d) -> p c d", c=HPC), Ident)

            def rope(tensors):
                # rows 0..31: t = t*cos + rotate_half(t)*sin; the half-swap
                # runs on the PE as a permutation matmul (no DMA latency).
                # ci-major across tensors so early chunks unblock scores fast
                for ci in range(NCH):
                    sl = slice(ci * CHUNK, (ci + 1) * CHUNK)
                    for ti, t in enumerate(tensors):
                        rps = pools[(2 * ci + ti) % 2].tile(
                            [ROT, CHUNK], f32, tag=f"ps{(2 * ci + ti) % 2}",
                            name="rps")
                        nc.tensor.matmul(rps[:], perm[:], t[0:ROT, sl],
                                         start=True, stop=True)
                        rotu = rop.tile([ROT, CHUNK], fp16, tag="rotu")
                        nc.vector.tensor_tensor(rotu[:], rps[:], sinT[:, sl], MULT)
                        nc.vector.tensor_tensor(t[0:ROT, sl], t[0:ROT, sl], cosT[:, sl], MULT)
                        nc.vector.tensor_tensor(t[0:ROT, sl], t[0:ROT, sl], rotu[:], ADD)

            def scores_t(ci, h, t, prs):
                # one transposed scores tile + exp -> fp16 probs tile
                pool = pools[t % 2]
                pss = pool.tile([128, CHUNK], f32, tag=f"ps{t % 2}", name="pss")
                off = (t - 4 * ci) * 128
                lo = max(off, 0)  # cols i < off never consumed
                nc.tensor.matmul(
                    pss[:, lo:], kT[h][:, t * 128:(t + 1) * 128],
                    qT[h][:, ci * CHUNK + lo:(ci + 1) * CHUNK],
                    start=True, stop=(off < 0))
                if off >= 0:
                    # diagonal tile: add causal mask via I @ maskb
                    nc.tensor.matmul(pss[:, off:off + 128], ident[:], maskb[:],
                                     start=False, stop=True)
                pr = prp.tile([128, CHUNK], fp16, tag="probs")
                nc.scalar.activation(pr[:, lo:], pss[:, lo:], Exp, scale=NORM)
                prs.append(pr)

            def ctx_io(ci, h, io, prs):
                # context + denominator; normalize; transpose back via PE.
                # pc rotates over psA/psB (4 chains in flight) so the DVE
                # reciprocal+scale latency never starves the PE.
                it = 4 * ci + io
                pc = pools[io % 2].tile([128, CHUNK], f32, tag=f"ps{io % 2}",
                                        name=f"pc{io}")
                for t in range(it + 1):
                    nc.tensor.matmul(
                        pc[:, 0:D + 1],
                        prs[t][:, io * 128:(io + 1) * 128],
                        vn[:, t, h, :],
                        start=(t == 0), stop=(t == it))
                rec = rcp.tile([128, 1], f32, tag="rec")
                nc.vector.reciprocal(rec[:], pc[:, D:D + 1])
                cn = cnp.tile([128, D], fp16, tag="ctxn")
                nc.vector.tensor_scalar_mul(cn[:], pc[:, 0:D], rec[:, 0:1])
                pt = psD.tile([128, D], fp16, tag="ps3", name="pt")
                nc.tensor.transpose(pt[:], cn[:], ident[:])
                nc.vector.tensor_scalar(
                    ctxT[h][:, it * 128:(it + 1) * 128], pt[:],
                    bvc[:, h:h + 1], None, op0=ADD)

            def ctx(ci, h, prs):
                for io in range(4):
                    ctx_io(ci, h, io, prs)

            def dense_pieces(ci, on_act=False):
                # 16 oc-chain closures for chunk ci's dense s-tiles; callers
                # interleave them between scores tiles to keep the PE fed
                # while the scalar engine drains the exp backlog
                items = []
                state = {}

                def mk(st, oc):
                    def run():
                        if oc == 0:
                            state[st] = stp.tile([128, HID], fp16, tag="stg",
                                                 name=f"stg{st}")
                        stg = state[st]
                        po = psD.tile([128, CHUNK], f32, tag="ps3", name="po")
                        for c in range(HPC):
                            nc.tensor.matmul(
                                po[:], ctxT[c][:, st * 128:(st + 1) * 128],
                                wd[:, c, oc * CHUNK:(oc + 1) * CHUNK],
                                start=(c == 0), stop=(c == HPC - 1))
                        osl = slice(oc * CHUNK, (oc + 1) * CHUNK)
                        if on_act and oc % 2 == 0:
                            nc.scalar.activation(stg[:, osl], po[:], Ident)
                        else:
                            nc.vector.tensor_copy(stg[:, osl], po[:])
                        if on_act and oc % 2 == 1:
                            hsl = slice((oc - 1) * CHUNK, (oc + 1) * CHUNK)
                            nc.sync.dma_start(
                                out_d[st * 128:(st + 1) * 128, hsl], stg[:, hsl])
                        elif not on_act and oc == NCH - 1:
                            nc.sync.dma_start(
                                out_d[st * 128:(st + 1) * 128, :], stg[:])
                    return run

                for st in range(4 * ci, 4 * ci + 4):
                    for oc in range(NCH):
                        items.append(mk(st, oc))
                return items

            def dense_st(st):
                for item in dense_pieces_for_st(st):
                    item()

            def dense_pieces_for_st(st):
                ci = st // 4
                all_items = dense_pieces(ci)
                return all_items[(st % 4) * NCH:(st % 4 + 1) * NCH]

            # ---- schedule: dense(ci) deferred into chunk ci+1's window so the
            # scalar engine's exp backlog never blocks the PE ----
            qk_proj(0, interleave=True)
            rope([kT[0], qT[0]])
            qk_proj(1, interleave=False)
            rope([kT[1], qT[1]])
            # chunk order 1,2,3,0: the big chunks' exp backlogs overlap
            # mid-kernel compute; tiny chunk 0 (8 exp tiles) lands last so
            # the tail is not exp-bound. dense(prev) fills each window.
            order = [0, 1, 2, 3]
            prs = {}
            for wi, ci in enumerate(order):
                ntile = 4 * ci + 4
                pr0, pr1 = [], []
                prs[ci] = (pr0, pr1)
                for t in range(ntile):
                    scores_t(ci, 0, t, pr0)
                if wi == 0:
                    v_proj(0, 8)
                for t in range(ntile):
                    scores_t(ci, 1, t, pr1)
                if wi == 0:
                    v_proj(8, 16)
                if wi >= 1:
                    for item in dense_pieces(order[wi - 1]):
                        item()
                ctx(ci, 0, pr0)
                if wi < len(order) - 1:
                    ctx(ci, 1, pr1)
                else:
                    # final window: stream each dense s-tile right after its
                    # ctx; copies on the now-empty scalar engine so DVE
                    # stays clear for the recip/scale chain
                    dnl = dense_pieces(ci, on_act=True)
                    for io in range(4):
                        ctx_io(ci, 1, io, pr1)
                        for item in dnl[io * NCH:(io + 1) * NCH]:
                            item()

    nc.compile()
    return nc


def _q8pair(x, scale):
    """Scaled 2-plane e4m3 split: x*scale = hi + lo to ~0.1%."""
    xs = (np.asarray(x, np.float32) * scale).astype(np.float32)
    hi = xs.astype(E4NP)
    lo = (xs - hi.astype(np.float32)).astype(E4NP)
    return hi, lo


def _row_major_128(a, ngroups):
    """[(g p), m] -> [p, (g m)] so DMA runs are >=512B contiguous."""
    g, m = ngroups, a.shape[1]
    return np.ascontiguousarray(
        a.reshape(g, 128, m).transpose(1, 0, 2).reshape(128, g * m))


def _prep_inputs(hidden_states, W_qkv, b_qkv, W_dense, b_dense):
    hid = np.asarray(hidden_states).reshape(S, HID)
    hT = np.ascontiguousarray(hid.T).astype(np.float32)   # [HID, S]
    ht8, hte = _q8pair(hT, SX)
    ht8 = _row_major_128(ht8, NKT)
    hte = _row_major_128(hte, NKT)

    inv_freq = 1.0 / (10000.0 ** (np.arange(0, ROT, 2, dtype=np.float64) / ROT))
    t = np.arange(S, dtype=np.float64)
    freqs = np.outer(t, inv_freq)                      # [s, rot/2]
    emb = np.concatenate([freqs, freqs], axis=1)       # [s, rot]
    cosT = np.ascontiguousarray(np.cos(emb).T).astype(F16NP)
    sinT = np.cos(emb - np.pi / 2).T                   # = sin
    sinTeff = np.concatenate([-sinT[: ROT // 2], sinT[ROT // 2:]], axis=0)
    sinTeff = np.ascontiguousarray(sinTeff).astype(F16NP)

    maskb = np.where(
        np.arange(128)[:, None] > np.arange(128)[None, :], MASK_NEG, 0.0
    ).astype(F16NP)
    ident = np.eye(128, dtype=F16NP)
    # rotate-half permutation: out[r] = t[(r+16) % 32]
    perm = np.zeros((ROT, ROT), F16NP)
    perm[(np.arange(ROT) + ROT // 2) % ROT, np.arange(ROT)] = 1.0

    in_maps = []
    for c in range(NCORES):
        heads = [HPC * c, HPC * c + 1]
        wq = np.concatenate([W_qkv[:, n * 384: n * 384 + 128] for n in heads], 1)
        wk = np.concatenate([W_qkv[:, n * 384 + 128: n * 384 + 256] for n in heads], 1)
        wv = np.concatenate([W_qkv[:, n * 384 + 256: n * 384 + 384] for n in heads], 1)
        wq8, wqe = _q8pair(wq, SWQK)
        wk8, wke = _q8pair(wk, SWQK)
        wv8, wve = _q8pair(wv, SWV)
        bq = np.stack([b_qkv[n * 384: n * 384 + 128] for n in heads], 1)
        bk = np.stack([b_qkv[n * 384 + 128: n * 384 + 256] for n in heads], 1)
        bv = np.concatenate([b_qkv[n * 384 + 256: n * 384 + 384] for n in heads])
        bqk = np.concatenate([bq, bk], axis=1).astype(np.float32)  # [128,4] q0 q1 k0 k1
        bvc = np.stack([bv[0:D], bv[D:2 * D]], 1).astype(np.float32)  # [128, 2]
        wdd = np.asarray(W_dense[c * HPC * D:(c + 1) * HPC * D, :], np.float32)
        in_maps.append({
            "ht8": ht8,
            "hte": hte,
            "wq8": _row_major_128(wq8, NKT),
            "wqe": _row_major_128(wqe, NKT),
            "wk8": _row_major_128(wk8, NKT),
            "wke": _row_major_128(wke, NKT),
            "wv8": _row_major_128(wv8, NKT),
            "wve": _row_major_128(wve, NKT),
            "wd": _row_major_128(wdd.astype(F16NP), HPC),
            "cosT": cosT,
            "sinTeff": sinTeff,
            "maskbias": maskb,
            "ident": ident,
            "perm": perm,
            "bqk": np.ascontiguousarray(bqk),
            "bvc": np.ascontiguousarray(bvc),
        })
    return in_maps


def _reduce(results, inputs):
    partial = np.zeros((S, HID), np.float64)
    for r in results:
        partial += r["partial"].astype(np.float64)
    out = (partial + np.asarray(inputs["b_dense"])[None, :]).astype(np.float32)
    return out.reshape(S, 1, HID)


def _run(inputs, trace=False):
    from concourse.bass_utils import run_bass_kernel_spmd

    if "nc" not in _cache:
        _cache["nc"] = _build_program()
    nc = _cache["nc"]
    in_maps = _prep_inputs(
        inputs["hidden_states"], inputs["W_qkv"], inputs["b_qkv"],
        inputs["W_dense"], inputs["b_dense"],
    )
    res = run_bass_kernel_spmd(nc, in_maps, list(range(NCORES)), trace=trace)
    return _reduce(res.results, inputs), res


def kernel(**inputs):
    out, _ = _run(inputs, trace=False)
    return out


# revision 73
# speedup vs baseline: 1.1129x; 1.0008x over previous
"""GPT-NeoX attention (s=2048, b=1, h=2048, nh=16, hd=128, rot=32) on 8 NeuronCores.

Sharding: tensor-parallel over heads (2 heads per core), row-parallel dense
with host-side partial reduction.

Per core: the QKV projections run as fp8e4 DoubleRow matmuls with 3-term
residual compensation (X@W ~ X8@W8 + Xe@W8 + X8@We, each plane pre-scaled
into e4m3's dynamic range) - 0.75x the bf16 PE cost at ~0.1% error. The
attention core (scores, softmax, context) and the dense matmul run in fp16.
Scores use a transposed layout; context is computed in natural layout with a
ones-column so the softmax denominator falls out of the matmul; a per-row
reciprocal scale normalizes on the vector engine; context transposes back on
the PE for the dense slice. Dense output of chunk i is deferred into chunk
i+1's window so the scalar engine's softmax-exp latency never stalls the PE.
"""

import math
import numpy as np
import ml_dtypes

S = 2048
HID = 2048
NH = 16
D = 128
ROT = 32
NCORES = 8
HPC = 2  # heads per core
CHUNK = 512
NKT = HID // 128  # 16 contraction tiles
NKP = NKT // 2    # 8 DoubleRow k-tile pairs
NCH = S // CHUNK  # 4 i-chunks
NST = S // 128    # 16 s-tiles
NORM = 1.0 / math.sqrt(D)
MASK_NEG = -1000.0 / NORM  # -1000 after the exp scale; exp() underflows to 0

# fp8 plane scales: place values in e4m3's sweet spot (max 240, min normal 2^-6)
SX = 32.0      # hidden-state planes
SWQK = 2048.0  # Wq/Wk planes -> q,k psum at 2^16, descaled in the PSUM->SBUF copy
SWV = 128.0    # Wv planes    -> v psum at 2^12 = 4096*v, cancelled by the ones column
QK_DESCALE = 1.0 / (SX * SWQK)
VSCALE = SX * SWV  # 4096; vn holds 4096*(v+b); ones col = 4096 so cn = ctx

E4NP = ml_dtypes.float8_e4m3
F16NP = np.float16

_cache = {}


def _build_program():
    from concourse import bass, bacc, tile
    from concourse.bass import mybir

    f32 = mybir.dt.float32
    fp16 = mybir.dt.float16
    fp8 = mybir.dt.float8e4
    Exp = mybir.ActivationFunctionType.Exp
    Ident = mybir.ActivationFunctionType.Identity
    ADD = mybir.AluOpType.add
    MULT = mybir.AluOpType.mult
    DR = mybir.MatmulPerfMode.DoubleRow

    nc = bacc.Bacc()

    # all dram tensors laid out [128, free] with >=512B contiguous runs
    ht8_d = nc.dram_tensor("ht8", [128, NKT * S], fp8, kind="ExternalInput")
    hte_d = nc.dram_tensor("hte", [128, NKT * S], fp8, kind="ExternalInput")
    wq8_d = nc.dram_tensor("wq8", [128, NKT * HPC * D], fp8, kind="ExternalInput")
    wqe_d = nc.dram_tensor("wqe", [128, NKT * HPC * D], fp8, kind="ExternalInput")
    wk8_d = nc.dram_tensor("wk8", [128, NKT * HPC * D], fp8, kind="ExternalInput")
    wke_d = nc.dram_tensor("wke", [128, NKT * HPC * D], fp8, kind="ExternalInput")
    wv8_d = nc.dram_tensor("wv8", [128, NKT * HPC * D], fp8, kind="ExternalInput")
    wve_d = nc.dram_tensor("wve", [128, NKT * HPC * D], fp8, kind="ExternalInput")
    wd_d = nc.dram_tensor("wd", [128, HPC * HID], fp16, kind="ExternalInput")
    cos_d = nc.dram_tensor("cosT", [ROT, S], fp16, kind="ExternalInput")
    sin_d = nc.dram_tensor("sinTeff", [ROT, S], fp16, kind="ExternalInput")
    mask_d = nc.dram_tensor("maskbias", [128, 128], fp16, kind="ExternalInput")
    perm_d = nc.dram_tensor("perm", [ROT, ROT], fp16, kind="ExternalInput")
    ident_d = nc.dram_tensor("ident", [128, 128], fp16, kind="ExternalInput")
    bqk_d = nc.dram_tensor("bqk", [128, 4], f32, kind="ExternalInput")
    bvc_d = nc.dram_tensor("bvc", [128, HPC], f32, kind="ExternalInput")
    out_d = nc.dram_tensor("partial", [S, HID], fp16, kind="ExternalOutput")

    with tile.TileContext(nc) as tc:
        with (
            tc.tile_pool(name="persist", bufs=1) as pp,
            tc.tile_pool(name="probs", bufs=40) as prp,
            tc.tile_pool(name="rotu", bufs=2) as rop,
            tc.tile_pool(name="ctxn", bufs=8) as cnp,
            tc.tile_pool(name="rec", bufs=12) as rcp,
            tc.tile_pool(name="stage", bufs=4) as stp,
            tc.tile_pool(name="psA", bufs=3, space="PSUM") as psA,
            tc.tile_pool(name="psB", bufs=2, space="PSUM") as psB,
            tc.tile_pool(name="psD", bufs=3, space="PSUM") as psD,
        ):
            pools = [psA, psB, psD]

            # ---- persistent SBUF tiles ----
            ht8 = pp.tile([128, NKT, S], fp8, tag="ht8")
            hte = pp.tile([128, NKT, S], fp8, tag="hte")
            wq8 = pp.tile([128, NKT, HPC * D], fp8, tag="wq8")
            wqe = pp.tile([128, NKT, HPC * D], fp8, tag="wqe")
            wk8 = pp.tile([128, NKT, HPC * D], fp8, tag="wk8")
            wke = pp.tile([128, NKT, HPC * D], fp8, tag="wke")
            wv8 = pp.tile([128, NKT, HPC * D], fp8, tag="wv8")
            wve = pp.tile([128, NKT, HPC * D], fp8, tag="wve")
            wd = pp.tile([128, HPC, HID], fp16, tag="wd")
            cosT = pp.tile([ROT, S], fp16, tag="cos")
            sinT = pp.tile([ROT, S], fp16, tag="sin")
            maskb = pp.tile([128, 128], fp16, tag="mask")
            perm = pp.tile([ROT, ROT], fp16, tag="perm")
            ident = pp.tile([128, 128], fp16, tag="ident")
            bqk = pp.tile([128, 4], f32, tag="bqk")
            bvc = pp.tile([128, HPC], f32, tag="bvc")
            qT = [pp.tile([128, S], fp16, tag=f"qT{h}", name=f"qT{h}") for h in range(HPC)]
            kT = [pp.tile([128, S], fp16, tag=f"kT{h}", name=f"kT{h}") for h in range(HPC)]
            # V natural layout (both heads) + ones column for the denominator
            vn = pp.tile([128, NST, HPC, D + 1], fp16, tag="vn")
            ctxT = [pp.tile([128, S], fp16, tag=f"ctxT{h}", name=f"ctxT{h}")
                    for h in range(HPC)]

            nc.vector.memset(vn[:, :, :, D:D + 1], VSCALE)

            # warm the activation function table while DMAs stream
            warm = pp.tile([128, 1], f32, tag="warm")
            nc.vector.memset(warm[:], 0.0)
            nc.scalar.activation(warm[:], warm[:], Exp)

            # ---- input DMAs: the first chains' (main/xres) terms need only
            # wk8/wq8 + ht planes; the we planes (wres pass) can arrive late ----
            wk8_r = wk8_d[:].rearrange("p (k m) -> p k m", k=NKT)
            wq8_r = wq8_d[:].rearrange("p (k m) -> p k m", k=NKT)
            ht8_r = ht8_d[:].rearrange("p (k s) -> p k s", k=NKT)
            hte_r = hte_d[:].rearrange("p (k s) -> p k s", k=NKT)
            # k-pair ht DMAs halve the HWDGE issue slots (the scarce
            # resource: ~630ns serialized issue per DMA); small side tensors
            # go through the Pool engine's SWDGE path, which skips HWDGE
            nc.sync.dma_start(wk8[:, 0:4, :], wk8_r[:, 0:4, :])
            nc.sync.dma_start(ht8[:, 0:2, 0:CHUNK], ht8_r[:, 0:2, 0:CHUNK])
            nc.sync.dma_start(wq8[:, 0:4, :], wq8_r[:, 0:4, :])
            nc.sync.dma_start(hte[:, 0:2, 0:CHUNK], hte_r[:, 0:2, 0:CHUNK])
            nc.sync.dma_start(ht8[:, 0:2, CHUNK:], ht8_r[:, 0:2, CHUNK:])
            nc.sync.dma_start(hte[:, 0:2, CHUNK:], hte_r[:, 0:2, CHUNK:])
            for k in range(2, NKT):
                nc.sync.dma_start(ht8[:, k, :], ht8_r[:, k, :])
                nc.sync.dma_start(hte[:, k, :], hte_r[:, k, :])
                if k == 2:
                    nc.sync.dma_start(wke[:], wke_d[:].rearrange("p (k m) -> p k m", k=NKT))
                    nc.sync.dma_start(wqe[:], wqe_d[:].rearrange("p (k m) -> p k m", k=NKT))
                if k == 4:
                    nc.sync.dma_start(wk8[:, 4:, :], wk8_r[:, 4:, :])
                    nc.sync.dma_start(wq8[:, 4:, :], wq8_r[:, 4:, :])
            nc.gpsimd.dma_start(cosT[:], cos_d[:])
            nc.gpsimd.dma_start(sinT[:], sin_d[:])
            nc.gpsimd.dma_start(maskb[:], mask_d[:])
            nc.gpsimd.dma_start(perm[:], perm_d[:])
            nc.gpsimd.dma_start(ident[:], ident_d[:])
            nc.gpsimd.dma_start(bqk[:], bqk_d[:])
            nc.gpsimd.dma_start(bvc[:], bvc_d[:])
            nc.sync.dma_start(wv8[:], wv8_d[:].rearrange("p (k m) -> p k m", k=NKT))
            nc.sync.dma_start(wve[:], wve_d[:].rearrange("p (k m) -> p k m", k=NKT))
            nc.sync.dma_start(wd[:], wd_d[:].rearrange("p (c o) -> p c o", c=HPC))

            def dr3_step(ps, j, w8, we, hcols, sl, first, last):
                # one k-pair step of a 3-term compensated chain
                kk = slice(2 * j, 2 * j + 2)
                nc.tensor.matmul(ps[:], w8[:, kk, hcols], ht8[:, kk, sl],
                                 start=first, stop=False, perf_mode=DR)
                nc.tensor.matmul(ps[:], w8[:, kk, hcols], hte[:, kk, sl],
                                 start=False, stop=False, perf_mode=DR)
                nc.tensor.matmul(ps[:], we[:, kk, hcols], ht8[:, kk, sl],
                                 start=False, stop=last, perf_mode=DR)

            def qk_proj(h, interleave):
                # qT/kT[h][d=128, s]; bias + 2^-16 descale in the PSUM->SBUF copy
                hcols = slice(h * D, (h + 1) * D)
                chains = []
                for ci in range(NCH):
                    sl = slice(ci * CHUNK, (ci + 1) * CHUNK)
                    for (w8, we, dst, bcol) in ((wk8, wke, kT, 2), (wq8, wqe, qT, 0)):
                        chains.append((w8, we, dst, bcol, sl))
                def chain_pool(c):
                    return (psA, "ps0") if c < 3 else (psB, "ps1") if c < 6 else (psD, "ps3")

                if interleave:
                    # j-major across all 8 chains so the PE rides the ht DMA
                    # staircase; the wres term lags 2 j-steps so the we
                    # planes (DMA'd mid-stream) never stall the pipeline
                    LAG = 1
                    pss = [chain_pool(c)[0].tile([128, CHUNK], f32,
                                                  tag=chain_pool(c)[1],
                                                  name=f"pss{c}")
                           for c in range(8)]
                    for j in range(NKP + LAG):
                        if j < NKP:
                            kk = slice(2 * j, 2 * j + 2)
                            for c, (w8, we, dst, bcol, sl) in enumerate(chains):
                                nc.tensor.matmul(pss[c][:], w8[:, kk, hcols],
                                                 ht8[:, kk, sl], start=(j == 0),
                                                 stop=False, perf_mode=DR)
                            for c, (w8, we, dst, bcol, sl) in enumerate(chains):
                                nc.tensor.matmul(pss[c][:], w8[:, kk, hcols],
                                                 hte[:, kk, sl], start=False,
                                                 stop=False, perf_mode=DR)
                        if j >= LAG:
                            jw = j - LAG
                            kk = slice(2 * jw, 2 * jw + 2)
                            for c, (w8, we, dst, bcol, sl) in enumerate(chains):
                                nc.tensor.matmul(pss[c][:], we[:, kk, hcols],
                                                 ht8[:, kk, sl], start=False,
                                                 stop=(jw == NKP - 1),
                                                 perf_mode=DR)
                    for c, (w8, we, dst, bcol, sl) in enumerate(chains):
                        nc.scalar.activation(dst[h][:, sl], pss[c][:], Ident,
                                             bias=bqk[:, bcol + h:bcol + h + 1],
                                             scale=QK_DESCALE)
                else:
                    for c, (w8, we, dst, bcol, sl) in enumerate(chains):
                        ps = chain_pool(c)[0].tile([128, CHUNK], f32,
                                                   tag=chain_pool(c)[1], name="ps")
                        for j in range(NKP):
                            dr3_step(ps, j, w8, we, hcols, sl, j == 0, j == NKP - 1)
                        nc.scalar.activation(dst[h][:, sl], ps[:], Ident,
                                             bias=bqk[:, bcol + h:bcol + h + 1],
                                             scale=QK_DESCALE)

            def v_proj(st_lo, st_hi):
                # vn[s-part, st, h, d] natural layout, both heads per chain
                for st in range(st_lo, st_hi):
                    ssl = slice(st * 128, (st + 1) * 128)
                    vp, vt = [(psA, "ps0"), (psB, "ps1"), (psD, "ps3")][st % 3]
                    ps = vp.tile([128, HPC * D], f32, tag=vt, name="ps")
                    for j in range(NKP):
                        kk = slice(2 * j, 2 * j + 2)
                        nc.tensor.matmul(ps[:], ht8[:, kk, ssl], wv8[:, kk, :],
                                         start=(j == 0), stop=False, perf_mode=DR)
                        nc.tensor.matmul(ps[:], hte[:, kk, ssl], wv8[:, kk, :],
                                         start=False, stop=False, perf_mode=DR)
                        nc.tensor.matmul(ps[:], ht8[:, kk, ssl], wve[:, kk, :],
                                         start=False, stop=(j == NKP - 1), perf_mode=DR)
                    # vn = 4096*v cast to fp16 (the v-bias is added
                    # per-partition in the post-transpose ctxT copy)
                    nc.scalar.activation(
                        vn[:, st, :, 0:D],
                        ps[:].rearrange("p (c d) -> p c d", c=HPC), Ident)

            def rope(tensors):
                # rows 0..31: t = t*cos + rotate_half(t)*sin; the half-swap
                # runs on the PE as a permutation matmul (no DMA latency).
                # ci-major across tensors so early chunks unblock scores fast
                for ci in range(NCH):
                    sl = slice(ci * CHUNK, (ci + 1) * CHUNK)
                    for ti, t in enumerate(tensors):
                        rps = pools[(2 * ci + ti) % 2].tile(
                            [ROT, CHUNK], f32, tag=f"ps{(2 * ci + ti) % 2}",
                            name="rps")
                        nc.tensor.matmul(rps[:], perm[:], t[0:ROT, sl],
                                         start=True, stop=True)
                        rotu = rop.tile([ROT, CHUNK], fp16, tag="rotu")
                        nc.vector.tensor_tensor(rotu[:], rps[:], sinT[:, sl], MULT)
                        nc.vector.tensor_tensor(t[0:ROT, sl], t[0:ROT, sl], cosT[:, sl], MULT)
                        nc.vector.tensor_tensor(t[0:ROT, sl], t[0:ROT, sl], rotu[:], ADD)

            def scores_t(ci, h, t, prs):
                # one transposed scores tile + exp -> fp16 probs tile
                pool = pools[t % 2]
                pss = pool.tile([128, CHUNK], f32, tag=f"ps{t % 2}", name="pss")
                off = (t - 4 * ci) * 128
                lo = max(off, 0)  # cols i < off never consumed
                nc.tensor.matmul(
                    pss[:, lo:], kT[h][:, t * 128:(t + 1) * 128],
                    qT[h][:, ci * CHUNK + lo:(ci + 1) * CHUNK],
                    start=True, stop=(off < 0))
                if off >= 0:
                    # diagonal tile: add causal mask via I @ maskb
                    nc.tensor.matmul(pss[:, off:off + 128], ident[:], maskb[:],
                                     start=False, stop=True)
                pr = prp.tile([128, CHUNK], fp16, tag="probs")
                nc.scalar.activation(pr[:, lo:], pss[:, lo:], Exp, scale=NORM)
                prs.append(pr)

            def ctx_io(ci, h, io, prs):
                # context + denominator; normalize; transpose back via PE.
                # pc rotates over psA/psB (4 chains in flight) so the DVE
                # reciprocal+scale latency never starves the PE.
                it = 4 * ci + io
                pc = pools[io % 2].tile([128, CHUNK], f32, tag=f"ps{io % 2}",
                                        name=f"pc{io}")
                for t in range(it + 1):
                    nc.tensor.matmul(
                        pc[:, 0:D + 1],
                        prs[t][:, io * 128:(io + 1) * 128],
                        vn[:, t, h, :],
                        start=(t == 0), stop=(t == it))
                rec = rcp.tile([128, 1], f32, tag="rec")
                nc.vector.reciprocal(rec[:], pc[:, D:D + 1])
                cn = cnp.tile([128, D], fp16, tag="ctxn")
                nc.vector.tensor_scalar_mul(cn[:], pc[:, 0:D], rec[:, 0:1])
                pt = psD.tile([128, D], fp16, tag="ps3", name="pt")
                nc.tensor.transpose(pt[:], cn[:], ident[:])
                nc.vector.tensor_scalar(
                    ctxT[h][:, it * 128:(it + 1) * 128], pt[:],
                    bvc[:, h:h + 1], None, op0=ADD)

            def ctx(ci, h, prs):
                for io in range(4):
                    ctx_io(ci, h, io, prs)

            def dense_pieces(ci, on_act=False):
                # 16 oc-chain closures for chunk ci's dense s-tiles; callers
                # interleave them between scores tiles to keep the PE fed
                # while the scalar engine drains the exp backlog
                items = []
                state = {}

                def mk(st, oc):
                    def run():
                        if oc == 0:
                            state[st] = stp.tile([128, HID], fp16, tag="stg",
                                                 name=f"stg{st}")
                        stg = state[st]
                        po = psD.tile([128, CHUNK], f32, tag="ps3", name="po")
                        for c in range(HPC):
                            nc.tensor.matmul(
                                po[:], ctxT[c][:, st * 128:(st + 1) * 128],
                                wd[:, c, oc * CHUNK:(oc + 1) * CHUNK],
                                start=(c == 0), stop=(c == HPC - 1))
                        osl = slice(oc * CHUNK, (oc + 1) * CHUNK)
                        if on_act and oc % 2 == 0:
                            nc.scalar.activation(stg[:, osl], po[:], Ident)
                        else:
                            nc.vector.tensor_copy(stg[:, osl], po[:])
                        if on_act and oc % 2 == 1:
                            hsl = slice((oc - 1) * CHUNK, (oc + 1) * CHUNK)
                            nc.sync.dma_start(
                                out_d[st * 128:(st + 1) * 128, hsl], stg[:, hsl])
                        elif not on_act and oc == NCH - 1:
                            nc.sync.dma_start(
                                out_d[st * 128:(st + 1) * 128, :], stg[:])
                    return run

                for st in range(4 * ci, 4 * ci + 4):
                    for oc in range(NCH):
                        items.append(mk(st, oc))
                return items

            def dense_st(st):
                for item in dense_pieces_for_st(st):
                    item()

            def dense_pieces_for_st(st):
                ci = st // 4
                all_items = dense_pieces(ci)
                return all_items[(st % 4) * NCH:(st % 4 + 1) * NCH]

            # ---- schedule: dense(ci) deferred into chunk ci+1's window so the
            # scalar engine's exp backlog never blocks the PE ----
            qk_proj(0, interleave=True)
            rope([kT[0], qT[0]])
            qk_proj(1, interleave=False)
            rope([kT[1], qT[1]])
            # chunk order 1,2,3,0: the big chunks' exp backlogs overlap
            # mid-kernel compute; tiny chunk 0 (8 exp tiles) lands last so
            # the tail is not exp-bound. dense(prev) fills each window.
            order = [0, 1, 2, 3]
            prs = {}
            for wi, ci in enumerate(order):
                ntile = 4 * ci + 4
                pr0, pr1 = [], []
                prs[ci] = (pr0, pr1)
                for t in range(ntile):
                    scores_t(ci, 0, t, pr0)
                    if wi == 0:
                        v_proj(2 * t, 2 * t + 2)
                for t in range(ntile):
                    scores_t(ci, 1, t, pr1)
                    if wi == 0:
                        v_proj(8 + 2 * t, 10 + 2 * t)
                if wi >= 1:
                    for item in dense_pieces(order[wi - 1]):
                        item()
                if wi < len(order) - 1:
                    ctx(ci, 0, pr0)
                    ctx(ci, 1, pr1)
                if wi == len(order) - 1:
                    # final window: stream each dense s-tile right after its
                    # ctx; copies on the now-empty scalar engine so DVE
                    # stays clear for the recip/scale chain
                    ctx(ci, 0, pr0)
                    dnl = dense_pieces(ci, on_act=True)
                    for io in range(4):
                        ctx_io(ci, 1, io, pr1)
                        for item in dnl[io * NCH:(io + 1) * NCH]:
                            item()

    nc.compile()
    return nc


def _q8pair(x, scale):
    """Scaled 2-plane e4m3 split: x*scale = hi + lo to ~0.1%."""
    xs = (np.asarray(x, np.float32) * scale).astype(np.float32)
    hi = xs.astype(E4NP)
    lo = (xs - hi.astype(np.float32)).astype(E4NP)
    return hi, lo


def _row_major_128(a, ngroups):
    """[(g p), m] -> [p, (g m)] so DMA runs are >=512B contiguous."""
    g, m = ngroups, a.shape[1]
    return np.ascontiguousarray(
        a.reshape(g, 128, m).transpose(1, 0, 2).reshape(128, g * m))


def _prep_inputs(hidden_states, W_qkv, b_qkv, W_dense, b_dense):
    hid = np.asarray(hidden_states).reshape(S, HID)
    hT = np.ascontiguousarray(hid.T).astype(np.float32)   # [HID, S]
    ht8, hte = _q8pair(hT, SX)
    ht8 = _row_major_128(ht8, NKT)
    hte = _row_major_128(hte, NKT)

    inv_freq = 1.0 / (10000.0 ** (np.arange(0, ROT, 2, dtype=np.float64) / ROT))
    t = np.arange(S, dtype=np.float64)
    freqs = np.outer(t, inv_freq)                      # [s, rot/2]
    emb = np.concatenate([freqs, freqs], axis=1)       # [s, rot]
    cosT = np.ascontiguousarray(np.cos(emb).T).astype(F16NP)
    sinT = np.cos(emb - np.pi / 2).T                   # = sin
    sinTeff = np.concatenate([-sinT[: ROT // 2], sinT[ROT // 2:]], axis=0)
    sinTeff = np.ascontiguousarray(sinTeff).astype(F16NP)

    maskb = np.where(
        np.arange(128)[:, None] > np.arange(128)[None, :], MASK_NEG, 0.0
    ).astype(F16NP)
    ident = np.eye(128, dtype=F16NP)
    # rotate-half permutation: out[r] = t[(r+16) % 32]
    perm = np.zeros((ROT, ROT), F16NP)
    perm[(np.arange(ROT) + ROT // 2) % ROT, np.arange(ROT)] = 1.0

    in_maps = []
    for c in range(NCORES):
        heads = [HPC * c, HPC * c + 1]
        wq = np.concatenate([W_qkv[:, n * 384: n * 384 + 128] for n in heads], 1)
        wk = np.concatenate([W_qkv[:, n * 384 + 128: n * 384 + 256] for n in heads], 1)
        wv = np.concatenate([W_qkv[:, n * 384 + 256: n * 384 + 384] for n in heads], 1)
        wq8, wqe = _q8pair(wq, SWQK)
        wk8, wke = _q8pair(wk, SWQK)
        wv8, wve = _q8pair(wv, SWV)
        bq = np.stack([b_qkv[n * 384: n * 384 + 128] for n in heads], 1)
        bk = np.stack([b_qkv[n * 384 + 128: n * 384 + 256] for n in heads], 1)
        bv = np.concatenate([b_qkv[n * 384 + 256: n * 384 + 384] for n in heads])
        bqk = np.concatenate([bq, bk], axis=1).astype(np.float32)  # [128,4] q0 q1 k0 k1
        bvc = np.stack([bv[0:D], bv[D:2 * D]], 1).astype(np.float32)  # [128, 2]
        wdd = np.asarray(W_dense[c * HPC * D:(c + 1) * HPC * D, :], np.float32)
        in_maps.append({
            "ht8": ht8,
            "hte": hte,
            "wq8": _row_major_128(wq8, NKT),
            "wqe": _row_major_128(wqe, NKT),
            "wk8": _row_major_128(wk8, NKT),
            "wke": _row_major_128(wke, NKT),
            "wv8": _row_major_128(wv8, NKT),
            "wve": _row_major_128(wve, NKT),
            "wd": _row_major_128(wdd.astype(F16NP), HPC),
            "cosT": cosT,
            "sinTeff": sinTeff,
            "maskbias": maskb,
            "ident": ident,
            "perm": perm,
            "bqk": np.ascontiguousarray(bqk),
            "bvc": np.ascontiguousarray(bvc),
        })
    return in_maps


def _reduce(results, inputs):
    partial = np.zeros((S, HID), np.float64)
    for r in results:
        partial += r["partial"].astype(np.float64)
    out = (partial + np.asarray(inputs["b_dense"])[None, :]).astype(np.float32)
    return out.reshape(S, 1, HID)


def _run(inputs, trace=False):
    from concourse.bass_utils import run_bass_kernel_spmd

    if "nc" not in _cache:
        _cache["nc"] = _build_program()
    nc = _cache["nc"]
    in_maps = _prep_inputs(
        inputs["hidden_states"], inputs["W_qkv"], inputs["b_qkv"],
        inputs["W_dense"], inputs["b_dense"],
    )
    res = run_bass_kernel_spmd(nc, in_maps, list(range(NCORES)), trace=trace)
    return _reduce(res.results, inputs), res


def kernel(**inputs):
    out, _ = _run(inputs, trace=False)
    return out


# revision 81
# speedup vs baseline: 1.1173x; 1.0040x over previous
"""GPT-NeoX attention (s=2048, b=1, h=2048, nh=16, hd=128, rot=32) on 8 NeuronCores.

Sharding: tensor-parallel over heads (2 heads per core), row-parallel dense
with host-side partial reduction.

Per core: the QKV projections run as fp8e4 DoubleRow matmuls with 3-term
residual compensation (X@W ~ X8@W8 + Xe@W8 + X8@We, each plane pre-scaled
into e4m3's dynamic range) - 0.75x the bf16 PE cost at ~0.1% error. The
attention core (scores, softmax, context) and the dense matmul run in fp16.
Scores use a transposed layout; context is computed in natural layout with a
ones-column so the softmax denominator falls out of the matmul; a per-row
reciprocal scale normalizes on the vector engine; context transposes back on
the PE for the dense slice. Dense output of chunk i is deferred into chunk
i+1's window so the scalar engine's softmax-exp latency never stalls the PE.
"""

import math
import numpy as np
import ml_dtypes

S = 2048
HID = 2048
NH = 16
D = 128
ROT = 32
NCORES = 8
HPC = 2  # heads per core
CHUNK = 512
NKT = HID // 128  # 16 contraction tiles
NKP = NKT // 2    # 8 DoubleRow k-tile pairs
NCH = S // CHUNK  # 4 i-chunks
NST = S // 128    # 16 s-tiles
NORM = 1.0 / math.sqrt(D)
MASK_NEG = -1000.0 / NORM  # -1000 after the exp scale; exp() underflows to 0

# fp8 plane scales: place values in e4m3's sweet spot (max 240, min normal 2^-6)
SX = 32.0      # hidden-state planes
SWQK = 2048.0  # Wq/Wk planes -> q,k psum at 2^16, descaled in the PSUM->SBUF copy
SWV = 128.0    # Wv planes    -> v psum at 2^12 = 4096*v, cancelled by the ones column
QK_DESCALE = 1.0 / (SX * SWQK)
VSCALE = SX * SWV  # 4096; vn holds 4096*(v+b); ones col = 4096 so cn = ctx

E4NP = ml_dtypes.float8_e4m3
F16NP = np.float16

_cache = {}


def _build_program():
    from concourse import bass, bacc, tile
    from concourse.bass import mybir

    f32 = mybir.dt.float32
    fp16 = mybir.dt.float16
    fp8 = mybir.dt.float8e4
    Exp = mybir.ActivationFunctionType.Exp
    Ident = mybir.ActivationFunctionType.Identity
    ADD = mybir.AluOpType.add
    MULT = mybir.AluOpType.mult
    DR = mybir.MatmulPerfMode.DoubleRow

    nc = bacc.Bacc()

    # all dram tensors laid out [128, free] with >=512B contiguous runs
    ht8_d = nc.dram_tensor("ht8", [128, NKT * S], fp8, kind="ExternalInput")
    hte_d = nc.dram_tensor("hte", [128, NKT * S], fp8, kind="ExternalInput")
    wq8_d = nc.dram_tensor("wq8", [128, NKT * HPC * D], fp8, kind="ExternalInput")
    wqe_d = nc.dram_tensor("wqe", [128, NKT * HPC * D], fp8, kind="ExternalInput")
    wk8_d = nc.dram_tensor("wk8", [128, NKT * HPC * D], fp8, kind="ExternalInput")
    wke_d = nc.dram_tensor("wke", [128, NKT * HPC * D], fp8, kind="ExternalInput")
    wv8_d = nc.dram_tensor("wv8", [128, NKT * HPC * D], fp8, kind="ExternalInput")
    wve_d = nc.dram_tensor("wve", [128, NKT * HPC * D], fp8, kind="ExternalInput")
    wd_d = nc.dram_tensor("wd", [128, HPC * HID], fp16, kind="ExternalInput")
    cos_d = nc.dram_tensor("cosT", [ROT, S], fp16, kind="ExternalInput")
    sin_d = nc.dram_tensor("sinTeff", [ROT, S], fp16, kind="ExternalInput")
    mask_d = nc.dram_tensor("maskbias", [128, 128], fp16, kind="ExternalInput")
    perm_d = nc.dram_tensor("perm", [ROT, ROT], fp16, kind="ExternalInput")
    ident_d = nc.dram_tensor("ident", [128, 128], fp16, kind="ExternalInput")
    bqk_d = nc.dram_tensor("bqk", [128, 4], f32, kind="ExternalInput")
    bvc_d = nc.dram_tensor("bvc", [128, HPC], f32, kind="ExternalInput")
    out_d = nc.dram_tensor("partial", [S, HID], fp16, kind="ExternalOutput")

    with tile.TileContext(nc) as tc:
        with (
            tc.tile_pool(name="persist", bufs=1) as pp,
            tc.tile_pool(name="probs", bufs=40) as prp,
            tc.tile_pool(name="rotu", bufs=2) as rop,
            tc.tile_pool(name="ctxn", bufs=12) as cnp,
            tc.tile_pool(name="rec", bufs=16) as rcp,
            tc.tile_pool(name="stage", bufs=4) as stp,
            tc.tile_pool(name="psA", bufs=3, space="PSUM") as psA,
            tc.tile_pool(name="psB", bufs=2, space="PSUM") as psB,
            tc.tile_pool(name="psD", bufs=3, space="PSUM") as psD,
        ):
            pools = [psA, psB, psD]

            # ---- persistent SBUF tiles ----
            ht8 = pp.tile([128, NKT, S], fp8, tag="ht8")
            hte = pp.tile([128, NKT, S], fp8, tag="hte")
            wq8 = pp.tile([128, NKT, HPC * D], fp8, tag="wq8")
            wqe = pp.tile([128, NKT, HPC * D], fp8, tag="wqe")
            wk8 = pp.tile([128, NKT, HPC * D], fp8, tag="wk8")
            wke = pp.tile([128, NKT, HPC * D], fp8, tag="wke")
            wv8 = pp.tile([128, NKT, HPC * D], fp8, tag="wv8")
            wve = pp.tile([128, NKT, HPC * D], fp8, tag="wve")
            wd = pp.tile([128, HPC, HID], fp16, tag="wd")
            cosT = pp.tile([ROT, S], fp16, tag="cos")
            sinT = pp.tile([ROT, S], fp16, tag="sin")
            maskb = pp.tile([128, 128], fp16, tag="mask")
            perm = pp.tile([ROT, ROT], fp16, tag="perm")
            ident = pp.tile([128, 128], fp16, tag="ident")
            bqk = pp.tile([128, 4], f32, tag="bqk")
            bvc = pp.tile([128, HPC], f32, tag="bvc")
            qT = [pp.tile([128, S], fp16, tag=f"qT{h}", name=f"qT{h}") for h in range(HPC)]
            kT = [pp.tile([128, S], fp16, tag=f"kT{h}", name=f"kT{h}") for h in range(HPC)]
            # V natural layout (both heads) + ones column for the denominator
            vn = pp.tile([128, NST, HPC, D + 1], fp16, tag="vn")
            ctxT = [pp.tile([128, S], fp16, tag=f"ctxT{h}", name=f"ctxT{h}")
                    for h in range(HPC)]

            nc.vector.memset(vn[:, :, :, D:D + 1], VSCALE)

            # warm the activation function table while DMAs stream
            warm = pp.tile([128, 1], f32, tag="warm")
            nc.vector.memset(warm[:], 0.0)
            nc.scalar.activation(warm[:], warm[:], Exp)

            # ---- input DMAs: the first chains' (main/xres) terms need only
            # wk8/wq8 + ht planes; the we planes (wres pass) can arrive late ----
            wk8_r = wk8_d[:].rearrange("p (k m) -> p k m", k=NKT)
            wq8_r = wq8_d[:].rearrange("p (k m) -> p k m", k=NKT)
            ht8_r = ht8_d[:].rearrange("p (k s) -> p k s", k=NKT)
            hte_r = hte_d[:].rearrange("p (k s) -> p k s", k=NKT)
            # k-pair ht DMAs halve the HWDGE issue slots (the scarce
            # resource: ~630ns serialized issue per DMA); small side tensors
            # go through the Pool engine's SWDGE path, which skips HWDGE
            nc.sync.dma_start(wk8[:, 0:4, :], wk8_r[:, 0:4, :])
            nc.sync.dma_start(ht8[:, 0:2, 0:CHUNK], ht8_r[:, 0:2, 0:CHUNK])
            nc.sync.dma_start(wq8[:, 0:4, :], wq8_r[:, 0:4, :])
            nc.sync.dma_start(hte[:, 0:2, 0:CHUNK], hte_r[:, 0:2, 0:CHUNK])
            nc.sync.dma_start(ht8[:, 0:2, CHUNK:], ht8_r[:, 0:2, CHUNK:])
            nc.sync.dma_start(hte[:, 0:2, CHUNK:], hte_r[:, 0:2, CHUNK:])
            for k in range(2, NKT):
                nc.sync.dma_start(ht8[:, k, :], ht8_r[:, k, :])
                nc.sync.dma_start(hte[:, k, :], hte_r[:, k, :])
                if k == 2:
                    nc.sync.dma_start(wke[:], wke_d[:].rearrange("p (k m) -> p k m", k=NKT))
                    nc.sync.dma_start(wqe[:], wqe_d[:].rearrange("p (k m) -> p k m", k=NKT))
                if k == 4:
                    nc.sync.dma_start(wk8[:, 4:, :], wk8_r[:, 4:, :])
                    nc.sync.dma_start(wq8[:, 4:, :], wq8_r[:, 4:, :])
            nc.gpsimd.dma_start(cosT[:], cos_d[:])
            nc.gpsimd.dma_start(sinT[:], sin_d[:])
            nc.gpsimd.dma_start(maskb[:], mask_d[:])
            nc.gpsimd.dma_start(perm[:], perm_d[:])
            nc.gpsimd.dma_start(ident[:], ident_d[:])
            nc.gpsimd.dma_start(bqk[:], bqk_d[:])
            nc.gpsimd.dma_start(bvc[:], bvc_d[:])
            nc.sync.dma_start(wv8[:], wv8_d[:].rearrange("p (k m) -> p k m", k=NKT))
            nc.sync.dma_start(wve[:], wve_d[:].rearrange("p (k m) -> p k m", k=NKT))
            nc.sync.dma_start(wd[:], wd_d[:].rearrange("p (c o) -> p c o", c=HPC))

            def dr3_step(ps, j, w8, we, hcols, sl, first, last):
                # one k-pair step of a 3-term compensated chain
                kk = slice(2 * j, 2 * j + 2)
                nc.tensor.matmul(ps[:], w8[:, kk, hcols], ht8[:, kk, sl],
                                 start=first, stop=False, perf_mode=DR)
                nc.tensor.matmul(ps[:], w8[:, kk, hcols], hte[:, kk, sl],
                                 start=False, stop=False, perf_mode=DR)
                nc.tensor.matmul(ps[:], we[:, kk, hcols], ht8[:, kk, sl],
                                 start=False, stop=last, perf_mode=DR)

            def qk_proj(h, interleave):
                # qT/kT[h][d=128, s]; bias + 2^-16 descale in the PSUM->SBUF copy
                hcols = slice(h * D, (h + 1) * D)
                chains = []
                for ci in range(NCH):
                    sl = slice(ci * CHUNK, (ci + 1) * CHUNK)
                    for (w8, we, dst, bcol) in ((wk8, wke, kT, 2), (wq8, wqe, qT, 0)):
                        chains.append((w8, we, dst, bcol, sl))
                def chain_pool(c):
                    return (psA, "ps0") if c < 3 else (psB, "ps1") if c < 6 else (psD, "ps3")

                if interleave:
                    # j-major across all 8 chains so the PE rides the ht DMA
                    # staircase; the wres term lags 2 j-steps so the we
                    # planes (DMA'd mid-stream) never stall the pipeline
                    LAG = 1
                    pss = [chain_pool(c)[0].tile([128, CHUNK], f32,
                                                  tag=chain_pool(c)[1],
                                                  name=f"pss{c}")
                           for c in range(8)]
                    for j in range(NKP + LAG):
                        if j < NKP:
                            kk = slice(2 * j, 2 * j + 2)
                            for c, (w8, we, dst, bcol, sl) in enumerate(chains):
                                nc.tensor.matmul(pss[c][:], w8[:, kk, hcols],
                                                 ht8[:, kk, sl], start=(j == 0),
                                                 stop=False, perf_mode=DR)
                            for c, (w8, we, dst, bcol, sl) in enumerate(chains):
                                nc.tensor.matmul(pss[c][:], w8[:, kk, hcols],
                                                 hte[:, kk, sl], start=False,
                                                 stop=False, perf_mode=DR)
                        if j >= LAG:
                            jw = j - LAG
                            kk = slice(2 * jw, 2 * jw + 2)
                            for c, (w8, we, dst, bcol, sl) in enumerate(chains):
                                nc.tensor.matmul(pss[c][:], we[:, kk, hcols],
                                                 ht8[:, kk, sl], start=False,
                                                 stop=(jw == NKP - 1),
                                                 perf_mode=DR)
                    for c, (w8, we, dst, bcol, sl) in enumerate(chains):
                        nc.scalar.activation(dst[h][:, sl], pss[c][:], Ident,
                                             bias=bqk[:, bcol + h:bcol + h + 1],
                                             scale=QK_DESCALE)
                else:
                    for c, (w8, we, dst, bcol, sl) in enumerate(chains):
                        ps = chain_pool(c)[0].tile([128, CHUNK], f32,
                                                   tag=chain_pool(c)[1], name="ps")
                        for j in range(NKP):
                            dr3_step(ps, j, w8, we, hcols, sl, j == 0, j == NKP - 1)
                        nc.scalar.activation(dst[h][:, sl], ps[:], Ident,
                                             bias=bqk[:, bcol + h:bcol + h + 1],
                                             scale=QK_DESCALE)

            def v_proj(st_lo, st_hi):
                # vn[s-part, st, h, d] natural layout, both heads per chain
                for st in range(st_lo, st_hi):
                    ssl = slice(st * 128, (st + 1) * 128)
                    vp, vt = [(psA, "ps0"), (psB, "ps1"), (psD, "ps3")][st % 3]
                    ps = vp.tile([128, HPC * D], f32, tag=vt, name="ps")
                    for j in range(NKP):
                        kk = slice(2 * j, 2 * j + 2)
                        nc.tensor.matmul(ps[:], ht8[:, kk, ssl], wv8[:, kk, :],
                                         start=(j == 0), stop=False, perf_mode=DR)
                        nc.tensor.matmul(ps[:], hte[:, kk, ssl], wv8[:, kk, :],
                                         start=False, stop=False, perf_mode=DR)
                        nc.tensor.matmul(ps[:], ht8[:, kk, ssl], wve[:, kk, :],
                                         start=False, stop=(j == NKP - 1), perf_mode=DR)
                    # vn = 4096*v cast to fp16 (the v-bias is added
                    # per-partition in the post-transpose ctxT copy)
                    nc.scalar.activation(
                        vn[:, st, :, 0:D],
                        ps[:].rearrange("p (c d) -> p c d", c=HPC), Ident)

            def rope(tensors):
                # rows 0..31: t = t*cos + rotate_half(t)*sin; the half-swap
                # runs on the PE as a permutation matmul (no DMA latency).
                # ci-major across tensors so early chunks unblock scores fast
                for ci in range(NCH):
                    sl = slice(ci * CHUNK, (ci + 1) * CHUNK)
                    for ti, t in enumerate(tensors):
                        rps = pools[(2 * ci + ti) % 2].tile(
                            [ROT, CHUNK], f32, tag=f"ps{(2 * ci + ti) % 2}",
                            name="rps")
                        nc.tensor.matmul(rps[:], perm[:], t[0:ROT, sl],
                                         start=True, stop=True)
                        rotu = rop.tile([ROT, CHUNK], fp16, tag="rotu")
                        nc.vector.tensor_tensor(rotu[:], rps[:], sinT[:, sl], MULT)
                        nc.vector.tensor_tensor(t[0:ROT, sl], t[0:ROT, sl], cosT[:, sl], MULT)
                        nc.vector.tensor_tensor(t[0:ROT, sl], t[0:ROT, sl], rotu[:], ADD)

            def scores_t(ci, h, t, prs):
                # one transposed scores tile + exp -> fp16 probs tile
                pool = pools[t % 2]
                pss = pool.tile([128, CHUNK], f32, tag=f"ps{t % 2}", name="pss")
                off = (t - 4 * ci) * 128
                lo = max(off, 0)  # cols i < off never consumed
                nc.tensor.matmul(
                    pss[:, lo:], kT[h][:, t * 128:(t + 1) * 128],
                    qT[h][:, ci * CHUNK + lo:(ci + 1) * CHUNK],
                    start=True, stop=(off < 0))
                if off >= 0:
                    # diagonal tile: add causal mask via I @ maskb
                    nc.tensor.matmul(pss[:, off:off + 128], ident[:], maskb[:],
                                     start=False, stop=True)
                pr = prp.tile([128, CHUNK], fp16, tag="probs")
                nc.scalar.activation(pr[:, lo:], pss[:, lo:], Exp, scale=NORM)
                prs.append(pr)

            def ctx_io(ci, h, io, prs):
                # context + denominator; normalize; transpose back via PE.
                # pc rotates over psA/psB (4 chains in flight) so the DVE
                # reciprocal+scale latency never starves the PE.
                it = 4 * ci + io
                pc = pools[io % 2].tile([128, CHUNK], f32, tag=f"ps{io % 2}",
                                        name=f"pc{io}")
                for t in range(it + 1):
                    nc.tensor.matmul(
                        pc[:, 0:D + 1],
                        prs[t][:, io * 128:(io + 1) * 128],
                        vn[:, t, h, :],
                        start=(t == 0), stop=(t == it))
                rec = rcp.tile([128, 1], f32, tag="rec")
                nc.vector.reciprocal(rec[:], pc[:, D:D + 1])
                cn = cnp.tile([128, D], fp16, tag="ctxn")
                nc.vector.tensor_scalar_mul(cn[:], pc[:, 0:D], rec[:, 0:1])
                pt = psD.tile([128, D], fp16, tag="ps3", name="pt")
                nc.tensor.transpose(pt[:], cn[:], ident[:])
                nc.vector.tensor_scalar(
                    ctxT[h][:, it * 128:(it + 1) * 128], pt[:],
                    bvc[:, h:h + 1], None, op0=ADD)

            def ctx(ci, h, prs):
                for io in range(4):
                    ctx_io(ci, h, io, prs)

            def dense_pieces(ci, on_act=False):
                # 16 oc-chain closures for chunk ci's dense s-tiles; callers
                # interleave them between scores tiles to keep the PE fed
                # while the scalar engine drains the exp backlog
                items = []
                state = {}

                def mk(st, oc):
                    def run():
                        if oc == 0:
                            state[st] = stp.tile([128, HID], fp16, tag="stg",
                                                 name=f"stg{st}")
                        stg = state[st]
                        po = psD.tile([128, CHUNK], f32, tag="ps3", name="po")
                        for c in range(HPC):
                            nc.tensor.matmul(
                                po[:], ctxT[c][:, st * 128:(st + 1) * 128],
                                wd[:, c, oc * CHUNK:(oc + 1) * CHUNK],
                                start=(c == 0), stop=(c == HPC - 1))
                        osl = slice(oc * CHUNK, (oc + 1) * CHUNK)
                        if on_act and oc % 2 == 0:
                            nc.scalar.activation(stg[:, osl], po[:], Ident)
                        else:
                            nc.vector.tensor_copy(stg[:, osl], po[:])
                        if on_act and oc % 2 == 1:
                            hsl = slice((oc - 1) * CHUNK, (oc + 1) * CHUNK)
                            nc.sync.dma_start(
                                out_d[st * 128:(st + 1) * 128, hsl], stg[:, hsl])
                        elif not on_act and oc == NCH - 1:
                            nc.sync.dma_start(
                                out_d[st * 128:(st + 1) * 128, :], stg[:])
                    return run

                for st in range(4 * ci, 4 * ci + 4):
                    for oc in range(NCH):
                        items.append(mk(st, oc))
                return items

            def dense_st(st):
                for item in dense_pieces_for_st(st):
                    item()

            def dense_pieces_for_st(st):
                ci = st // 4
                all_items = dense_pieces(ci)
                return all_items[(st % 4) * NCH:(st % 4 + 1) * NCH]

            # ---- schedule: dense(ci) deferred into chunk ci+1's window so the
            # scalar engine's exp backlog never blocks the PE ----
            qk_proj(0, interleave=True)
            rope([kT[0], qT[0]])
            qk_proj(1, interleave=False)
            rope([kT[1], qT[1]])
            # chunk order 1,2,3,0: the big chunks' exp backlogs overlap
            # mid-kernel compute; tiny chunk 0 (8 exp tiles) lands last so
            # the tail is not exp-bound. dense(prev) fills each window.
            order = [0, 1, 2, 3]
            prs = {}
            for wi, ci in enumerate(order):
                ntile = 4 * ci + 4
                pr0, pr1 = [], []
                prs[ci] = (pr0, pr1)
                for t in range(ntile):
                    if wi == 0:
                        v_proj(2 * t, 2 * t + 2)
                    scores_t(ci, 0, t, pr0)
                if wi >= 1:
                    for item in dense_pieces(order[wi - 1]):
                        item()
                for t in range(ntile):
                    if wi == 0:
                        v_proj(8 + 2 * t, 10 + 2 * t)
                    scores_t(ci, 1, t, pr1)
                if wi < len(order) - 1:
                    ctx(ci, 0, pr0)
                    ctx(ci, 1, pr1)
                if wi == len(order) - 1:
                    # final window: stream each dense s-tile right after its
                    # ctx; copies on the now-empty scalar engine so DVE
                    # stays clear for the recip/scale chain
                    ctx(ci, 0, pr0)
                    dnl = dense_pieces(ci, on_act=True)
                    for io in range(4):
                        ctx_io(ci, 1, io, pr1)
                        for item in dnl[io * NCH:(io + 1) * NCH]:
                            item()

    nc.compile()
    return nc


def _q8pair(x, scale):
    """Scaled 2-plane e4m3 split: x*scale = hi + lo to ~0.1%."""
    xs = (np.asarray(x, np.float32) * scale).astype(np.float32)
    hi = xs.astype(E4NP)
    lo = (xs - hi.astype(np.float32)).astype(E4NP)
    return hi, lo


def _row_major_128(a, ngroups):
    """[(g p), m] -> [p, (g m)] so DMA runs are >=512B contiguous."""
    g, m = ngroups, a.shape[1]
    return np.ascontiguousarray(
        a.reshape(g, 128, m).transpose(1, 0, 2).reshape(128, g * m))


def _prep_inputs(hidden_states, W_qkv, b_qkv, W_dense, b_dense):
    hid = np.asarray(hidden_states).reshape(S, HID)
    hT = np.ascontiguousarray(hid.T).astype(np.float32)   # [HID, S]
    ht8, hte = _q8pair(hT, SX)
    ht8 = _row_major_128(ht8, NKT)
    hte = _row_major_128(hte, NKT)

    inv_freq = 1.0 / (10000.0 ** (np.arange(0, ROT, 2, dtype=np.float64) / ROT))
    t = np.arange(S, dtype=np.float64)
    freqs = np.outer(t, inv_freq)                      # [s, rot/2]
    emb = np.concatenate([freqs, freqs], axis=1)       # [s, rot]
    cosT = np.ascontiguousarray(np.cos(emb).T).astype(F16NP)
    sinT = np.cos(emb - np.pi / 2).T                   # = sin
    sinTeff = np.concatenate([-sinT[: ROT // 2], sinT[ROT // 2:]], axis=0)
    sinTeff = np.ascontiguousarray(sinTeff).astype(F16NP)

    maskb = np.where(
        np.arange(128)[:, None] > np.arange(128)[None, :], MASK_NEG, 0.0
    ).astype(F16NP)
    ident = np.eye(128, dtype=F16NP)
    # rotate-half permutation: out[r] = t[(r+16) % 32]
    perm = np.zeros((ROT, ROT), F16NP)
    perm[(np.arange(ROT) + ROT // 2) % ROT, np.arange(ROT)] = 1.0

    in_maps = []
    for c in range(NCORES):
        heads = [HPC * c, HPC * c + 1]
        wq = np.concatenate([W_qkv[:, n * 384: n * 384 + 128] for n in heads], 1)
        wk = np.concatenate([W_qkv[:, n * 384 + 128: n * 384 + 256] for n in heads], 1)
        wv = np.concatenate([W_qkv[:, n * 384 + 256: n * 384 + 384] for n in heads], 1)
        wq8, wqe = _q8pair(wq, SWQK)
        wk8, wke = _q8pair(wk, SWQK)
        wv8, wve = _q8pair(wv, SWV)
        bq = np.stack([b_qkv[n * 384: n * 384 + 128] for n in heads], 1)
        bk = np.stack([b_qkv[n * 384 + 128: n * 384 + 256] for n in heads], 1)
        bv = np.concatenate([b_qkv[n * 384 + 256: n * 384 + 384] for n in heads])
        bqk = np.concatenate([bq, bk], axis=1).astype(np.float32)  # [128,4] q0 q1 k0 k1
        bvc = np.stack([bv[0:D], bv[D:2 * D]], 1).astype(np.float32)  # [128, 2]
        wdd = np.asarray(W_dense[c * HPC * D:(c + 1) * HPC * D, :], np.float32)
        in_maps.append({
            "ht8": ht8,
            "hte": hte,
            "wq8": _row_major_128(wq8, NKT),
            "wqe": _row_major_128(wqe, NKT),
            "wk8": _row_major_128(wk8, NKT),
            "wke": _row_major_128(wke, NKT),
            "wv8": _row_major_128(wv8, NKT),
            "wve": _row_major_128(wve, NKT),
            "wd": _row_major_128(wdd.astype(F16NP), HPC),
            "cosT": cosT,
            "sinTeff": sinTeff,
            "maskbias": maskb,
            "ident": ident,
            "perm": perm,
            "bqk": np.ascontiguousarray(bqk),
            "bvc": np.ascontiguousarray(bvc),
        })
    return in_maps


def _reduce(results, inputs):
    partial = np.zeros((S, HID), np.float64)
    for r in results:
        partial += r["partial"].astype(np.float64)
    out = (partial + np.asarray(inputs["b_dense"])[None, :]).astype(np.float32)
    return out.reshape(S, 1, HID)


def _run(inputs, trace=False):
    from concourse.bass_utils import run_bass_kernel_spmd

    if "nc" not in _cache:
        _cache["nc"] = _build_program()
    nc = _cache["nc"]
    in_maps = _prep_inputs(
        inputs["hidden_states"], inputs["W_qkv"], inputs["b_qkv"],
        inputs["W_dense"], inputs["b_dense"],
    )
    res = run_bass_kernel_spmd(nc, in_maps, list(range(NCORES)), trace=trace)
    return _reduce(res.results, inputs), res


def kernel(**inputs):
    out, _ = _run(inputs, trace=False)
    return out


# revision 86
# speedup vs baseline: 1.1301x; 1.0115x over previous
"""GPT-NeoX attention (s=2048, b=1, h=2048, nh=16, hd=128, rot=32) on 8 NeuronCores.

Sharding: tensor-parallel over heads (2 heads per core), row-parallel dense
with host-side partial reduction.

Per core: the QKV projections run as fp8e4 DoubleRow matmuls with 3-term
residual compensation (X@W ~ X8@W8 + Xe@W8 + X8@We, each plane pre-scaled
into e4m3's dynamic range) - 0.75x the bf16 PE cost at ~0.1% error. The
attention core (scores, softmax, context) and the dense matmul run in fp16.
Scores use a transposed layout; context is computed in natural layout with a
ones-column so the softmax denominator falls out of the matmul; a per-row
reciprocal scale normalizes on the vector engine; context transposes back on
the PE for the dense slice. Dense output of chunk i is deferred into chunk
i+1's window so the scalar engine's softmax-exp latency never stalls the PE.
"""

import math
import numpy as np
import ml_dtypes

S = 2048
HID = 2048
NH = 16
D = 128
ROT = 32
NCORES = 8
HPC = 2  # heads per core
CHUNK = 512
NKT = HID // 128  # 16 contraction tiles
NKP = NKT // 2    # 8 DoubleRow k-tile pairs
NCH = S // CHUNK  # 4 i-chunks
NST = S // 128    # 16 s-tiles
NORM = 1.0 / math.sqrt(D)
MASK_NEG = -1000.0 / NORM  # -1000 after the exp scale; exp() underflows to 0

# fp8 plane scales: place values in e4m3's sweet spot (max 240, min normal 2^-6)
SX = 32.0      # hidden-state planes
SWQK = 2048.0  # Wq/Wk planes -> q,k psum at 2^16, descaled in the PSUM->SBUF copy
SWV = 128.0    # Wv planes    -> v psum at 2^12 = 4096*v, cancelled by the ones column
QK_DESCALE = 1.0 / (SX * SWQK)
VSCALE = SX * SWV  # 4096; vn holds 4096*(v+b); ones col = 4096 so cn = ctx

E4NP = ml_dtypes.float8_e4m3
F16NP = np.float16

_cache = {}


def _build_program():
    from concourse import bass, bacc, tile
    from concourse.bass import mybir

    f32 = mybir.dt.float32
    fp16 = mybir.dt.float16
    fp8 = mybir.dt.float8e4
    Exp = mybir.ActivationFunctionType.Exp
    Ident = mybir.ActivationFunctionType.Identity
    ADD = mybir.AluOpType.add
    MULT = mybir.AluOpType.mult
    DR = mybir.MatmulPerfMode.DoubleRow

    nc = bacc.Bacc()

    # all dram tensors laid out [128, free] with >=512B contiguous runs
    ht8_d = nc.dram_tensor("ht8", [128, NKT * S], fp8, kind="ExternalInput")
    hte_d = nc.dram_tensor("hte", [128, NKT * S], fp8, kind="ExternalInput")
    wq8_d = nc.dram_tensor("wq8", [128, NKT * HPC * D], fp8, kind="ExternalInput")
    wqe_d = nc.dram_tensor("wqe", [128, NKT * HPC * D], fp8, kind="ExternalInput")
    wk8_d = nc.dram_tensor("wk8", [128, NKT * HPC * D], fp8, kind="ExternalInput")
    wke_d = nc.dram_tensor("wke", [128, NKT * HPC * D], fp8, kind="ExternalInput")
    wv8_d = nc.dram_tensor("wv8", [128, NKT * HPC * D], fp8, kind="ExternalInput")
    wve_d = nc.dram_tensor("wve", [128, NKT * HPC * D], fp8, kind="ExternalInput")
    wd_d = nc.dram_tensor("wd", [128, HPC * HID], fp16, kind="ExternalInput")
    cos_d = nc.dram_tensor("cosT", [ROT, S], fp16, kind="ExternalInput")
    sin_d = nc.dram_tensor("sinTeff", [ROT, S], fp16, kind="ExternalInput")
    mask_d = nc.dram_tensor("maskbias", [128, 128], fp16, kind="ExternalInput")
    perm_d = nc.dram_tensor("perm", [ROT, ROT], fp16, kind="ExternalInput")
    ident_d = nc.dram_tensor("ident", [128, 128], fp16, kind="ExternalInput")
    bqk_d = nc.dram_tensor("bqk", [128, 4], f32, kind="ExternalInput")
    bvc_d = nc.dram_tensor("bvc", [128, HPC], f32, kind="ExternalInput")
    out_d = nc.dram_tensor("partial", [S, HID], fp16, kind="ExternalOutput")

    with tile.TileContext(nc) as tc:
        with (
            tc.tile_pool(name="persist", bufs=1) as pp,
            tc.tile_pool(name="probs", bufs=46) as prp,
            tc.tile_pool(name="rotu", bufs=2) as rop,
            tc.tile_pool(name="ctxn", bufs=12) as cnp,
            tc.tile_pool(name="rec", bufs=16) as rcp,
            tc.tile_pool(name="stage", bufs=4) as stp,
            tc.tile_pool(name="psA", bufs=3, space="PSUM") as psA,
            tc.tile_pool(name="psB", bufs=2, space="PSUM") as psB,
            tc.tile_pool(name="psD", bufs=3, space="PSUM") as psD,
        ):
            pools = [psA, psB, psD]

            # ---- persistent SBUF tiles ----
            ht8 = pp.tile([128, NKT, S], fp8, tag="ht8")
            hte = pp.tile([128, NKT, S], fp8, tag="hte")
            wq8 = pp.tile([128, NKT, HPC * D], fp8, tag="wq8")
            wqe = pp.tile([128, NKT, HPC * D], fp8, tag="wqe")
            wk8 = pp.tile([128, NKT, HPC * D], fp8, tag="wk8")
            wke = pp.tile([128, NKT, HPC * D], fp8, tag="wke")
            wv8 = pp.tile([128, NKT, HPC * D], fp8, tag="wv8")
            wve = pp.tile([128, NKT, HPC * D], fp8, tag="wve")
            wd = pp.tile([128, HPC, HID], fp16, tag="wd")
            cosT = pp.tile([ROT, S], fp16, tag="cos")
            sinT = pp.tile([ROT, S], fp16, tag="sin")
            maskb = pp.tile([128, 128], fp16, tag="mask")
            perm = pp.tile([ROT, ROT], fp16, tag="perm")
            ident = pp.tile([128, 128], fp16, tag="ident")
            bqk = pp.tile([128, 4], f32, tag="bqk")
            bvc = pp.tile([128, HPC], f32, tag="bvc")
            qT = [pp.tile([128, S], fp16, tag=f"qT{h}", name=f"qT{h}") for h in range(HPC)]
            kT = [pp.tile([128, S], fp16, tag=f"kT{h}", name=f"kT{h}") for h in range(HPC)]
            # V natural layout (both heads) + ones column for the denominator
            vn = pp.tile([128, NST, HPC, D + 1], fp16, tag="vn")
            ctxT = [pp.tile([128, S], fp16, tag=f"ctxT{h}", name=f"ctxT{h}")
                    for h in range(HPC)]

            nc.vector.memset(vn[:, :, :, D:D + 1], VSCALE)

            # warm the activation function table while DMAs stream
            warm = pp.tile([128, 1], f32, tag="warm")
            nc.vector.memset(warm[:], 0.0)
            nc.scalar.activation(warm[:], warm[:], Exp)

            # ---- input DMAs: the first chains' (main/xres) terms need only
            # wk8/wq8 + ht planes; the we planes (wres pass) can arrive late ----
            wk8_r = wk8_d[:].rearrange("p (k m) -> p k m", k=NKT)
            wq8_r = wq8_d[:].rearrange("p (k m) -> p k m", k=NKT)
            ht8_r = ht8_d[:].rearrange("p (k s) -> p k s", k=NKT)
            hte_r = hte_d[:].rearrange("p (k s) -> p k s", k=NKT)
            # k-pair ht DMAs halve the HWDGE issue slots (the scarce
            # resource: ~630ns serialized issue per DMA); small side tensors
            # go through the Pool engine's SWDGE path, which skips HWDGE
            nc.sync.dma_start(wk8[:, 0:4, :], wk8_r[:, 0:4, :])
            nc.sync.dma_start(ht8[:, 0:2, 0:CHUNK], ht8_r[:, 0:2, 0:CHUNK])
            nc.sync.dma_start(wq8[:, 0:4, :], wq8_r[:, 0:4, :])
            nc.sync.dma_start(hte[:, 0:2, 0:CHUNK], hte_r[:, 0:2, 0:CHUNK])
            nc.sync.dma_start(ht8[:, 0:2, CHUNK:], ht8_r[:, 0:2, CHUNK:])
            nc.sync.dma_start(hte[:, 0:2, CHUNK:], hte_r[:, 0:2, CHUNK:])
            for k in range(2, NKT):
                nc.sync.dma_start(ht8[:, k, :], ht8_r[:, k, :])
                nc.sync.dma_start(hte[:, k, :], hte_r[:, k, :])
                if k == 2:
                    nc.sync.dma_start(wke[:], wke_d[:].rearrange("p (k m) -> p k m", k=NKT))
                    nc.sync.dma_start(wqe[:], wqe_d[:].rearrange("p (k m) -> p k m", k=NKT))
                if k == 4:
                    nc.sync.dma_start(wk8[:, 4:, :], wk8_r[:, 4:, :])
                    nc.sync.dma_start(wq8[:, 4:, :], wq8_r[:, 4:, :])
            nc.gpsimd.dma_start(cosT[:], cos_d[:])
            nc.gpsimd.dma_start(sinT[:], sin_d[:])
            nc.gpsimd.dma_start(maskb[:], mask_d[:])
            nc.gpsimd.dma_start(perm[:], perm_d[:])
            nc.gpsimd.dma_start(ident[:], ident_d[:])
            nc.gpsimd.dma_start(bqk[:], bqk_d[:])
            nc.gpsimd.dma_start(bvc[:], bvc_d[:])
            nc.sync.dma_start(wv8[:], wv8_d[:].rearrange("p (k m) -> p k m", k=NKT))
            nc.sync.dma_start(wve[:], wve_d[:].rearrange("p (k m) -> p k m", k=NKT))
            nc.sync.dma_start(wd[:], wd_d[:].rearrange("p (c o) -> p c o", c=HPC))

            def dr3_step(ps, j, w8, we, hcols, sl, first, last):
                # one k-pair step of a 3-term compensated chain
                kk = slice(2 * j, 2 * j + 2)
                nc.tensor.matmul(ps[:], w8[:, kk, hcols], ht8[:, kk, sl],
                                 start=first, stop=False, perf_mode=DR)
                nc.tensor.matmul(ps[:], w8[:, kk, hcols], hte[:, kk, sl],
                                 start=False, stop=False, perf_mode=DR)
                nc.tensor.matmul(ps[:], we[:, kk, hcols], ht8[:, kk, sl],
                                 start=False, stop=last, perf_mode=DR)

            def qk_proj(h, interleave):
                # qT/kT[h][d=128, s]; bias + 2^-16 descale in the PSUM->SBUF copy
                hcols = slice(h * D, (h + 1) * D)
                chains = []
                for ci in range(NCH):
                    sl = slice(ci * CHUNK, (ci + 1) * CHUNK)
                    for (w8, we, dst, bcol) in ((wk8, wke, kT, 2), (wq8, wqe, qT, 0)):
                        chains.append((w8, we, dst, bcol, sl))
                def chain_pool(c):
                    return (psA, "ps0") if c < 3 else (psB, "ps1") if c < 6 else (psD, "ps3")

                if interleave:
                    # j-major across all 8 chains so the PE rides the ht DMA
                    # staircase; the wres term lags 2 j-steps so the we
                    # planes (DMA'd mid-stream) never stall the pipeline
                    LAG = 1
                    pss = [chain_pool(c)[0].tile([128, CHUNK], f32,
                                                  tag=chain_pool(c)[1],
                                                  name=f"pss{c}")
                           for c in range(8)]
                    for j in range(NKP + LAG):
                        if j < NKP:
                            kk = slice(2 * j, 2 * j + 2)
                            for c, (w8, we, dst, bcol, sl) in enumerate(chains):
                                nc.tensor.matmul(pss[c][:], w8[:, kk, hcols],
                                                 ht8[:, kk, sl], start=(j == 0),
                                                 stop=False, perf_mode=DR)
                            for c, (w8, we, dst, bcol, sl) in enumerate(chains):
                                nc.tensor.matmul(pss[c][:], w8[:, kk, hcols],
                                                 hte[:, kk, sl], start=False,
                                                 stop=False, perf_mode=DR)
                        if j >= LAG:
                            jw = j - LAG
                            kk = slice(2 * jw, 2 * jw + 2)
                            for c, (w8, we, dst, bcol, sl) in enumerate(chains):
                                nc.tensor.matmul(pss[c][:], we[:, kk, hcols],
                                                 ht8[:, kk, sl], start=False,
                                                 stop=(jw == NKP - 1),
                                                 perf_mode=DR)
                    for c, (w8, we, dst, bcol, sl) in enumerate(chains):
                        nc.scalar.activation(dst[h][:, sl], pss[c][:], Ident,
                                             bias=bqk[:, bcol + h:bcol + h + 1],
                                             scale=QK_DESCALE)
                else:
                    for c, (w8, we, dst, bcol, sl) in enumerate(chains):
                        ps = chain_pool(c)[0].tile([128, CHUNK], f32,
                                                   tag=chain_pool(c)[1], name="ps")
                        for j in range(NKP):
                            dr3_step(ps, j, w8, we, hcols, sl, j == 0, j == NKP - 1)
                        nc.scalar.activation(dst[h][:, sl], ps[:], Ident,
                                             bias=bqk[:, bcol + h:bcol + h + 1],
                                             scale=QK_DESCALE)

            def v_proj(st_lo, st_hi):
                # vn[s-part, st, h, d] natural layout, both heads per chain
                for st in range(st_lo, st_hi):
                    ssl = slice(st * 128, (st + 1) * 128)
                    vp, vt = [(psA, "ps0"), (psB, "ps1"), (psD, "ps3")][st % 3]
                    ps = vp.tile([128, HPC * D], f32, tag=vt, name="ps")
                    for j in range(NKP):
                        kk = slice(2 * j, 2 * j + 2)
                        nc.tensor.matmul(ps[:], ht8[:, kk, ssl], wv8[:, kk, :],
                                         start=(j == 0), stop=False, perf_mode=DR)
                        nc.tensor.matmul(ps[:], hte[:, kk, ssl], wv8[:, kk, :],
                                         start=False, stop=False, perf_mode=DR)
                        nc.tensor.matmul(ps[:], ht8[:, kk, ssl], wve[:, kk, :],
                                         start=False, stop=(j == NKP - 1), perf_mode=DR)
                    # vn = 4096*v cast to fp16 (the v-bias is added
                    # per-partition in the post-transpose ctxT copy)
                    nc.scalar.activation(
                        vn[:, st, :, 0:D],
                        ps[:].rearrange("p (c d) -> p c d", c=HPC), Ident)

            def rope(tensors):
                # rows 0..31: t = t*cos + rotate_half(t)*sin; the half-swap
                # runs on the PE as a permutation matmul (no DMA latency).
                # ci-major across tensors so early chunks unblock scores fast
                for ci in range(NCH):
                    sl = slice(ci * CHUNK, (ci + 1) * CHUNK)
                    for ti, t in enumerate(tensors):
                        rps = pools[(2 * ci + ti) % 2].tile(
                            [ROT, CHUNK], f32, tag=f"ps{(2 * ci + ti) % 2}",
                            name="rps")
                        nc.tensor.matmul(rps[:], perm[:], t[0:ROT, sl],
                                         start=True, stop=True)
                        rotu = rop.tile([ROT, CHUNK], fp16, tag="rotu")
                        nc.vector.tensor_tensor(rotu[:], rps[:], sinT[:, sl], MULT)
                        nc.vector.tensor_tensor(t[0:ROT, sl], t[0:ROT, sl], cosT[:, sl], MULT)
                        nc.vector.tensor_tensor(t[0:ROT, sl], t[0:ROT, sl], rotu[:], ADD)

            def scores_t(ci, h, t, prs):
                # one transposed scores tile + exp -> fp16 probs tile
                pool = pools[t % 2]
                pss = pool.tile([128, CHUNK], f32, tag=f"ps{t % 2}", name="pss")
                off = (t - 4 * ci) * 128
                lo = max(off, 0)  # cols i < off never consumed
                nc.tensor.matmul(
                    pss[:, lo:], kT[h][:, t * 128:(t + 1) * 128],
                    qT[h][:, ci * CHUNK + lo:(ci + 1) * CHUNK],
                    start=True, stop=(off < 0))
                if off >= 0:
                    # diagonal tile: add causal mask via I @ maskb
                    nc.tensor.matmul(pss[:, off:off + 128], ident[:], maskb[:],
                                     start=False, stop=True)
                pr = prp.tile([128, CHUNK], fp16, tag="probs")
                nc.scalar.activation(pr[:, lo:], pss[:, lo:], Exp, scale=NORM)
                prs.append(pr)

            def ctx_io(ci, h, io, prs):
                # context + denominator; normalize; transpose back via PE.
                # pc rotates over psA/psB (4 chains in flight) so the DVE
                # reciprocal+scale latency never starves the PE.
                it = 4 * ci + io
                pc = pools[io % 2].tile([128, CHUNK], f32, tag=f"ps{io % 2}",
                                        name=f"pc{io}")
                for t in range(it + 1):
                    nc.tensor.matmul(
                        pc[:, 0:D + 1],
                        prs[t][:, io * 128:(io + 1) * 128],
                        vn[:, t, h, :],
                        start=(t == 0), stop=(t == it))
                rec = rcp.tile([128, 1], f32, tag="rec")
                nc.vector.reciprocal(rec[:], pc[:, D:D + 1])
                cn = cnp.tile([128, D], fp16, tag="ctxn")
                nc.vector.tensor_scalar_mul(cn[:], pc[:, 0:D], rec[:, 0:1])
                pt = psD.tile([128, D], fp16, tag="ps3", name="pt")
                nc.tensor.transpose(pt[:], cn[:], ident[:])
                nc.vector.tensor_scalar(
                    ctxT[h][:, it * 128:(it + 1) * 128], pt[:],
                    bvc[:, h:h + 1], None, op0=ADD)

            def ctx(ci, h, prs):
                for io in range(4):
                    ctx_io(ci, h, io, prs)

            def dense_pieces(ci, on_act=False):
                # 16 oc-chain closures for chunk ci's dense s-tiles; callers
                # interleave them between scores tiles to keep the PE fed
                # while the scalar engine drains the exp backlog
                items = []
                state = {}

                def mk(st, oc):
                    def run():
                        if oc == 0:
                            state[st] = stp.tile([128, HID], fp16, tag="stg",
                                                 name=f"stg{st}")
                        stg = state[st]
                        po = psD.tile([128, CHUNK], f32, tag="ps3", name="po")
                        for c in range(HPC):
                            nc.tensor.matmul(
                                po[:], ctxT[c][:, st * 128:(st + 1) * 128],
                                wd[:, c, oc * CHUNK:(oc + 1) * CHUNK],
                                start=(c == 0), stop=(c == HPC - 1))
                        osl = slice(oc * CHUNK, (oc + 1) * CHUNK)
                        if on_act and oc % 2 == 0:
                            nc.scalar.activation(stg[:, osl], po[:], Ident)
                        else:
                            nc.vector.tensor_copy(stg[:, osl], po[:])
                        if on_act and oc % 2 == 1:
                            hsl = slice((oc - 1) * CHUNK, (oc + 1) * CHUNK)
                            nc.sync.dma_start(
                                out_d[st * 128:(st + 1) * 128, hsl], stg[:, hsl])
                        elif not on_act and oc == NCH - 1:
                            nc.sync.dma_start(
                                out_d[st * 128:(st + 1) * 128, :], stg[:])
                    return run

                for st in range(4 * ci, 4 * ci + 4):
                    for oc in range(NCH):
                        items.append(mk(st, oc))
                return items

            def dense_st(st):
                for item in dense_pieces_for_st(st):
                    item()

            def dense_pieces_for_st(st):
                ci = st // 4
                all_items = dense_pieces(ci)
                return all_items[(st % 4) * NCH:(st % 4 + 1) * NCH]

            # ---- schedule: dense(ci) deferred into chunk ci+1's window so the
            # scalar engine's exp backlog never blocks the PE ----
            qk_proj(0, interleave=True)
            rope([kT[0], qT[0]])
            qk_proj(1, interleave=False)
            rope([kT[1], qT[1]])
            # deferred-dense schedule with cross-window scores hoisting:
            # window ci = [dense(prev)][scores h1][ctx h0][ctx h1 interleaved
            # with the NEXT chunk's scores h0]. ctx h1 is exp-gated on big
            # chunks, so next-chunk scores tiles are free PE filler there.
            order = [0, 1, 2, 3]
            prs = {c: ([], []) for c in order}
            for t in range(4):
                v_proj(2 * t, 2 * t + 2)
                scores_t(0, 0, t, prs[0][0])
            for wi, ci in enumerate(order):
                ntile = 4 * ci + 4
                pr0, pr1 = prs[ci]
                if wi >= 1:
                    for item in dense_pieces(order[wi - 1]):
                        item()
                for t in range(ntile):
                    if wi == 0:
                        v_proj(8 + 2 * t, 10 + 2 * t)
                    scores_t(ci, 1, t, pr1)
                if wi < len(order) - 1:
                    nxt = order[wi + 1]
                    nt2 = 4 * nxt + 4
                    si = 0
                    for io in range(4):
                        ctx_io(ci, 0, io, pr0)
                    for io in range(4):
                        ctx_io(ci, 1, io, pr1)
                        tgt = ((io + 1) * (io + 2)) * nt2 // 20
                        while si < tgt:
                            scores_t(nxt, 0, si, prs[nxt][0])
                            si += 1
                else:
                    ctx(ci, 0, pr0)
                    # final window: stream each dense s-tile right after its
                    # ctx; copies on the now-empty scalar engine so DVE
                    # stays clear for the recip/scale chain
                    dnl = dense_pieces(ci, on_act=True)
                    for io in range(4):
                        ctx_io(ci, 1, io, pr1)
                        for item in dnl[io * NCH:(io + 1) * NCH]:
                            item()

    nc.compile()
    return nc


def _q8pair(x, scale):
    """Scaled 2-plane e4m3 split: x*scale = hi + lo to ~0.1%."""
    xs = (np.asarray(x, np.float32) * scale).astype(np.float32)
    hi = xs.astype(E4NP)
    lo = (xs - hi.astype(np.float32)).astype(E4NP)
    return hi, lo


def _row_major_128(a, ngroups):
    """[(g p), m] -> [p, (g m)] so DMA runs are >=512B contiguous."""
    g, m = ngroups, a.shape[1]
    return np.ascontiguousarray(
        a.reshape(g, 128, m).transpose(1, 0, 2).reshape(128, g * m))


def _prep_inputs(hidden_states, W_qkv, b_qkv, W_dense, b_dense):
    hid = np.asarray(hidden_states).reshape(S, HID)
    hT = np.ascontiguousarray(hid.T).astype(np.float32)   # [HID, S]
    ht8, hte = _q8pair(hT, SX)
    ht8 = _row_major_128(ht8, NKT)
    hte = _row_major_128(hte, NKT)

    inv_freq = 1.0 / (10000.0 ** (np.arange(0, ROT, 2, dtype=np.float64) / ROT))
    t = np.arange(S, dtype=np.float64)
    freqs = np.outer(t, inv_freq)                      # [s, rot/2]
    emb = np.concatenate([freqs, freqs], axis=1)       # [s, rot]
    cosT = np.ascontiguousarray(np.cos(emb).T).astype(F16NP)
    sinT = np.cos(emb - np.pi / 2).T                   # = sin
    sinTeff = np.concatenate([-sinT[: ROT // 2], sinT[ROT // 2:]], axis=0)
    sinTeff = np.ascontiguousarray(sinTeff).astype(F16NP)

    maskb = np.where(
        np.arange(128)[:, None] > np.arange(128)[None, :], MASK_NEG, 0.0
    ).astype(F16NP)
    ident = np.eye(128, dtype=F16NP)
    # rotate-half permutation: out[r] = t[(r+16) % 32]
    perm = np.zeros((ROT, ROT), F16NP)
    perm[(np.arange(ROT) + ROT // 2) % ROT, np.arange(ROT)] = 1.0

    in_maps = []
    for c in range(NCORES):
        heads = [HPC * c, HPC * c + 1]
        wq = np.concatenate([W_qkv[:, n * 384: n * 384 + 128] for n in heads], 1)
        wk = np.concatenate([W_qkv[:, n * 384 + 128: n * 384 + 256] for n in heads], 1)
        wv = np.concatenate([W_qkv[:, n * 384 + 256: n * 384 + 384] for n in heads], 1)
        wq8, wqe = _q8pair(wq, SWQK)
        wk8, wke = _q8pair(wk, SWQK)
        wv8, wve = _q8pair(wv, SWV)
        bq = np.stack([b_qkv[n * 384: n * 384 + 128] for n in heads], 1)
        bk = np.stack([b_qkv[n * 384 + 128: n * 384 + 256] for n in heads], 1)
        bv = np.concatenate([b_qkv[n * 384 + 256: n * 384 + 384] for n in heads])
        bqk = np.concatenate([bq, bk], axis=1).astype(np.float32)  # [128,4] q0 q1 k0 k1
        bvc = np.stack([bv[0:D], bv[D:2 * D]], 1).astype(np.float32)  # [128, 2]
        wdd = np.asarray(W_dense[c * HPC * D:(c + 1) * HPC * D, :], np.float32)
        in_maps.append({
            "ht8": ht8,
            "hte": hte,
            "wq8": _row_major_128(wq8, NKT),
            "wqe": _row_major_128(wqe, NKT),
            "wk8": _row_major_128(wk8, NKT),
            "wke": _row_major_128(wke, NKT),
            "wv8": _row_major_128(wv8, NKT),
            "wve": _row_major_128(wve, NKT),
            "wd": _row_major_128(wdd.astype(F16NP), HPC),
            "cosT": cosT,
            "sinTeff": sinTeff,
            "maskbias": maskb,
            "ident": ident,
            "perm": perm,
            "bqk": np.ascontiguousarray(bqk),
            "bvc": np.ascontiguousarray(bvc),
        })
    return in_maps


def _reduce(results, inputs):
    partial = np.zeros((S, HID), np.float64)
    for r in results:
        partial += r["partial"].astype(np.float64)
    out = (partial + np.asarray(inputs["b_dense"])[None, :]).astype(np.float32)
    return out.reshape(S, 1, HID)


def _run(inputs, trace=False):
    from concourse.bass_utils import run_bass_kernel_spmd

    if "nc" not in _cache:
        _cache["nc"] = _build_program()
    nc = _cache["nc"]
    in_maps = _prep_inputs(
        inputs["hidden_states"], inputs["W_qkv"], inputs["b_qkv"],
        inputs["W_dense"], inputs["b_dense"],
    )
    res = run_bass_kernel_spmd(nc, in_maps, list(range(NCORES)), trace=trace)
    return _reduce(res.results, inputs), res


def kernel(**inputs):
    out, _ = _run(inputs, trace=False)
    return out


# revision 91
# speedup vs baseline: 1.1306x; 1.0004x over previous
"""GPT-NeoX attention (s=2048, b=1, h=2048, nh=16, hd=128, rot=32) on 8 NeuronCores.

Sharding: tensor-parallel over heads (2 heads per core), row-parallel dense
with host-side partial reduction.

Per core: the QKV projections run as fp8e4 DoubleRow matmuls with 3-term
residual compensation (X@W ~ X8@W8 + Xe@W8 + X8@We, each plane pre-scaled
into e4m3's dynamic range) - 0.75x the bf16 PE cost at ~0.1% error. The
attention core (scores, softmax, context) and the dense matmul run in fp16.
Scores use a transposed layout; context is computed in natural layout with a
ones-column so the softmax denominator falls out of the matmul; a per-row
reciprocal scale normalizes on the vector engine; context transposes back on
the PE for the dense slice. Dense output of chunk i is deferred into chunk
i+1's window so the scalar engine's softmax-exp latency never stalls the PE.
"""

import math
import numpy as np
import ml_dtypes

S = 2048
HID = 2048
NH = 16
D = 128
ROT = 32
NCORES = 8
HPC = 2  # heads per core
CHUNK = 512
NKT = HID // 128  # 16 contraction tiles
NKP = NKT // 2    # 8 DoubleRow k-tile pairs
NCH = S // CHUNK  # 4 i-chunks
NST = S // 128    # 16 s-tiles
NORM = 1.0 / math.sqrt(D)
MASK_NEG = -1000.0 / NORM  # -1000 after the exp scale; exp() underflows to 0

# fp8 plane scales: place values in e4m3's sweet spot (max 240, min normal 2^-6)
SX = 32.0      # hidden-state planes
SWQK = 2048.0  # Wq/Wk planes -> q,k psum at 2^16, descaled in the PSUM->SBUF copy
SWV = 128.0    # Wv planes    -> v psum at 2^12 = 4096*v, cancelled by the ones column
QK_DESCALE = 1.0 / (SX * SWQK)
VSCALE = SX * SWV  # 4096; vn holds 4096*(v+b); ones col = 4096 so cn = ctx

E4NP = ml_dtypes.float8_e4m3
F16NP = np.float16

_cache = {}


def _build_program():
    from concourse import bass, bacc, tile
    from concourse.bass import mybir

    f32 = mybir.dt.float32
    fp16 = mybir.dt.float16
    fp8 = mybir.dt.float8e4
    Exp = mybir.ActivationFunctionType.Exp
    Ident = mybir.ActivationFunctionType.Identity
    ADD = mybir.AluOpType.add
    MULT = mybir.AluOpType.mult
    DR = mybir.MatmulPerfMode.DoubleRow

    nc = bacc.Bacc()

    # all dram tensors laid out [128, free] with >=512B contiguous runs
    ht8_d = nc.dram_tensor("ht8", [128, NKT * S], fp8, kind="ExternalInput")
    hte_d = nc.dram_tensor("hte", [128, NKT * S], fp8, kind="ExternalInput")
    wq8_d = nc.dram_tensor("wq8", [128, NKT * HPC * D], fp8, kind="ExternalInput")
    wqe_d = nc.dram_tensor("wqe", [128, NKT * HPC * D], fp8, kind="ExternalInput")
    wk8_d = nc.dram_tensor("wk8", [128, NKT * HPC * D], fp8, kind="ExternalInput")
    wke_d = nc.dram_tensor("wke", [128, NKT * HPC * D], fp8, kind="ExternalInput")
    wv8_d = nc.dram_tensor("wv8", [128, NKT * HPC * D], fp8, kind="ExternalInput")
    wve_d = nc.dram_tensor("wve", [128, NKT * HPC * D], fp8, kind="ExternalInput")
    wd_d = nc.dram_tensor("wd", [128, HPC * HID], fp16, kind="ExternalInput")
    cos_d = nc.dram_tensor("cosT", [ROT, S], fp16, kind="ExternalInput")
    sin_d = nc.dram_tensor("sinTeff", [ROT, S], fp16, kind="ExternalInput")
    mask_d = nc.dram_tensor("maskbias", [128, 128], fp16, kind="ExternalInput")
    perm_d = nc.dram_tensor("perm", [ROT, ROT], fp16, kind="ExternalInput")
    ident_d = nc.dram_tensor("ident", [128, 128], fp16, kind="ExternalInput")
    bqk_d = nc.dram_tensor("bqk", [128, 4], f32, kind="ExternalInput")
    bvc_d = nc.dram_tensor("bvc", [128, HPC], f32, kind="ExternalInput")
    out_d = nc.dram_tensor("partial", [S, HID], fp16, kind="ExternalOutput")

    with tile.TileContext(nc) as tc:
        with (
            tc.tile_pool(name="persist", bufs=1) as pp,
            tc.tile_pool(name="probs", bufs=46) as prp,
            tc.tile_pool(name="rotu", bufs=2) as rop,
            tc.tile_pool(name="ctxn", bufs=12) as cnp,
            tc.tile_pool(name="rec", bufs=16) as rcp,
            tc.tile_pool(name="stage", bufs=4) as stp,
            tc.tile_pool(name="psA", bufs=3, space="PSUM") as psA,
            tc.tile_pool(name="psB", bufs=2, space="PSUM") as psB,
            tc.tile_pool(name="psD", bufs=3, space="PSUM") as psD,
        ):
            pools = [psA, psB, psD]

            # ---- persistent SBUF tiles ----
            ht8 = pp.tile([128, NKT, S], fp8, tag="ht8")
            hte = pp.tile([128, NKT, S], fp8, tag="hte")
            wq8 = pp.tile([128, NKT, HPC * D], fp8, tag="wq8")
            wqe = pp.tile([128, NKT, HPC * D], fp8, tag="wqe")
            wk8 = pp.tile([128, NKT, HPC * D], fp8, tag="wk8")
            wke = pp.tile([128, NKT, HPC * D], fp8, tag="wke")
            wv8 = pp.tile([128, NKT, HPC * D], fp8, tag="wv8")
            wve = pp.tile([128, NKT, HPC * D], fp8, tag="wve")
            wd = pp.tile([128, HPC, HID], fp16, tag="wd")
            cosT = pp.tile([ROT, S], fp16, tag="cos")
            sinT = pp.tile([ROT, S], fp16, tag="sin")
            maskb = pp.tile([128, 128], fp16, tag="mask")
            perm = pp.tile([ROT, ROT], fp16, tag="perm")
            ident = pp.tile([128, 128], fp16, tag="ident")
            bqk = pp.tile([128, 4], f32, tag="bqk")
            bvc = pp.tile([128, HPC], f32, tag="bvc")
            qT = [pp.tile([128, S], fp16, tag=f"qT{h}", name=f"qT{h}") for h in range(HPC)]
            kT = [pp.tile([128, S], fp16, tag=f"kT{h}", name=f"kT{h}") for h in range(HPC)]
            # V natural layout (both heads) + ones column for the denominator
            vn = pp.tile([128, NST, HPC, D + 1], fp16, tag="vn")
            ctxT = [pp.tile([128, S], fp16, tag=f"ctxT{h}", name=f"ctxT{h}")
                    for h in range(HPC)]

            nc.vector.memset(vn[:, :, :, D:D + 1], VSCALE)

            # warm the activation function table while DMAs stream
            warm = pp.tile([128, 1], f32, tag="warm")
            nc.vector.memset(warm[:], 0.0)
            nc.scalar.activation(warm[:], warm[:], Exp)

            # ---- input DMAs: the first chains' (main/xres) terms need only
            # wk8/wq8 + ht planes; the we planes (wres pass) can arrive late ----
            wk8_r = wk8_d[:].rearrange("p (k m) -> p k m", k=NKT)
            wq8_r = wq8_d[:].rearrange("p (k m) -> p k m", k=NKT)
            ht8_r = ht8_d[:].rearrange("p (k s) -> p k s", k=NKT)
            hte_r = hte_d[:].rearrange("p (k s) -> p k s", k=NKT)
            # k-pair ht DMAs halve the HWDGE issue slots (the scarce
            # resource: ~630ns serialized issue per DMA); small side tensors
            # go through the Pool engine's SWDGE path, which skips HWDGE
            nc.sync.dma_start(wk8[:, 0:4, :], wk8_r[:, 0:4, :])
            nc.sync.dma_start(ht8[:, 0:2, 0:CHUNK], ht8_r[:, 0:2, 0:CHUNK])
            nc.sync.dma_start(wq8[:, 0:4, :], wq8_r[:, 0:4, :])
            nc.sync.dma_start(hte[:, 0:2, 0:CHUNK], hte_r[:, 0:2, 0:CHUNK])
            nc.sync.dma_start(ht8[:, 0:2, CHUNK:], ht8_r[:, 0:2, CHUNK:])
            nc.sync.dma_start(hte[:, 0:2, CHUNK:], hte_r[:, 0:2, CHUNK:])
            for k in range(2, NKT):
                nc.sync.dma_start(ht8[:, k, :], ht8_r[:, k, :])
                nc.sync.dma_start(hte[:, k, :], hte_r[:, k, :])
                if k == 2:
                    nc.sync.dma_start(wke[:], wke_d[:].rearrange("p (k m) -> p k m", k=NKT))
                    nc.sync.dma_start(wqe[:], wqe_d[:].rearrange("p (k m) -> p k m", k=NKT))
                if k == 4:
                    nc.sync.dma_start(wk8[:, 4:, :], wk8_r[:, 4:, :])
                    nc.sync.dma_start(wq8[:, 4:, :], wq8_r[:, 4:, :])
            nc.gpsimd.dma_start(cosT[:], cos_d[:])
            nc.gpsimd.dma_start(sinT[:], sin_d[:])
            nc.gpsimd.dma_start(maskb[:], mask_d[:])
            nc.gpsimd.dma_start(perm[:], perm_d[:])
            nc.gpsimd.dma_start(ident[:], ident_d[:])
            nc.gpsimd.dma_start(bqk[:], bqk_d[:])
            nc.gpsimd.dma_start(bvc[:], bvc_d[:])
            nc.sync.dma_start(wv8[:], wv8_d[:].rearrange("p (k m) -> p k m", k=NKT))
            nc.sync.dma_start(wve[:], wve_d[:].rearrange("p (k m) -> p k m", k=NKT))
            nc.sync.dma_start(wd[:], wd_d[:].rearrange("p (c o) -> p c o", c=HPC))

            def dr3_step(ps, j, w8, we, hcols, sl, first, last):
                # one k-pair step of a 3-term compensated chain
                kk = slice(2 * j, 2 * j + 2)
                nc.tensor.matmul(ps[:], w8[:, kk, hcols], ht8[:, kk, sl],
                                 start=first, stop=False, perf_mode=DR)
                nc.tensor.matmul(ps[:], w8[:, kk, hcols], hte[:, kk, sl],
                                 start=False, stop=False, perf_mode=DR)
                nc.tensor.matmul(ps[:], we[:, kk, hcols], ht8[:, kk, sl],
                                 start=False, stop=last, perf_mode=DR)

            def qk_proj(h, interleave):
                # qT/kT[h][d=128, s]; bias + 2^-16 descale in the PSUM->SBUF copy
                hcols = slice(h * D, (h + 1) * D)
                chains = []
                for ci in range(NCH):
                    sl = slice(ci * CHUNK, (ci + 1) * CHUNK)
                    for (w8, we, dst, bcol) in ((wk8, wke, kT, 2), (wq8, wqe, qT, 0)):
                        chains.append((w8, we, dst, bcol, sl))
                def chain_pool(c):
                    return (psA, "ps0") if c < 3 else (psB, "ps1") if c < 6 else (psD, "ps3")

                if interleave:
                    # j-major across all 8 chains so the PE rides the ht DMA
                    # staircase; the wres term lags 2 j-steps so the we
                    # planes (DMA'd mid-stream) never stall the pipeline
                    LAG = 1
                    pss = [chain_pool(c)[0].tile([128, CHUNK], f32,
                                                  tag=chain_pool(c)[1],
                                                  name=f"pss{c}")
                           for c in range(8)]
                    for j in range(NKP + LAG):
                        if j < NKP:
                            kk = slice(2 * j, 2 * j + 2)
                            for c, (w8, we, dst, bcol, sl) in enumerate(chains):
                                nc.tensor.matmul(pss[c][:], w8[:, kk, hcols],
                                                 ht8[:, kk, sl], start=(j == 0),
                                                 stop=False, perf_mode=DR)
                            for c, (w8, we, dst, bcol, sl) in enumerate(chains):
                                nc.tensor.matmul(pss[c][:], w8[:, kk, hcols],
                                                 hte[:, kk, sl], start=False,
                                                 stop=False, perf_mode=DR)
                        if j >= LAG:
                            jw = j - LAG
                            kk = slice(2 * jw, 2 * jw + 2)
                            for c, (w8, we, dst, bcol, sl) in enumerate(chains):
                                nc.tensor.matmul(pss[c][:], we[:, kk, hcols],
                                                 ht8[:, kk, sl], start=False,
                                                 stop=(jw == NKP - 1),
                                                 perf_mode=DR)
                    for c, (w8, we, dst, bcol, sl) in enumerate(chains):
                        nc.scalar.activation(dst[h][:, sl], pss[c][:], Ident,
                                             bias=bqk[:, bcol + h:bcol + h + 1],
                                             scale=QK_DESCALE)
                else:
                    for c, (w8, we, dst, bcol, sl) in enumerate(chains):
                        ps = chain_pool(c)[0].tile([128, CHUNK], f32,
                                                   tag=chain_pool(c)[1], name="ps")
                        for j in range(NKP):
                            dr3_step(ps, j, w8, we, hcols, sl, j == 0, j == NKP - 1)
                        nc.scalar.activation(dst[h][:, sl], ps[:], Ident,
                                             bias=bqk[:, bcol + h:bcol + h + 1],
                                             scale=QK_DESCALE)

            def v_proj(st_lo, st_hi):
                # vn[s-part, st, h, d] natural layout, both heads per chain
                for st in range(st_lo, st_hi):
                    ssl = slice(st * 128, (st + 1) * 128)
                    vp, vt = [(psA, "ps0"), (psB, "ps1"), (psD, "ps3")][st % 3]
                    ps = vp.tile([128, HPC * D], f32, tag=vt, name="ps")
                    for j in range(NKP):
                        kk = slice(2 * j, 2 * j + 2)
                        nc.tensor.matmul(ps[:], ht8[:, kk, ssl], wv8[:, kk, :],
                                         start=(j == 0), stop=False, perf_mode=DR)
                        nc.tensor.matmul(ps[:], hte[:, kk, ssl], wv8[:, kk, :],
                                         start=False, stop=False, perf_mode=DR)
                        nc.tensor.matmul(ps[:], ht8[:, kk, ssl], wve[:, kk, :],
                                         start=False, stop=(j == NKP - 1), perf_mode=DR)
                    # vn = 4096*v cast to fp16 (the v-bias is added
                    # per-partition in the post-transpose ctxT copy)
                    nc.scalar.activation(
                        vn[:, st, :, 0:D],
                        ps[:].rearrange("p (c d) -> p c d", c=HPC), Ident)

            def rope(tensors):
                # rows 0..31: t = t*cos + rotate_half(t)*sin; the half-swap
                # runs on the PE as a permutation matmul (no DMA latency).
                # ci-major across tensors so early chunks unblock scores fast
                for ci in range(NCH):
                    sl = slice(ci * CHUNK, (ci + 1) * CHUNK)
                    for ti, t in enumerate(tensors):
                        rps = pools[(2 * ci + ti) % 2].tile(
                            [ROT, CHUNK], f32, tag=f"ps{(2 * ci + ti) % 2}",
                            name="rps")
                        nc.tensor.matmul(rps[:], perm[:], t[0:ROT, sl],
                                         start=True, stop=True)
                        rotu = rop.tile([ROT, CHUNK], fp16, tag="rotu")
                        nc.vector.tensor_tensor(rotu[:], rps[:], sinT[:, sl], MULT)
                        nc.vector.tensor_tensor(t[0:ROT, sl], t[0:ROT, sl], cosT[:, sl], MULT)
                        nc.vector.tensor_tensor(t[0:ROT, sl], t[0:ROT, sl], rotu[:], ADD)

            def scores_t(ci, h, t, prs):
                # one transposed scores tile + exp -> fp16 probs tile
                pool = pools[t % 2]
                pss = pool.tile([128, CHUNK], f32, tag=f"ps{t % 2}", name="pss")
                off = (t - 4 * ci) * 128
                lo = max(off, 0)  # cols i < off never consumed
                nc.tensor.matmul(
                    pss[:, lo:], kT[h][:, t * 128:(t + 1) * 128],
                    qT[h][:, ci * CHUNK + lo:(ci + 1) * CHUNK],
                    start=True, stop=(off < 0))
                if off >= 0:
                    # diagonal tile: add causal mask via I @ maskb
                    nc.tensor.matmul(pss[:, off:off + 128], ident[:], maskb[:],
                                     start=False, stop=True)
                pr = prp.tile([128, CHUNK], fp16, tag="probs")
                nc.scalar.activation(pr[:, lo:], pss[:, lo:], Exp, scale=NORM)
                prs.append(pr)

            def ctx_io(ci, h, io, prs):
                # context + denominator; normalize; transpose back via PE.
                # pc rotates over psA/psB (4 chains in flight) so the DVE
                # reciprocal+scale latency never starves the PE.
                it = 4 * ci + io
                pc = pools[io % 2].tile([128, CHUNK], f32, tag=f"ps{io % 2}",
                                        name=f"pc{io}")
                for t in range(it + 1):
                    nc.tensor.matmul(
                        pc[:, 0:D + 1],
                        prs[t][:, io * 128:(io + 1) * 128],
                        vn[:, t, h, :],
                        start=(t == 0), stop=(t == it))
                rec = rcp.tile([128, 1], f32, tag="rec")
                nc.vector.reciprocal(rec[:], pc[:, D:D + 1])
                cn = cnp.tile([128, D], fp16, tag="ctxn")
                nc.vector.tensor_scalar_mul(cn[:], pc[:, 0:D], rec[:, 0:1])
                pt = psD.tile([128, D], fp16, tag="ps3", name="pt")
                nc.tensor.transpose(pt[:], cn[:], ident[:])
                nc.vector.tensor_scalar(
                    ctxT[h][:, it * 128:(it + 1) * 128], pt[:],
                    bvc[:, h:h + 1], None, op0=ADD)

            def ctx(ci, h, prs):
                for io in range(4):
                    ctx_io(ci, h, io, prs)

            def dense_pieces(ci, on_act=False):
                # 16 oc-chain closures for chunk ci's dense s-tiles; callers
                # interleave them between scores tiles to keep the PE fed
                # while the scalar engine drains the exp backlog
                items = []
                state = {}

                def mk(st, oc):
                    def run():
                        if oc == 0:
                            state[st] = stp.tile([128, HID], fp16, tag="stg",
                                                 name=f"stg{st}")
                        stg = state[st]
                        po = psD.tile([128, CHUNK], f32, tag="ps3", name="po")
                        for c in range(HPC):
                            nc.tensor.matmul(
                                po[:], ctxT[c][:, st * 128:(st + 1) * 128],
                                wd[:, c, oc * CHUNK:(oc + 1) * CHUNK],
                                start=(c == 0), stop=(c == HPC - 1))
                        osl = slice(oc * CHUNK, (oc + 1) * CHUNK)
                        if on_act and oc % 2 == 0:
                            nc.scalar.activation(stg[:, osl], po[:], Ident)
                        else:
                            nc.vector.tensor_copy(stg[:, osl], po[:])
                        if on_act and oc % 2 == 1:
                            hsl = slice((oc - 1) * CHUNK, (oc + 1) * CHUNK)
                            nc.sync.dma_start(
                                out_d[st * 128:(st + 1) * 128, hsl], stg[:, hsl])
                        elif not on_act and oc == NCH - 1:
                            nc.sync.dma_start(
                                out_d[st * 128:(st + 1) * 128, :], stg[:])
                    return run

                for st in range(4 * ci, 4 * ci + 4):
                    for oc in range(NCH):
                        items.append(mk(st, oc))
                return items

            def dense_st(st):
                for item in dense_pieces_for_st(st):
                    item()

            def dense_pieces_for_st(st):
                ci = st // 4
                all_items = dense_pieces(ci)
                return all_items[(st % 4) * NCH:(st % 4 + 1) * NCH]

            # ---- schedule: dense(ci) deferred into chunk ci+1's window so the
            # scalar engine's exp backlog never blocks the PE ----
            qk_proj(0, interleave=True)
            rope([kT[0], qT[0]])
            qk_proj(1, interleave=False)
            rope([kT[1], qT[1]])
            # deferred-dense schedule with cross-window scores hoisting:
            # window ci = [dense(prev)][scores h1][ctx h0][ctx h1 interleaved
            # with the NEXT chunk's scores h0]. ctx h1 is exp-gated on big
            # chunks, so next-chunk scores tiles are free PE filler there.
            order = [0, 1, 2, 3]
            prs = {c: ([], []) for c in order}
            for t in range(4):
                v_proj(2 * t, 2 * t + 2)
                scores_t(0, 0, t, prs[0][0])
            for wi, ci in enumerate(order):
                ntile = 4 * ci + 4
                pr0, pr1 = prs[ci]
                if wi >= 1:
                    dnp = dense_pieces(order[wi - 1])
                    for item in dnp[:4] + dnp[8:] + dnp[4:8]:
                        item()
                for t in range(ntile):
                    if wi == 0:
                        v_proj(8 + 2 * t, 10 + 2 * t)
                    scores_t(ci, 1, t, pr1)
                if wi < len(order) - 1:
                    nxt = order[wi + 1]
                    nt2 = 4 * nxt + 4
                    si = 0
                    for io in range(4):
                        ctx_io(ci, 0, io, pr0)
                        if io >= 2 and si < nt2 // 8:
                            scores_t(nxt, 0, si, prs[nxt][0])
                            si += 1
                    for io in range(4):
                        ctx_io(ci, 1, io, pr1)
                        tgt = ((io + 1) * (io + 2)) * nt2 // 20
                        while si < tgt:
                            scores_t(nxt, 0, si, prs[nxt][0])
                            si += 1
                else:
                    ctx(ci, 0, pr0)
                    # final window: stream each dense s-tile right after its
                    # ctx; copies on the now-empty scalar engine so DVE
                    # stays clear for the recip/scale chain
                    dnl = dense_pieces(ci, on_act=True)
                    for io in range(4):
                        ctx_io(ci, 1, io, pr1)
                        for item in dnl[io * NCH:(io + 1) * NCH]:
                            item()

    nc.compile()
    return nc


def _q8pair(x, scale):
    """Scaled 2-plane e4m3 split: x*scale = hi + lo to ~0.1%."""
    xs = (np.asarray(x, np.float32) * scale).astype(np.float32)
    hi = xs.astype(E4NP)
    lo = (xs - hi.astype(np.float32)).astype(E4NP)
    return hi, lo


def _row_major_128(a, ngroups):
    """[(g p), m] -> [p, (g m)] so DMA runs are >=512B contiguous."""
    g, m = ngroups, a.shape[1]
    return np.ascontiguousarray(
        a.reshape(g, 128, m).transpose(1, 0, 2).reshape(128, g * m))


def _prep_inputs(hidden_states, W_qkv, b_qkv, W_dense, b_dense):
    hid = np.asarray(hidden_states).reshape(S, HID)
    hT = np.ascontiguousarray(hid.T).astype(np.float32)   # [HID, S]
    ht8, hte = _q8pair(hT, SX)
    ht8 = _row_major_128(ht8, NKT)
    hte = _row_major_128(hte, NKT)

    inv_freq = 1.0 / (10000.0 ** (np.arange(0, ROT, 2, dtype=np.float64) / ROT))
    t = np.arange(S, dtype=np.float64)
    freqs = np.outer(t, inv_freq)                      # [s, rot/2]
    emb = np.concatenate([freqs, freqs], axis=1)       # [s, rot]
    cosT = np.ascontiguousarray(np.cos(emb).T).astype(F16NP)
    sinT = np.cos(emb - np.pi / 2).T                   # = sin
    sinTeff = np.concatenate([-sinT[: ROT // 2], sinT[ROT // 2:]], axis=0)
    sinTeff = np.ascontiguousarray(sinTeff).astype(F16NP)

    maskb = np.where(
        np.arange(128)[:, None] > np.arange(128)[None, :], MASK_NEG, 0.0
    ).astype(F16NP)
    ident = np.eye(128, dtype=F16NP)
    # rotate-half permutation: out[r] = t[(r+16) % 32]
    perm = np.zeros((ROT, ROT), F16NP)
    perm[(np.arange(ROT) + ROT // 2) % ROT, np.arange(ROT)] = 1.0

    in_maps = []
    for c in range(NCORES):
        heads = [HPC * c, HPC * c + 1]
        wq = np.concatenate([W_qkv[:, n * 384: n * 384 + 128] for n in heads], 1)
        wk = np.concatenate([W_qkv[:, n * 384 + 128: n * 384 + 256] for n in heads], 1)
        wv = np.concatenate([W_qkv[:, n * 384 + 256: n * 384 + 384] for n in heads], 1)
        wq8, wqe = _q8pair(wq, SWQK)
        wk8, wke = _q8pair(wk, SWQK)
        wv8, wve = _q8pair(wv, SWV)
        bq = np.stack([b_qkv[n * 384: n * 384 + 128] for n in heads], 1)
        bk = np.stack([b_qkv[n * 384 + 128: n * 384 + 256] for n in heads], 1)
        bv = np.concatenate([b_qkv[n * 384 + 256: n * 384 + 384] for n in heads])
        bqk = np.concatenate([bq, bk], axis=1).astype(np.float32)  # [128,4] q0 q1 k0 k1
        bvc = np.stack([bv[0:D], bv[D:2 * D]], 1).astype(np.float32)  # [128, 2]
        wdd = np.asarray(W_dense[c * HPC * D:(c + 1) * HPC * D, :], np.float32)
        in_maps.append({
            "ht8": ht8,
            "hte": hte,
            "wq8": _row_major_128(wq8, NKT),
            "wqe": _row_major_128(wqe, NKT),
            "wk8": _row_major_128(wk8, NKT),
            "wke": _row_major_128(wke, NKT),
            "wv8": _row_major_128(wv8, NKT),
            "wve": _row_major_128(wve, NKT),
            "wd": _row_major_128(wdd.astype(F16NP), HPC),
            "cosT": cosT,
            "sinTeff": sinTeff,
            "maskbias": maskb,
            "ident": ident,
            "perm": perm,
            "bqk": np.ascontiguousarray(bqk),
            "bvc": np.ascontiguousarray(bvc),
        })
    return in_maps


def _reduce(results, inputs):
    partial = np.zeros((S, HID), np.float64)
    for r in results:
        partial += r["partial"].astype(np.float64)
    out = (partial + np.asarray(inputs["b_dense"])[None, :]).astype(np.float32)
    return out.reshape(S, 1, HID)


def _run(inputs, trace=False):
    from concourse.bass_utils import run_bass_kernel_spmd

    if "nc" not in _cache:
        _cache["nc"] = _build_program()
    nc = _cache["nc"]
    in_maps = _prep_inputs(
        inputs["hidden_states"], inputs["W_qkv"], inputs["b_qkv"],
        inputs["W_dense"], inputs["b_dense"],
    )
    res = run_bass_kernel_spmd(nc, in_maps, list(range(NCORES)), trace=trace)
    return _reduce(res.results, inputs), res


def kernel(**inputs):
    out, _ = _run(inputs, trace=False)
    return out
